# revision 28
# baseline (speedup 1.0000x reference)
"""GAT (graph attention) message-passing kernel for Trainium2, 8 NeuronCores.

Strategy (graph/data parallel, dst-sharded):
  - Nodes are partitioned across 8 cores by destination id (12500 each).
  - Edges are sharded by dst partition, sorted by (dst-block, src-subtable),
    and padded so every core runs an identical (SPMD) program.
  - Per step, every core projects ALL nodes (h = x @ [W | W@attn_l]) into an
    fp8 row table in its HBM ([h(256B) | el(4B) | pad] @ 512B stride).  The
    table rows use a permuted layout (node l -> row (l%128)*196 + l//128 per
    25088-row subtable) so an 8-block projection batch stores 8 consecutive
    512B rows per partition with ONE contiguous descriptor per partition;
    lhs loads cover 1024 contiguous xT columns.  This cuts the sync-engine
    (DMA descriptor-gen) time ~8x vs per-block DMAs.
  - Per edge chunk (128 edges), h[src] rows are indirect-gathered
    (gpsimd dma_gather, 4 SWDGE queues round robin).  The per-call idx count
    is a per-core RUNTIME register (value_load from a counts table), so each
    core only transfers its true edges; SPMD padding slots are trimmed.
  - Attention scores: er via host-precomputed transposed one-hot masks
    (fp8, streamed from HBM per superblock) matmul'd with er_sb on TensorE,
    el added on DVE; [softmax-denominator | weighted message sum] accumulate
    into per-dst-block PSUM with mask matmuls.
  - Block epilogue: normalize by the segment sum, head-mean, residual update.
  - Between the 2 conv steps, the updated x (transposed, bf16) is AllGathered
    across the 8 cores in 8-block (4-superblock) chunks.
"""

import os
import math
import numpy as np
import ml_dtypes

import concourse.bass as bass
import concourse.tile as tile
import concourse.mybir as mybir
from concourse import library_config
from concourse.library_overlay import lower_extended_insts
from concourse.bass_utils import run_bass_kernel_spmd

BF16 = mybir.dt.bfloat16
F32 = mybir.dt.float32
F8 = mybir.dt.float8e4
I16 = mybir.dt.int16
I32 = mybir.dt.int32
AF = mybir.ActivationFunctionType
ALU = mybir.AluOpType

NEG_SLOPE = 0.2
STEP = int(os.environ.get("GAT_STEPS", "2"))
SKIP_COLL = bool(int(os.environ.get("GAT_SKIP_COLL", "0")))
SKIP_GATHER = bool(int(os.environ.get("GAT_SKIP_GATHER", "0")))
N_QUEUES = int(os.environ.get("GAT_QUEUES", "4"))
SINGLE_PACKET = bool(int(os.environ.get("GAT_SINGLE_PACKET", "0")))
DYN_CNT = bool(int(os.environ.get("GAT_DYN_CNT", "1")))
N_CORES = 8
SB = 2            # blocks per superblock (PSUM accumulators alive at once)
OCT = 8           # blocks per projection/AllGather batch
MAX_CALL = int(os.environ.get("GAT_MAX_CALL", "8"))  # chunks per dma_gather call
GS = 8            # chunks per elementwise batch group
# With per-core runtime idx counts the padding must be NEGATIVE: the Q7
# ucode trims trailing negative idxs and the decode-side ring reservation
# uses the num_idxs register — both sides then agree on the descriptor
# count.  (Negative pads with a full static register, or zero pads with a
# trimmed register, desync the ring and hang the DMA engines.)
PAD_IDX = -1 if DYN_CNT else 0

_last_results = None  # BassKernelResults stash for test harness


def _bf(x):
    return np.asarray(x, np.float32).astype(ml_dtypes.bfloat16)


def _f8(x):
    return np.asarray(x, np.float32).astype(ml_dtypes.float8_e4m3fn)


# ----------------------------------------------------------------------------
# host-side preprocessing
# ----------------------------------------------------------------------------

def _plan_and_arrays(src, dst, N):
    """Shard/sort/pad edges; build the shared chunk plan and per-core arrays."""
    Nl = N // N_CORES                 # 12500
    NB = (Nl + 127) // 128            # 98
    NBP = NB * 128                    # 12544 padded per-core region
    NSB = (NB + SB - 1) // SB         # 49
    CPST = 2                          # core regions per subtable
    ST_ROWS = CPST * NBP              # 25088 = 196*128
    SRD = ST_ROWS // 128              # 196
    NST = N_CORES // CPST             # 4

    # permuted padded gather-row id for each global src node
    def rowof(s):
        npad = NBP * (s // Nl) + (s % Nl)
        l = npad % ST_ROWS
        return (l % 128) * SRD + l // 128, npad // ST_ROWS

    core = dst // Nl
    percore = []
    for p in range(N_CORES):
        sel = np.nonzero(core == p)[0]
        s = src[sel].astype(np.int64)
        d = (dst[sel] - p * Nl).astype(np.int64)
        blk = d >> 7
        row, st = rowof(s)
        order = np.lexsort((row, st, blk))
        percore.append((row[order], d[order], blk[order], st[order]))

    counts = np.zeros((N_CORES, NB, NST), np.int64)
    for p in range(N_CORES):
        _, _, blk, st = percore[p]
        np.add.at(counts, (p, blk, st), 1)
    nchunks = (counts.max(axis=0) + 127) // 128          # [NB, NST]

    # canonical chunk emission order.  One call per (b, st) run; per-core
    # TRUE idx counts ride in a counts table read into the gather's
    # num_idxs register at runtime: padding below the count is idx 0
    # (transferred, masked out), trailing padding is negative (trimmed).
    chunk_meta = []   # (isb, st, b) per chunk
    calls = []        # (st, chunk_lo, n_chunks, [(b, run_lo, nch_b), ...])
    for isb in range(NSB):
        blocks = list(range(isb * SB, min((isb + 1) * SB, NB)))
        for st in range(NST):
            for b in blocks:
                run_lo = len(chunk_meta)
                for _ in range(int(nchunks[b, st])):
                    chunk_meta.append((isb, st, b))
                n = len(chunk_meta) - run_lo
                o = run_lo
                while n > 0:
                    take = min(n, MAX_CALL)
                    calls.append((st, o, take, [(b, run_lo, take)]))
                    o += take
                    n -= take
    NCH = len(chunk_meta)

    # first/last chunk index per (isb, b) for PSUM start/stop flags
    first = {}
    last = {}
    for ci, (isb, st, b) in enumerate(chunk_meta):
        key = (isb, b)
        if key not in first:
            first[key] = ci
        last[key] = ci

    # per-core edge arrays in padded chunk order + per-call true counts
    idx_all = np.full((N_CORES, NCH * 128), PAD_IDX, np.int16)
    doff_all = np.full((N_CORES, NCH * 128), 255.0, np.float32)
    cnt_all = np.zeros((N_CORES, len(calls)), np.int32)
    for p in range(N_CORES):
        s, d, blk, st = percore[p]
        runs = {}
        i = 0
        M = len(s)
        while i < M:
            k = (blk[i], st[i])
            j = i
            while j < M and blk[j] == k[0] and st[j] == k[1]:
                j += 1
            runs[k] = (i, j)
            i = j
        cursor = {k: v[0] for k, v in runs.items()}
        for ci, (isb, t, b) in enumerate(chunk_meta):
            base = ci * 128
            k = (b, t)
            if k in runs:
                lo = cursor[k]
                hi = min(lo + 128, runs[k][1])
                n = hi - lo
                cursor[k] = hi
                if n > 0:
                    idx_all[p, base:base + n] = s[lo:hi].astype(np.int16)
                    doff_all[p, base:base + n] = (d[lo:hi] - b * 128).astype(np.float32)
        for k, (lo, hi) in runs.items():
            assert cursor[k] == hi, "edge run not fully consumed"
        for ci_call, (t, lo, nch, runs_b) in enumerate(calls):
            cnt = 0
            for (b, run_lo, nch_b) in runs_b:
                c = int(counts[p, b, t])
                if c > 0:
                    cnt = max(cnt, int(np.clip(
                        (run_lo - lo) * 128 + c, 0, nch * 128)))
            cnt_all[p, ci_call] = cnt
            # padding below the runtime count must be >= 0 (transferred,
            # masked); only trailing padding may be negative (trimmed)
            seg = idx_all[p, lo * 128: lo * 128 + cnt]
            seg[seg < 0] = 0

    # gather-call wrapped idx layout: per call [16, n/16], concat on free axis
    idxw_cols = NCH * 8
    idx_wrapped = np.zeros((N_CORES, 16, idxw_cols), np.int16)
    col = 0
    call_cols = []
    for (t, lo, nch, _) in calls:
        n = nch * 128
        for p in range(N_CORES):
            seg = idx_all[p, lo * 128: lo * 128 + n]
            idx_wrapped[p, :, col:col + n // 16] = seg.reshape(-1, 16).T
        call_cols.append(col)
        col += n // 16
    assert col == idxw_cols

    groups = []
    for (t, lo, nch, _) in calls:
        g = lo
        while g < lo + nch:
            take = min(GS, lo + nch - g)
            groups.append((t, lo, g, take))  # (st, call_lo, group_lo, size)
            g += take

    # chunks per superblock (for per-sb mask loads)
    sb_c0 = [None] * NSB
    sb_nch = [0] * NSB
    for ci, (isb, st, b) in enumerate(chunk_meta):
        if sb_c0[isb] is None:
            sb_c0[isb] = ci
        sb_nch[isb] += 1
    max_chsb = max(sb_nch)

    return dict(Nl=Nl, NB=NB, NBP=NBP, NSB=NSB, NST=NST, ST_ROWS=ST_ROWS,
                SRD=SRD, NCH=NCH,
                chunk_meta=chunk_meta, calls=calls, call_cols=call_cols,
                groups=groups, first=first, last=last,
                idx_wrapped=idx_wrapped, doff_raw=doff_all, cnt_all=cnt_all,
                idxw_cols=idxw_cols, sb_c0=sb_c0, sb_nch=sb_nch,
                max_chsb=max_chsb)


# ----------------------------------------------------------------------------
# device program
# ----------------------------------------------------------------------------

def _split_multi_waits(nc):
    """walrus codegen only accepts one sync-wait per instruction; hoist any
    extra waits onto same-engine NOPs inserted right before the instruction."""
    n_id = 0
    for f in nc.m.functions:
        for blk in f.blocks:
            out = []
            for ins in blk.instructions:
                si = ins.sync_info
                if si is not None and len(si.on_wait) > 1 \
                        and ins.engine is not None:
                    waits = list(si.on_wait)
                    for w in waits[:-1]:
                        nop = mybir.InstNoOp(name=f"I-wsplit-{n_id}", ins=[],
                                             outs=[])
                        n_id += 1
                        nop.engine = ins.engine
                        nop.sync_info = mybir.SyncInfo(on_wait=[w],
                                                       on_update=[])
                        nc.inst_map[nop.name] = nop
                        out.append(nop)
                    ins.sync_info = mybir.SyncInfo(on_wait=[waits[-1]],
                                                   on_update=list(si.on_update))
                out.append(ins)
            blk.instructions = out

def _ap(base, *dims):
    """Rebuild AP with the same tensor/offset/partition dim, custom free dims."""
    return bass.AP(base.tensor, base.offset,
                   [list(base.ap[0])] + [list(d) for d in dims])


def _dram_ap(t, offset, pdim, *dims):
    """DRAM AP with custom partition dim and free dims (offset in elems)."""
    base = t.ap()
    return bass.AP(base.tensor, offset,
                   [list(pdim)] + [list(d) for d in dims])


def _build(meta, N, D, H):
    Nl, NB, NBP, NSB, NST = (meta["Nl"], meta["NB"], meta["NBP"], meta["NSB"],
                             meta["NST"])
    ST_ROWS, SRD = meta["ST_ROWS"], meta["SRD"]
    MAXCHSB = meta["max_chsb"]
    NCALLS = len(meta["calls"])
    HD = H * D            # 256
    RW = HD + H           # 260 elems (h | el), fp8 -> 260B used
    TW = 512              # fp8 table row stride: 512B (gather elem size)
    NOCT = (NB + OCT - 1) // OCT      # 13 projection/AG batches
    octs = [(j, min(OCT, NB - OCT * j)) for j in range(NOCT)]

    nc = bass.Bass("TRN2", target_bir_lowering=False, debug=False,
                   enable_asserts=False, num_devices=N_CORES,
                   num_swdge_queues=N_QUEUES,
                   dynamic_dma_scratch_size=32768)

    # ---- DRAM tensors
    xT_in = nc.dram_tensor("xT_in", [D, NBP * N_CORES], BF16,
                           kind="ExternalInput")
    xTl_in = nc.dram_tensor("xTl_in", [D, NBP], BF16, kind="ExternalInput")
    x_in = nc.dram_tensor("x_in", [128, NB, D], F32, kind="ExternalInput")
    c0_in = nc.dram_tensor("c0_in", [128, NB, D], F32, kind="ExternalInput")
    waug_in = nc.dram_tensor("waug_in", [D, RW], BF16, kind="ExternalInput")
    wr_in = nc.dram_tensor("wr_in", [D, H], BF16, kind="ExternalInput")
    ident_in = nc.dram_tensor("ident_in", [128, 128], BF16, kind="ExternalInput")
    scal_in = nc.dram_tensor("scal_in", [128, 4], F32, kind="ExternalInput")
    idx_in = nc.dram_tensor("idx_in", [128, meta["idxw_cols"]], I16,
                            kind="ExternalInput")
    cnt_in = nc.dram_tensor("cnt_in", [128, NCALLS], I32,
                            kind="ExternalInput")
    mt8_in = nc.dram_tensor("mt8_in", [128, meta["NCH"] * 128], F8,
                            kind="ExternalInput")
    m8_in = nc.dram_tensor("m8_in", [128, meta["NCH"] * 128], F8,
                           kind="ExternalInput")

    # double-buffered row table (step-1 projection writes overlap step-0
    # gather reads), split per subtable so gathers start as soon as their
    # subtable's projection slice has landed
    tables = [[nc.dram_tensor("table%d_%d" % (s, t), [ST_ROWS, TW], F8,
                              kind="Internal") for t in range(NST)]
              for s in range(STEP)]
    x_mid = nc.dram_tensor("x_mid", [128, NB, D], F32, kind="Internal")
    # per-octblock xT shards + AllGather outputs (chunked collective so
    # step-1 projection can start as soon as each octblock's AG lands)
    oct_cols = [128 * nblk for (_, nblk) in octs]
    xT_sh = [nc.dram_tensor("xT_sh%d" % k, [D, oct_cols[k]], BF16,
                            kind="Internal") for k in range(NOCT)]
    xT_ag = [nc.dram_tensor("xT_ag%d" % k, [D * N_CORES, oct_cols[k]], BF16,
                            kind="Internal", addr_space="Shared")
             for k in range(NOCT)]
    x_out = nc.dram_tensor("x_out", [Nl, D], F32, kind="ExternalOutput")

    from contextlib import ExitStack
    with tile.TileContext(nc) as tc, ExitStack() as es_:
        nc.gpsimd.load_library(library_config.mlp)
        # per-call runtime gather idx counts cycle through a few dedicated
        # Pool registers (allocated before tile pools exhaust the pool)
        cnt_regs = [nc.gpsimd.alloc_register("gidx%d" % i) for i in range(4)]
        cp = es_.enter_context(tc.tile_pool(name="consts", bufs=1))
        pools = {}
        for nm, bufs in [("xt", 4), ("rows", 8), ("mask", 2), ("m8p", 2),
                         ("rhs", 6), ("sm", 8), ("tbl", 3), ("blk", 6),
                         ("big", 4)]:
            pools[nm] = es_.enter_context(tc.tile_pool(name=nm, bufs=bufs))
        pA = es_.enter_context(tc.tile_pool(name="pacc", bufs=2, space="PSUM"))
        pB = es_.enter_context(tc.tile_pool(name="per8", bufs=1, space="PSUM"))
        pC = es_.enter_context(tc.tile_pool(name="ppj", bufs=3, space="PSUM"))

        # ---- load constants
        ident_t = cp.tile([128, 128], BF16, tag="ident")
        waug_t = cp.tile([D, RW], BF16, tag="waug")
        wr_t = cp.tile([D, H], BF16, tag="wr")
        scal_t = cp.tile([128, 4], F32, tag="scal")
        idx_t = cp.tile([128, meta["idxw_cols"]], I16, tag="idx")
        cnt_t = cp.tile([128, NCALLS], I32, tag="cnt")
        for t, s in [(ident_t, ident_in), (waug_t, waug_in), (wr_t, wr_in),
                     (scal_t, scal_in), (idx_t, idx_in), (cnt_t, cnt_in)]:
            nc.sync.dma_start(t[:], s.ap()[:])

        # zero-init rotating buffers whose stale contents are DMA'd or fed
        # to matmuls before every lane is overwritten (per-core gather trim
        # leaves pad slots stale; tbl junk columns are stored to DRAM)
        for _ in range(8):
            rz = pools["rows"].tile([128, MAX_CALL, TW], F8, tag="rows")
            nc.vector.memset(rz[:], 0)
        for _ in range(3):
            tz = pools["tbl"].tile([128, OCT, TW], F8, tag="tbl")
            nc.vector.memset(tz[:, :, RW:TW], 0)

        tails = {NB - 1: Nl - 128 * (NB - 1)}
        nidx_regs = {}

        def nidx_reg(n):
            if n not in nidx_regs:
                nidx_regs[n] = nc.gpsimd.to_reg(n)
            return nidx_regs[n]

        def proj_oct(step, r, j):
            """Project blocks j*8..j*8+nblk of core region r into the fp8
            row table (batched: 1 lhs load, nblk matmuls, 1 store)."""
            nblk = octs[j][1]
            w = 128 * nblk
            xt = pools["xt"].tile([D, 128 * OCT], BF16, tag="projlhs")
            if step == 0:
                g0 = NBP * r + 128 * OCT * j
                nc.sync.dma_start(xt[:, :w], xT_in.ap()[:, g0:g0 + w])
            else:
                nc.sync.dma_start(xt[:, :w],
                                  xT_ag[j].ap()[D * r:D * (r + 1), :])
            tb = pools["tbl"].tile([128, OCT, TW], F8, tag="tbl")
            for k in range(nblk):
                pp = pC.tile([128, RW], F32, tag="pj")
                nc.tensor.matmul(pp[:], xt[:, 128 * k:128 * (k + 1)],
                                 waug_t[:], start=True, stop=True)
                # in the step-0 prologue the DVE is idle; split the PSUM
                # eviction across scalar+vector so it isn't scalar-bound
                if step == 0 and k % 2 == 1:
                    nc.vector.tensor_copy(tb[:, k, 0:RW], pp[:])
                else:
                    nc.scalar.activation(tb[:, k, 0:RW], pp[:], AF.Copy)
            st_i = r // 2
            rb = NB * (r % 2) + OCT * j
            nc.sync.dma_start(
                _dram_ap(tables[step][st_i], rb * TW, [SRD * TW, 128],
                         [1, nblk * TW]),
                _ap(tb[:], [1, nblk * TW]))

        for step in range(STEP):
            # ---------------------------------------------- step-0 projection
            # (step-1 projection is emitted interleaved into step 0's
            # superblock loop, gated on the per-octblock AllGathers)
            if step == 0:
                for r in range(N_CORES):
                    for j in range(NOCT):
                        proj_oct(0, r, j)

            # ------------------------------------------------ gather + attn
            x_src = x_in if step == 0 else x_mid
            table = tables[step]
            call_i = 0
            group_i = 0
            for isb in range(NSB):
                blocks = list(range(isb * SB, min((isb + 1) * SB, NB)))
                nb = len(blocks)
                b0 = blocks[0]
                oc = isb // 4          # owning octblock (4 sbs per oct)
                oco = 256 * (isb % 4)  # column offset within octblock
                acc = pA.tile([128, SB, 512], F32, tag="acc")
                er8w = pB.tile([128, 512], F32, tag="er8w")
                x4 = pools["blk"].tile([128, SB, D], F32, tag="x4")
                c04 = pools["blk"].tile([128, SB, D], F32, tag="c04")
                nc.sync.dma_start(x4[:, :nb, :], x_src.ap()[:, b0:b0 + nb, :])
                nc.sync.dma_start(c04[:, :nb, :], c0_in.ap()[:, b0:b0 + nb, :])
                # x4p = (1-alpha) * x4 + c0
                x4p = pools["blk"].tile([128, SB, D], F32, tag="x4p")
                nc.vector.scalar_tensor_tensor(
                    x4p[:, :nb, :], x4[:, :nb, :], scal_t[:, 0:1],
                    c04[:, :nb, :], op0=ALU.mult, op1=ALU.add)
                # er_sb: batched lhs load for both blocks of the superblock
                xtb = pools["xt"].tile([D, SB * 128], BF16, tag="erlhs")
                if step == 0:
                    nc.sync.dma_start(
                        xtb[:, :nb * 128],
                        xTl_in.ap()[:, 128 * b0:128 * (b0 + nb)])
                else:
                    nc.sync.dma_start(
                        xtb[:, :nb * 128],
                        xT_sh[oc].ap()[:, oco:oco + nb * 128])
                er_sb = {}
                for j, b in enumerate(blocks):
                    nc.tensor.matmul(acc[:, j, 264:264 + H],
                                     xtb[:, 128 * j:128 * (j + 1)], wr_t[:],
                                     start=True, stop=True)
                    es = pools["sm"].tile([128, H], F8, tag="erblk%d" % j)
                    nc.scalar.activation(es[:], acc[:, j, 264:264 + H], AF.Copy)
                    er_sb[b] = es

                # per-superblock mask streams (one DMA each)
                sb_c0 = meta["sb_c0"][isb]
                chsb = meta["sb_nch"][isb]
                mt = pools["mask"].tile([128, MAXCHSB * 128], F8, tag="mt")
                nc.sync.dma_start(
                    mt[:, :chsb * 128],
                    mt8_in.ap()[:, sb_c0 * 128:(sb_c0 + chsb) * 128])
                m8 = pools["m8p"].tile([128, MAXCHSB, 128], F8, tag="m8")
                nc.sync.dma_start(
                    _ap(m8[:], [128, chsb], [1, 128]),
                    m8_in.ap()[:, sb_c0 * 128:(sb_c0 + chsb) * 128])

                # walk this superblock's calls/groups/chunks
                while call_i < len(meta["calls"]):
                    st, lo, nch, _ = meta["calls"][call_i]
                    if lo >= len(meta["chunk_meta"]) or \
                       meta["chunk_meta"][lo][0] != isb:
                        break
                    n = nch * 128
                    rows = pools["rows"].tile([128, MAX_CALL, TW], F8,
                                              tag="rows")
                    icol = meta["call_cols"][call_i]
                    rows_ap = _ap(rows[:], [TW, nch], [1, TW])
                    tbl_ap = table[st].ap()[:]
                    if not SKIP_GATHER:
                        if DYN_CNT:
                            cv = cnt_regs[call_i % len(cnt_regs)]
                            nc.gpsimd.reg_load(
                                cv, cnt_t[0:1, call_i:call_i + 1])
                        else:
                            cv = nidx_reg(n)
                        nc.gpsimd.dma_gather(
                            rows_ap, tbl_ap, idx_t[:, icol:icol + n // 16],
                            num_idxs=n, num_idxs_reg=cv, elem_size=TW,
                            single_packet=SINGLE_PACKET,
                            queue_num=call_i % N_QUEUES)
                    call_i += 1

                    while group_i < len(meta["groups"]):
                        gst, glo_call, g, gs = meta["groups"][group_i]
                        if glo_call != lo:
                            break
                        group_i += 1
                        cc0 = g - lo   # chunk offset within call
                        kk = g - sb_c0  # chunk offset within superblock
                        # er per edge via fp8 mask matmul, into a
                        # rotating window of the dedicated er8 PSUM bank
                        ew = 32 * (group_i % 16)
                        er8 = er8w[:, ew:ew + GS * H]
                        for k in range(gs):
                            ci = g + k
                            _, _, b = meta["chunk_meta"][ci]
                            nc.tensor.matmul(er8[:, H * k:H * (k + 1)],
                                             mt[:, 128 * (kk + k):
                                                128 * (kk + k + 1)],
                                             er_sb[b], start=True, stop=True,
                                             skip_group_check=True)
                        # t8 = er8 + el (gathered, fp8)
                        t8 = pools["sm"].tile([128, GS * H], BF16, tag="t8")
                        nc.vector.tensor_tensor(
                            t8[:, :gs * H], er8[:, :gs * H],
                            _ap(rows[:, cc0:cc0 + gs, HD:HD + H],
                                [TW, gs], [1, H]),
                            op=ALU.add)
                        lr8 = pools["sm"].tile([128, GS * H], BF16, tag="lr8")
                        nc.scalar.activation(lr8[:, :gs * H], t8[:, :gs * H],
                                             AF.Lrelu, alpha=NEG_SLOPE)
                        rhs8 = pools["rhs"].tile([128, GS, RW], BF16, tag="rhs8")
                        nc.scalar.activation(
                            _ap(rhs8[:], [RW, gs], [1, H]),
                            _ap(lr8[:], [H, gs], [1, H]), AF.Exp)
                        # h is d-major (host permutes waug) so the weight
                        # broadcast is innermost-contiguous over heads
                        nc.vector.tensor_tensor(
                            _ap(rhs8[:, :, H:RW], [RW, gs], [H, D], [1, H]),
                            _ap(rows[:, cc0:cc0 + gs, 0:HD],
                                [TW, gs], [H, D], [1, H]),
                            _ap(rhs8[:], [RW, gs], [0, D], [1, H]),
                            op=ALU.mult)
                        for k in range(gs):
                            ci = g + k
                            _, _, b = meta["chunk_meta"][ci]
                            j = b - b0
                            nc.tensor.matmul(
                                acc[:, j, 0:RW], m8[:, kk + k, :],
                                rhs8[:, k, :],
                                start=(meta["first"][(isb, b)] == ci),
                                stop=(meta["last"][(isb, b)] == ci),
                                skip_group_check=True)

                # ---- superblock epilogue (batched over blocks)
                smax = pools["sm"].tile([128, SB * H], F32, tag="smax")
                nc.vector.tensor_scalar(
                    _ap(smax[:], [H, nb], [1, H]),
                    _ap(acc[:], [512, nb], [1, H]),
                    1e-30, None, op0=ALU.max)
                srec = pools["sm"].tile([128, SB * H], F32, tag="srec")
                nc.vector.reciprocal(srec[:, :nb * H], smax[:, :nb * H])
                srec2 = pools["sm"].tile([128, SB * H], BF16, tag="srec2")
                nc.vector.tensor_scalar(
                    srec2[:, :nb * H], srec[:, :nb * H], scal_t[:, 1:2], None,
                    op0=ALU.mult)
                onb = pools["big"].tile([128, SB, D, H], BF16, tag="onb")
                nc.scalar.activation(
                    _ap(onb[:], [H * D, nb], [1, H * D]),
                    _ap(acc[:, :, H:RW], [512, nb], [1, H * D]), AF.Copy)
                # d-major layout: head index is innermost-contiguous, so the
                # normalize multiply and head reduce run in DVE 2x mode
                onorm = pools["big"].tile([128, SB, D, H], BF16, tag="onorm")
                nc.vector.tensor_tensor(
                    _ap(onorm[:], [H * D, nb], [H, D], [1, H]),
                    _ap(onb[:], [H * D, nb], [H, D], [1, H]),
                    _ap(srec2[:], [H, nb], [0, D], [1, H]),
                    op=ALU.mult)
                red = pools["blk"].tile([128, SB, D], BF16, tag="red")
                with nc.allow_low_precision(reason="4-way head mean in bf16"):
                    nc.vector.tensor_reduce(
                        _ap(red[:], [D, nb], [1, D]),
                        _ap(onorm[:], [H * D, nb], [H, D], [1, H]),
                        axis=mybir.AxisListType.X, op=ALU.add)
                xn = pools["blk"].tile([128, SB, D], F32, tag="xn")
                nc.vector.tensor_add(xn[:, :nb, :], x4p[:, :nb, :],
                                     red[:, :nb, :])
                if step < STEP - 1:
                    xnb = pools["blk"].tile([128, SB, D], BF16, tag="xnb")
                    nc.scalar.activation(xnb[:, :nb, :], xn[:, :nb, :],
                                         AF.Copy)
                    nc.sync.dma_start(x_mid.ap()[:, b0:b0 + nb, :],
                                      xn[:, :nb, :])
                    xts = pools["sm"].tile([D, SB, 128], BF16, tag="xts")
                    for j, b in enumerate(blocks):
                        tp = pC.tile([D, 128], BF16, tag="pj")
                        nc.tensor.transpose(tp[:], xnb[:, j, :], ident_t[:])
                        nc.scalar.activation(xts[:, j, :], tp[:], AF.Copy)
                    nc.sync.dma_start(
                        xT_sh[oc].ap()[:, oco:oco + nb * 128],
                        _ap(xts[:], [1, nb * 128]))
                    # octblock AllGather of the updated xT once its 4 sbs are
                    # done; then emit step-1 projection for a 2-octblock-
                    # earlier AG so engine FIFO heads never stall on an
                    # in-flight collective
                    if isb % 4 == 3 or isb == NSB - 1:
                        if not SKIP_COLL:
                            nc.gpsimd.collective_compute(
                                "AllGather", ALU.bypass,
                                replica_groups=[list(range(N_CORES))],
                                ins=[xT_sh[oc].ap()[:]],
                                outs=[xT_ag[oc].ap()[:]])
                        if oc - 2 >= 0:
                            for r in range(N_CORES):
                                proj_oct(step + 1, r, oc - 2)
                else:
                    for j, b in enumerate(blocks):
                        w = tails.get(b, 128)
                        nc.sync.dma_start(x_out.ap()[128 * b:128 * b + w, :],
                                          xn[:w, j, :])
            assert call_i == len(meta["calls"]) and \
                group_i == len(meta["groups"])

            if step < STEP - 1:
                for k in range(max(0, NOCT - 2), NOCT):
                    for r in range(N_CORES):
                        proj_oct(step + 1, r, k)

    _split_multi_waits(nc)
    lower_extended_insts(nc)
    return nc


# ----------------------------------------------------------------------------
# entry point
# ----------------------------------------------------------------------------

def kernel(x, x0, src, dst, W, attn_l, attn_r, alpha, lamda, **kw):
    global _last_results
    x = np.asarray(x, np.float32)
    x0 = np.asarray(x0, np.float32)
    src = np.asarray(src)
    dst = np.asarray(dst)
    W = np.asarray(W, np.float32)
    attn_l = np.asarray(attn_l, np.float32)
    attn_r = np.asarray(attn_r, np.float32)
    alpha_f = float(np.asarray(alpha))
    lamda_f = float(np.asarray(lamda))

    N, D = x.shape
    H = attn_l.shape[0]
    assert N % N_CORES == 0
    meta = _plan_and_arrays(src, dst, N)
    Nl, NB, NBP = meta["Nl"], meta["NB"], meta["NBP"]

    nc = _build(meta, N, D, H)

    # host-side weight prep.  The projection's h columns are permuted
    # d-major (h index innermost) so on-device head broadcasts/reductions
    # are innermost-contiguous (DVE 2x mode).
    W3 = W.reshape(D, H, D)
    WL = np.einsum("khd,hd->kh", W3, attn_l)
    WR = np.einsum("khd,hd->kh", W3, attn_r)
    W_dm = np.ascontiguousarray(W3.transpose(0, 2, 1)).reshape(D, H * D)
    waug = _bf(np.concatenate([W_dm, WL], axis=1))
    wr = _bf(WR)
    ident = _bf(np.eye(128, dtype=np.float32))
    scal = np.zeros((128, 4), np.float32)
    scal[:, 0] = 1.0 - alpha_f
    scal[:, 1] = alpha_f / H
    c0 = (alpha_f * lamda_f) * x0

    d_idx = np.arange(128, dtype=np.float32)
    # zero-padded per-core-region transposed x: [D, NBP*8]
    xTp = np.zeros((D, NBP * N_CORES), np.float32)
    for r in range(N_CORES):
        xTp[:, NBP * r:NBP * r + Nl] = x[Nl * r:Nl * (r + 1)].T
    xTp = _bf(xTp)
    in_maps = []
    for p in range(N_CORES):
        lo = p * Nl
        xl = np.zeros((NBP, D), np.float32)
        xl[:Nl] = x[lo:lo + Nl]
        c0l = np.zeros((NBP, D), np.float32)
        c0l[:Nl] = c0[lo:lo + Nl]
        # transposed multi-chunk one-hot mask: mt8[d, ci*128+e] =
        # (dst_off(ci, e) == d), fp8 {0,1}
        mt8 = _f8(meta["doff_raw"][p][None, :] == d_idx[:, None])
        # untransposed: m8[e, ci*128+d] = (dst_off(ci, e) == d)
        dd = meta["doff_raw"][p].reshape(-1, 128)
        m8h = _f8((dd[:, :, None] == d_idx[None, None, :])
                  .transpose(1, 0, 2).reshape(128, -1))
        cnt = np.zeros((128, len(meta["calls"])), np.int32)
        cnt[0] = meta["cnt_all"][p]
        in_maps.append({
            "xT_in": np.ascontiguousarray(xTp),
            "xTl_in": np.ascontiguousarray(_bf(xl.T)),
            "x_in": np.ascontiguousarray(
                xl.reshape(NB, 128, D).transpose(1, 0, 2)),
            "c0_in": np.ascontiguousarray(
                c0l.reshape(NB, 128, D).transpose(1, 0, 2)),
            "waug_in": waug, "wr_in": wr,
            "ident_in": ident,
            "scal_in": scal,
            "idx_in": np.ascontiguousarray(
                np.tile(meta["idx_wrapped"][p], (8, 1))),
            "cnt_in": cnt,
            "mt8_in": np.ascontiguousarray(mt8),
            "m8_in": np.ascontiguousarray(m8h),
        })

    trace = bool(int(os.environ.get("GAT_TRACE", "0")))
    res = run_bass_kernel_spmd(nc, in_maps, core_ids=list(range(N_CORES)),
                               trace=trace,
                               trace_cores=[0] if trace else None,
                               stitch_traces=False)
    _last_results = res
    out = np.concatenate([res.results[p]["x_out"] for p in range(N_CORES)],
                         axis=0)
    return out.astype(np.float32)


# revision 31
# speedup vs baseline: 1.0716x; 1.0716x over previous
"""GAT (graph attention) message-passing kernel for Trainium2, 8 NeuronCores.

Strategy (graph/data parallel, dst-sharded):
  - Nodes are partitioned across 8 cores by destination id (12500 each).
  - Edges are sharded by dst partition, sorted by (dst-block, src-subtable),
    and padded so every core runs an identical (SPMD) program.
  - Per step, every core projects ALL nodes (h = x @ [W | W@attn_l]) into an
    fp8 row table in its HBM ([h(256B) | el(4B) | pad] @ 512B stride).  The
    table rows use a permuted layout (node l -> row (l%128)*196 + l//128 per
    25088-row subtable) so an 8-block projection batch stores 8 consecutive
    512B rows per partition with ONE contiguous descriptor per partition;
    lhs loads cover 1024 contiguous xT columns.  This cuts the sync-engine
    (DMA descriptor-gen) time ~8x vs per-block DMAs.
  - Per edge chunk (128 edges), h[src] rows are indirect-gathered
    (gpsimd dma_gather, 4 SWDGE queues round robin).  The per-call idx count
    is a per-core RUNTIME register (value_load from a counts table), so each
    core only transfers its true edges; SPMD padding slots are trimmed.
  - Attention scores: er via host-precomputed transposed one-hot masks
    (fp8, streamed from HBM per superblock) matmul'd with er_sb on TensorE,
    el added on DVE; [softmax-denominator | weighted message sum] accumulate
    into per-dst-block PSUM with mask matmuls.
  - Block epilogue: normalize by the segment sum, head-mean, residual update.
  - Between the 2 conv steps, the updated x (transposed, bf16) is AllGathered
    across the 8 cores in 8-block (4-superblock) chunks.
"""

import os
import math
import numpy as np
import ml_dtypes

import concourse.bass as bass
import concourse.tile as tile
import concourse.mybir as mybir
from concourse import library_config
from concourse.library_overlay import lower_extended_insts
from concourse.bass_utils import run_bass_kernel_spmd

BF16 = mybir.dt.bfloat16
F32 = mybir.dt.float32
F8 = mybir.dt.float8e4
I16 = mybir.dt.int16
I32 = mybir.dt.int32
AF = mybir.ActivationFunctionType
ALU = mybir.AluOpType

NEG_SLOPE = 0.2
STEP = int(os.environ.get("GAT_STEPS", "2"))
SKIP_COLL = bool(int(os.environ.get("GAT_SKIP_COLL", "0")))
SKIP_GATHER = bool(int(os.environ.get("GAT_SKIP_GATHER", "0")))
N_QUEUES = int(os.environ.get("GAT_QUEUES", "4"))
SINGLE_PACKET = bool(int(os.environ.get("GAT_SINGLE_PACKET", "1")))
DYN_CNT = bool(int(os.environ.get("GAT_DYN_CNT", "1")))
N_CORES = 8
SB = 2            # blocks per superblock (PSUM accumulators alive at once)
OCT = 8           # blocks per projection/AllGather batch
MAX_CALL = int(os.environ.get("GAT_MAX_CALL", "8"))  # chunks per dma_gather call
GS = 8            # chunks per elementwise batch group
# With per-core runtime idx counts the padding must be NEGATIVE: the Q7
# ucode trims trailing negative idxs and the decode-side ring reservation
# uses the num_idxs register — both sides then agree on the descriptor
# count.  (Negative pads with a full static register, or zero pads with a
# trimmed register, desync the ring and hang the DMA engines.)
PAD_IDX = -1 if DYN_CNT else 0

_last_results = None  # BassKernelResults stash for test harness


def _bf(x):
    return np.asarray(x, np.float32).astype(ml_dtypes.bfloat16)


def _f8(x):
    return np.asarray(x, np.float32).astype(ml_dtypes.float8_e4m3fn)


# ----------------------------------------------------------------------------
# host-side preprocessing
# ----------------------------------------------------------------------------

def _plan_and_arrays(src, dst, N):
    """Shard/sort/pad edges; build the shared chunk plan and per-core arrays."""
    Nl = N // N_CORES                 # 12500
    NB = (Nl + 127) // 128            # 98
    NBP = NB * 128                    # 12544 padded per-core region
    NSB = (NB + SB - 1) // SB         # 49
    CPST = 2                          # core regions per subtable
    ST_ROWS = CPST * NBP              # 25088 = 196*128
    SRD = ST_ROWS // 128              # 196
    NST = N_CORES // CPST             # 4

    # permuted padded gather-row id for each global src node
    def rowof(s):
        npad = NBP * (s // Nl) + (s % Nl)
        l = npad % ST_ROWS
        return (l % 128) * SRD + l // 128, npad // ST_ROWS

    core = dst // Nl
    percore = []
    for p in range(N_CORES):
        sel = np.nonzero(core == p)[0]
        s = src[sel].astype(np.int64)
        d = (dst[sel] - p * Nl).astype(np.int64)
        blk = d >> 7
        row, st = rowof(s)
        order = np.lexsort((row, st, blk))
        percore.append((row[order], d[order], blk[order], st[order]))

    counts = np.zeros((N_CORES, NB, NST), np.int64)
    for p in range(N_CORES):
        _, _, blk, st = percore[p]
        np.add.at(counts, (p, blk, st), 1)
    nchunks = (counts.max(axis=0) + 127) // 128          # [NB, NST]

    # canonical chunk emission order.  One call per (b, st) run; per-core
    # TRUE idx counts ride in a counts table read into the gather's
    # num_idxs register at runtime: padding below the count is idx 0
    # (transferred, masked out), trailing padding is negative (trimmed).
    chunk_meta = []   # (isb, st, b) per chunk
    calls = []        # (st, chunk_lo, n_chunks, [(b, run_lo, nch_b), ...])
    for isb in range(NSB):
        blocks = list(range(isb * SB, min((isb + 1) * SB, NB)))
        for st in range(NST):
            for b in blocks:
                run_lo = len(chunk_meta)
                for _ in range(int(nchunks[b, st])):
                    chunk_meta.append((isb, st, b))
                n = len(chunk_meta) - run_lo
                o = run_lo
                while n > 0:
                    take = min(n, MAX_CALL)
                    calls.append((st, o, take, [(b, run_lo, take)]))
                    o += take
                    n -= take
    NCH = len(chunk_meta)

    # first/last chunk index per (isb, b) for PSUM start/stop flags
    first = {}
    last = {}
    for ci, (isb, st, b) in enumerate(chunk_meta):
        key = (isb, b)
        if key not in first:
            first[key] = ci
        last[key] = ci

    # per-core edge arrays in padded chunk order + per-call true counts
    idx_all = np.full((N_CORES, NCH * 128), PAD_IDX, np.int16)
    doff_all = np.full((N_CORES, NCH * 128), 255.0, np.float32)
    cnt_all = np.zeros((N_CORES, len(calls)), np.int32)
    for p in range(N_CORES):
        s, d, blk, st = percore[p]
        runs = {}
        i = 0
        M = len(s)
        while i < M:
            k = (blk[i], st[i])
            j = i
            while j < M and blk[j] == k[0] and st[j] == k[1]:
                j += 1
            runs[k] = (i, j)
            i = j
        cursor = {k: v[0] for k, v in runs.items()}
        for ci, (isb, t, b) in enumerate(chunk_meta):
            base = ci * 128
            k = (b, t)
            if k in runs:
                lo = cursor[k]
                hi = min(lo + 128, runs[k][1])
                n = hi - lo
                cursor[k] = hi
                if n > 0:
                    idx_all[p, base:base + n] = s[lo:hi].astype(np.int16)
                    doff_all[p, base:base + n] = (d[lo:hi] - b * 128).astype(np.float32)
        for k, (lo, hi) in runs.items():
            assert cursor[k] == hi, "edge run not fully consumed"
        for ci_call, (t, lo, nch, runs_b) in enumerate(calls):
            cnt = 0
            for (b, run_lo, nch_b) in runs_b:
                c = int(counts[p, b, t])
                if c > 0:
                    cnt = max(cnt, int(np.clip(
                        (run_lo - lo) * 128 + c, 0, nch * 128)))
            cnt_all[p, ci_call] = cnt
            # padding below the runtime count must be >= 0 (transferred,
            # masked); only trailing padding may be negative (trimmed)
            seg = idx_all[p, lo * 128: lo * 128 + cnt]
            seg[seg < 0] = 0

    # gather-call wrapped idx layout: per call [16, n/16], concat on free axis
    idxw_cols = NCH * 8
    idx_wrapped = np.zeros((N_CORES, 16, idxw_cols), np.int16)
    col = 0
    call_cols = []
    for (t, lo, nch, _) in calls:
        n = nch * 128
        for p in range(N_CORES):
            seg = idx_all[p, lo * 128: lo * 128 + n]
            idx_wrapped[p, :, col:col + n // 16] = seg.reshape(-1, 16).T
        call_cols.append(col)
        col += n // 16
    assert col == idxw_cols

    groups = []
    for (t, lo, nch, _) in calls:
        g = lo
        while g < lo + nch:
            take = min(GS, lo + nch - g)
            groups.append((t, lo, g, take))  # (st, call_lo, group_lo, size)
            g += take

    # chunks per superblock (for per-sb mask loads)
    sb_c0 = [None] * NSB
    sb_nch = [0] * NSB
    for ci, (isb, st, b) in enumerate(chunk_meta):
        if sb_c0[isb] is None:
            sb_c0[isb] = ci
        sb_nch[isb] += 1
    max_chsb = max(sb_nch)

    return dict(Nl=Nl, NB=NB, NBP=NBP, NSB=NSB, NST=NST, ST_ROWS=ST_ROWS,
                SRD=SRD, NCH=NCH,
                chunk_meta=chunk_meta, calls=calls, call_cols=call_cols,
                groups=groups, first=first, last=last,
                idx_wrapped=idx_wrapped, doff_raw=doff_all, cnt_all=cnt_all,
                idxw_cols=idxw_cols, sb_c0=sb_c0, sb_nch=sb_nch,
                max_chsb=max_chsb)


# ----------------------------------------------------------------------------
# device program
# ----------------------------------------------------------------------------

def _split_multi_waits(nc):
    """walrus codegen only accepts one sync-wait per instruction; hoist any
    extra waits onto same-engine NOPs inserted right before the instruction."""
    n_id = 0
    for f in nc.m.functions:
        for blk in f.blocks:
            out = []
            for ins in blk.instructions:
                si = ins.sync_info
                if si is not None and len(si.on_wait) > 1 \
                        and ins.engine is not None:
                    waits = list(si.on_wait)
                    for w in waits[:-1]:
                        nop = mybir.InstNoOp(name=f"I-wsplit-{n_id}", ins=[],
                                             outs=[])
                        n_id += 1
                        nop.engine = ins.engine
                        nop.sync_info = mybir.SyncInfo(on_wait=[w],
                                                       on_update=[])
                        nc.inst_map[nop.name] = nop
                        out.append(nop)
                    ins.sync_info = mybir.SyncInfo(on_wait=[waits[-1]],
                                                   on_update=list(si.on_update))
                out.append(ins)
            blk.instructions = out

def _ap(base, *dims):
    """Rebuild AP with the same tensor/offset/partition dim, custom free dims."""
    return bass.AP(base.tensor, base.offset,
                   [list(base.ap[0])] + [list(d) for d in dims])


def _dram_ap(t, offset, pdim, *dims):
    """DRAM AP with custom partition dim and free dims (offset in elems)."""
    base = t.ap()
    return bass.AP(base.tensor, offset,
                   [list(pdim)] + [list(d) for d in dims])


def _build(meta, N, D, H):
    Nl, NB, NBP, NSB, NST = (meta["Nl"], meta["NB"], meta["NBP"], meta["NSB"],
                             meta["NST"])
    ST_ROWS, SRD = meta["ST_ROWS"], meta["SRD"]
    MAXCHSB = meta["max_chsb"]
    NCALLS = len(meta["calls"])
    HD = H * D            # 256
    RW = HD + H           # 260 elems (h | el), fp8 -> 260B used
    TW = 512              # fp8 table row stride: 512B (gather elem size)
    NOCT = (NB + OCT - 1) // OCT      # 13 projection/AG batches
    octs = [(j, min(OCT, NB - OCT * j)) for j in range(NOCT)]

    nc = bass.Bass("TRN2", target_bir_lowering=False, debug=False,
                   enable_asserts=False, num_devices=N_CORES,
                   num_swdge_queues=N_QUEUES,
                   dynamic_dma_scratch_size=32768)

    # ---- DRAM tensors
    xT_in = nc.dram_tensor("xT_in", [D, NBP * N_CORES], BF16,
                           kind="ExternalInput")
    xTl_in = nc.dram_tensor("xTl_in", [D, NBP], BF16, kind="ExternalInput")
    x_in = nc.dram_tensor("x_in", [128, NB, D], F32, kind="ExternalInput")
    c0_in = nc.dram_tensor("c0_in", [128, NB, D], F32, kind="ExternalInput")
    waug_in = nc.dram_tensor("waug_in", [D, RW], BF16, kind="ExternalInput")
    wr_in = nc.dram_tensor("wr_in", [D, H], BF16, kind="ExternalInput")
    ident_in = nc.dram_tensor("ident_in", [128, 128], BF16, kind="ExternalInput")
    scal_in = nc.dram_tensor("scal_in", [128, 4], F32, kind="ExternalInput")
    idx_in = nc.dram_tensor("idx_in", [128, meta["idxw_cols"]], I16,
                            kind="ExternalInput")
    cnt_in = nc.dram_tensor("cnt_in", [128, NCALLS], I32,
                            kind="ExternalInput")
    mt8_in = nc.dram_tensor("mt8_in", [128, meta["NCH"] * 128], F8,
                            kind="ExternalInput")
    m8_in = nc.dram_tensor("m8_in", [128, meta["NCH"] * 128], F8,
                           kind="ExternalInput")

    # double-buffered row table (step-1 projection writes overlap step-0
    # gather reads), split per subtable so gathers start as soon as their
    # subtable's projection slice has landed
    tables = [[nc.dram_tensor("table%d_%d" % (s, t), [ST_ROWS, TW], F8,
                              kind="Internal") for t in range(NST)]
              for s in range(STEP)]
    x_mid = nc.dram_tensor("x_mid", [128, NB, D], F32, kind="Internal")
    # per-octblock xT shards + AllGather outputs (chunked collective so
    # step-1 projection can start as soon as each octblock's AG lands)
    oct_cols = [128 * nblk for (_, nblk) in octs]
    xT_sh = [nc.dram_tensor("xT_sh%d" % k, [D, oct_cols[k]], BF16,
                            kind="Internal") for k in range(NOCT)]
    xT_ag = [nc.dram_tensor("xT_ag%d" % k, [D * N_CORES, oct_cols[k]], BF16,
                            kind="Internal", addr_space="Shared")
             for k in range(NOCT)]
    x_out = nc.dram_tensor("x_out", [Nl, D], F32, kind="ExternalOutput")

    from contextlib import ExitStack
    with tile.TileContext(nc) as tc, ExitStack() as es_:
        nc.gpsimd.load_library(library_config.mlp)
        # per-call runtime gather idx counts cycle through a few dedicated
        # Pool registers (allocated before tile pools exhaust the pool)
        cnt_regs = [nc.gpsimd.alloc_register("gidx%d" % i) for i in range(4)]
        cp = es_.enter_context(tc.tile_pool(name="consts", bufs=1))
        pools = {}
        for nm, bufs in [("xt", 4), ("rows", 8), ("mask", 2), ("m8p", 2),
                         ("rhs", 6), ("sm", 8), ("tbl", 3), ("blk", 6),
                         ("big", 4)]:
            pools[nm] = es_.enter_context(tc.tile_pool(name=nm, bufs=bufs))
        pA = es_.enter_context(tc.tile_pool(name="pacc", bufs=2, space="PSUM"))
        pB = es_.enter_context(tc.tile_pool(name="per8", bufs=1, space="PSUM"))
        pC = es_.enter_context(tc.tile_pool(name="ppj", bufs=3, space="PSUM"))

        # ---- load constants
        ident_t = cp.tile([128, 128], BF16, tag="ident")
        waug_t = cp.tile([D, RW], BF16, tag="waug")
        wr_t = cp.tile([D, H], BF16, tag="wr")
        scal_t = cp.tile([128, 4], F32, tag="scal")
        idx_t = cp.tile([128, meta["idxw_cols"]], I16, tag="idx")
        cnt_t = cp.tile([128, NCALLS], I32, tag="cnt")
        for t, s in [(ident_t, ident_in), (waug_t, waug_in), (wr_t, wr_in),
                     (scal_t, scal_in), (idx_t, idx_in), (cnt_t, cnt_in)]:
            nc.sync.dma_start(t[:], s.ap()[:])

        # zero-init rotating buffers whose stale contents are DMA'd or fed
        # to matmuls before every lane is overwritten (per-core gather trim
        # leaves pad slots stale; tbl junk columns are stored to DRAM)
        for _ in range(8):
            rz = pools["rows"].tile([128, MAX_CALL, TW], F8, tag="rows")
            nc.vector.memset(rz[:], 0)
        for _ in range(3):
            tz = pools["tbl"].tile([128, OCT, TW], F8, tag="tbl")
            nc.vector.memset(tz[:, :, RW:TW], 0)

        tails = {NB - 1: Nl - 128 * (NB - 1)}
        nidx_regs = {}

        def nidx_reg(n):
            if n not in nidx_regs:
                nidx_regs[n] = nc.gpsimd.to_reg(n)
            return nidx_regs[n]

        def proj_oct(step, r, j):
            """Project blocks j*8..j*8+nblk of core region r into the fp8
            row table (batched: 1 lhs load, nblk matmuls, 1 store)."""
            nblk = octs[j][1]
            w = 128 * nblk
            xt = pools["xt"].tile([D, 128 * OCT], BF16, tag="projlhs")
            if step == 0:
                g0 = NBP * r + 128 * OCT * j
                nc.sync.dma_start(xt[:, :w], xT_in.ap()[:, g0:g0 + w])
            else:
                nc.sync.dma_start(xt[:, :w],
                                  xT_ag[j].ap()[D * r:D * (r + 1), :])
            tb = pools["tbl"].tile([128, OCT, TW], F8, tag="tbl")
            for k in range(nblk):
                pp = pC.tile([128, RW], F32, tag="pj")
                nc.tensor.matmul(pp[:], xt[:, 128 * k:128 * (k + 1)],
                                 waug_t[:], start=True, stop=True)
                # in the step-0 prologue the DVE is idle; split the PSUM
                # eviction across scalar+vector so it isn't scalar-bound
                if step == 0 and k % 2 == 1:
                    nc.vector.tensor_copy(tb[:, k, 0:RW], pp[:])
                else:
                    nc.scalar.activation(tb[:, k, 0:RW], pp[:], AF.Copy)
            st_i = r // 2
            rb = NB * (r % 2) + OCT * j
            nc.sync.dma_start(
                _dram_ap(tables[step][st_i], rb * TW, [SRD * TW, 128],
                         [1, nblk * TW]),
                _ap(tb[:], [1, nblk * TW]))

        for step in range(STEP):
            # ---------------------------------------------- step-0 projection
            # (step-1 projection is emitted interleaved into step 0's
            # superblock loop, gated on the per-octblock AllGathers)
            if step == 0:
                for r in range(N_CORES):
                    for j in range(NOCT):
                        proj_oct(0, r, j)

            # ------------------------------------------------ gather + attn
            x_src = x_in if step == 0 else x_mid
            table = tables[step]
            call_i = 0
            group_i = 0
            for isb in range(NSB):
                blocks = list(range(isb * SB, min((isb + 1) * SB, NB)))
                nb = len(blocks)
                b0 = blocks[0]
                oc = isb // 4          # owning octblock (4 sbs per oct)
                oco = 256 * (isb % 4)  # column offset within octblock
                acc = pA.tile([128, SB, 512], F32, tag="acc")
                er8w = pB.tile([128, 512], F32, tag="er8w")
                x4 = pools["blk"].tile([128, SB, D], F32, tag="x4")
                c04 = pools["blk"].tile([128, SB, D], F32, tag="c04")
                nc.sync.dma_start(x4[:, :nb, :], x_src.ap()[:, b0:b0 + nb, :])
                nc.sync.dma_start(c04[:, :nb, :], c0_in.ap()[:, b0:b0 + nb, :])
                # x4p = (1-alpha) * x4 + c0
                x4p = pools["blk"].tile([128, SB, D], F32, tag="x4p")
                nc.vector.scalar_tensor_tensor(
                    x4p[:, :nb, :], x4[:, :nb, :], scal_t[:, 0:1],
                    c04[:, :nb, :], op0=ALU.mult, op1=ALU.add)
                # er_sb: batched lhs load for both blocks of the superblock
                xtb = pools["xt"].tile([D, SB * 128], BF16, tag="erlhs")
                if step == 0:
                    nc.sync.dma_start(
                        xtb[:, :nb * 128],
                        xTl_in.ap()[:, 128 * b0:128 * (b0 + nb)])
                else:
                    nc.sync.dma_start(
                        xtb[:, :nb * 128],
                        xT_sh[oc].ap()[:, oco:oco + nb * 128])
                er_sb = {}
                for j, b in enumerate(blocks):
                    nc.tensor.matmul(acc[:, j, 264:264 + H],
                                     xtb[:, 128 * j:128 * (j + 1)], wr_t[:],
                                     start=True, stop=True)
                    es = pools["sm"].tile([128, H], F8, tag="erblk%d" % j)
                    nc.scalar.activation(es[:], acc[:, j, 264:264 + H], AF.Copy)
                    er_sb[b] = es

                # per-superblock mask streams (one DMA each)
                sb_c0 = meta["sb_c0"][isb]
                chsb = meta["sb_nch"][isb]
                mt = pools["mask"].tile([128, MAXCHSB * 128], F8, tag="mt")
                nc.sync.dma_start(
                    mt[:, :chsb * 128],
                    mt8_in.ap()[:, sb_c0 * 128:(sb_c0 + chsb) * 128])
                m8 = pools["m8p"].tile([128, MAXCHSB, 128], F8, tag="m8")
                nc.sync.dma_start(
                    _ap(m8[:], [128, chsb], [1, 128]),
                    m8_in.ap()[:, sb_c0 * 128:(sb_c0 + chsb) * 128])

                # walk this superblock's calls/groups/chunks
                while call_i < len(meta["calls"]):
                    st, lo, nch, _ = meta["calls"][call_i]
                    if lo >= len(meta["chunk_meta"]) or \
                       meta["chunk_meta"][lo][0] != isb:
                        break
                    n = nch * 128
                    rows = pools["rows"].tile([128, MAX_CALL, TW], F8,
                                              tag="rows")
                    icol = meta["call_cols"][call_i]
                    rows_ap = _ap(rows[:], [TW, nch], [1, TW])
                    tbl_ap = table[st].ap()[:]
                    if not SKIP_GATHER:
                        if DYN_CNT:
                            cv = cnt_regs[call_i % len(cnt_regs)]
                            nc.gpsimd.reg_load(
                                cv, cnt_t[0:1, call_i:call_i + 1])
                        else:
                            cv = nidx_reg(n)
                        nc.gpsimd.dma_gather(
                            rows_ap, tbl_ap, idx_t[:, icol:icol + n // 16],
                            num_idxs=n, num_idxs_reg=cv, elem_size=TW,
                            single_packet=SINGLE_PACKET,
                            queue_num=call_i % N_QUEUES)
                    call_i += 1

                    while group_i < len(meta["groups"]):
                        gst, glo_call, g, gs = meta["groups"][group_i]
                        if glo_call != lo:
                            break
                        group_i += 1
                        cc0 = g - lo   # chunk offset within call
                        kk = g - sb_c0  # chunk offset within superblock
                        # er per edge via fp8 mask matmul, into a
                        # rotating window of the dedicated er8 PSUM bank
                        ew = 32 * (group_i % 16)
                        er8 = er8w[:, ew:ew + GS * H]
                        for k in range(gs):
                            ci = g + k
                            _, _, b = meta["chunk_meta"][ci]
                            nc.tensor.matmul(er8[:, H * k:H * (k + 1)],
                                             mt[:, 128 * (kk + k):
                                                128 * (kk + k + 1)],
                                             er_sb[b], start=True, stop=True,
                                             skip_group_check=True)
                        # t8 = er8 + el (gathered, fp8)
                        t8 = pools["sm"].tile([128, GS * H], BF16, tag="t8")
                        nc.vector.tensor_tensor(
                            t8[:, :gs * H], er8[:, :gs * H],
                            _ap(rows[:, cc0:cc0 + gs, HD:HD + H],
                                [TW, gs], [1, H]),
                            op=ALU.add)
                        lr8 = pools["sm"].tile([128, GS * H], BF16, tag="lr8")
                        nc.vector.scalar_tensor_tensor(
                            lr8[:, :gs * H], t8[:, :gs * H], NEG_SLOPE,
                            t8[:, :gs * H], op0=ALU.mult, op1=ALU.max)
                        rhs8 = pools["rhs"].tile([128, GS, RW], BF16, tag="rhs8")
                        nc.scalar.activation(
                            _ap(rhs8[:], [RW, gs], [1, H]),
                            _ap(lr8[:], [H, gs], [1, H]), AF.Exp)
                        # h is d-major (host permutes waug) so the weight
                        # broadcast is innermost-contiguous over heads
                        nc.vector.tensor_tensor(
                            _ap(rhs8[:, :, H:RW], [RW, gs], [H, D], [1, H]),
                            _ap(rows[:, cc0:cc0 + gs, 0:HD],
                                [TW, gs], [H, D], [1, H]),
                            _ap(rhs8[:], [RW, gs], [0, D], [1, H]),
                            op=ALU.mult)
                        for k in range(gs):
                            ci = g + k
                            _, _, b = meta["chunk_meta"][ci]
                            j = b - b0
                            nc.tensor.matmul(
                                acc[:, j, 0:RW], m8[:, kk + k, :],
                                rhs8[:, k, :],
                                start=(meta["first"][(isb, b)] == ci),
                                stop=(meta["last"][(isb, b)] == ci),
                                skip_group_check=True)

                # ---- superblock epilogue (batched over blocks)
                smax = pools["sm"].tile([128, SB * H], F32, tag="smax")
                nc.vector.tensor_scalar(
                    _ap(smax[:], [H, nb], [1, H]),
                    _ap(acc[:], [512, nb], [1, H]),
                    1e-30, None, op0=ALU.max)
                srec = pools["sm"].tile([128, SB * H], F32, tag="srec")
                nc.vector.reciprocal(srec[:, :nb * H], smax[:, :nb * H])
                srec2 = pools["sm"].tile([128, SB * H], BF16, tag="srec2")
                nc.vector.tensor_scalar(
                    srec2[:, :nb * H], srec[:, :nb * H], scal_t[:, 1:2], None,
                    op0=ALU.mult)
                onb = pools["big"].tile([128, SB, D, H], BF16, tag="onb")
                nc.scalar.activation(
                    _ap(onb[:], [H * D, nb], [1, H * D]),
                    _ap(acc[:, :, H:RW], [512, nb], [1, H * D]), AF.Copy)
                # d-major layout: head index is innermost-contiguous, so the
                # normalize multiply and head reduce run in DVE 2x mode
                onorm = pools["big"].tile([128, SB, D, H], BF16, tag="onorm")
                nc.vector.tensor_tensor(
                    _ap(onorm[:], [H * D, nb], [H, D], [1, H]),
                    _ap(onb[:], [H * D, nb], [H, D], [1, H]),
                    _ap(srec2[:], [H, nb], [0, D], [1, H]),
                    op=ALU.mult)
                red = pools["blk"].tile([128, SB, D], BF16, tag="red")
                with nc.allow_low_precision(reason="4-way head mean in bf16"):
                    nc.vector.tensor_reduce(
                        _ap(red[:], [D, nb], [1, D]),
                        _ap(onorm[:], [H * D, nb], [H, D], [1, H]),
                        axis=mybir.AxisListType.X, op=ALU.add)
                xn = pools["blk"].tile([128, SB, D], F32, tag="xn")
                nc.vector.tensor_add(xn[:, :nb, :], x4p[:, :nb, :],
                                     red[:, :nb, :])
                if step < STEP - 1:
                    xnb = pools["blk"].tile([128, SB, D], BF16, tag="xnb")
                    nc.vector.tensor_copy(xnb[:, :nb, :], xn[:, :nb, :])
                    nc.sync.dma_start(x_mid.ap()[:, b0:b0 + nb, :],
                                      xn[:, :nb, :])
                    xts = pools["sm"].tile([D, SB, 128], BF16, tag="xts")
                    for j, b in enumerate(blocks):
                        tp = pC.tile([D, 128], BF16, tag="pj")
                        nc.tensor.transpose(tp[:], xnb[:, j, :], ident_t[:])
                        nc.scalar.activation(xts[:, j, :], tp[:], AF.Copy)
                    nc.sync.dma_start(
                        xT_sh[oc].ap()[:, oco:oco + nb * 128],
                        _ap(xts[:], [1, nb * 128]))
                    # octblock AllGather of the updated xT once its 4 sbs are
                    # done; then emit step-1 projection for a 2-octblock-
                    # earlier AG so engine FIFO heads never stall on an
                    # in-flight collective
                    if isb % 4 == 3 or isb == NSB - 1:
                        if not SKIP_COLL:
                            nc.gpsimd.collective_compute(
                                "AllGather", ALU.bypass,
                                replica_groups=[list(range(N_CORES))],
                                ins=[xT_sh[oc].ap()[:]],
                                outs=[xT_ag[oc].ap()[:]])
                        if oc - 2 >= 0:
                            for r in range(N_CORES):
                                proj_oct(step + 1, r, oc - 2)
                else:
                    for j, b in enumerate(blocks):
                        w = tails.get(b, 128)
                        nc.sync.dma_start(x_out.ap()[128 * b:128 * b + w, :],
                                          xn[:w, j, :])
            assert call_i == len(meta["calls"]) and \
                group_i == len(meta["groups"])

            if step < STEP - 1:
                for k in range(max(0, NOCT - 2), NOCT):
                    for r in range(N_CORES):
                        proj_oct(step + 1, r, k)

    _split_multi_waits(nc)
    lower_extended_insts(nc)
    return nc


# ----------------------------------------------------------------------------
# entry point
# ----------------------------------------------------------------------------

def kernel(x, x0, src, dst, W, attn_l, attn_r, alpha, lamda, **kw):
    global _last_results
    x = np.asarray(x, np.float32)
    x0 = np.asarray(x0, np.float32)
    src = np.asarray(src)
    dst = np.asarray(dst)
    W = np.asarray(W, np.float32)
    attn_l = np.asarray(attn_l, np.float32)
    attn_r = np.asarray(attn_r, np.float32)
    alpha_f = float(np.asarray(alpha))
    lamda_f = float(np.asarray(lamda))

    N, D = x.shape
    H = attn_l.shape[0]
    assert N % N_CORES == 0
    meta = _plan_and_arrays(src, dst, N)
    Nl, NB, NBP = meta["Nl"], meta["NB"], meta["NBP"]

    nc = _build(meta, N, D, H)

    # host-side weight prep.  The projection's h columns are permuted
    # d-major (h index innermost) so on-device head broadcasts/reductions
    # are innermost-contiguous (DVE 2x mode).
    W3 = W.reshape(D, H, D)
    WL = np.einsum("khd,hd->kh", W3, attn_l)
    WR = np.einsum("khd,hd->kh", W3, attn_r)
    W_dm = np.ascontiguousarray(W3.transpose(0, 2, 1)).reshape(D, H * D)
    waug = _bf(np.concatenate([W_dm, WL], axis=1))
    wr = _bf(WR)
    ident = _bf(np.eye(128, dtype=np.float32))
    scal = np.zeros((128, 4), np.float32)
    scal[:, 0] = 1.0 - alpha_f
    scal[:, 1] = alpha_f / H
    c0 = (alpha_f * lamda_f) * x0

    d_idx = np.arange(128, dtype=np.float32)
    # zero-padded per-core-region transposed x: [D, NBP*8]
    xTp = np.zeros((D, NBP * N_CORES), np.float32)
    for r in range(N_CORES):
        xTp[:, NBP * r:NBP * r + Nl] = x[Nl * r:Nl * (r + 1)].T
    xTp = _bf(xTp)
    in_maps = []
    for p in range(N_CORES):
        lo = p * Nl
        xl = np.zeros((NBP, D), np.float32)
        xl[:Nl] = x[lo:lo + Nl]
        c0l = np.zeros((NBP, D), np.float32)
        c0l[:Nl] = c0[lo:lo + Nl]
        # transposed multi-chunk one-hot mask: mt8[d, ci*128+e] =
        # (dst_off(ci, e) == d), fp8 {0,1}
        mt8 = _f8(meta["doff_raw"][p][None, :] == d_idx[:, None])
        # untransposed: m8[e, ci*128+d] = (dst_off(ci, e) == d)
        dd = meta["doff_raw"][p].reshape(-1, 128)
        m8h = _f8((dd[:, :, None] == d_idx[None, None, :])
                  .transpose(1, 0, 2).reshape(128, -1))
        cnt = np.zeros((128, len(meta["calls"])), np.int32)
        cnt[0] = meta["cnt_all"][p]
        in_maps.append({
            "xT_in": np.ascontiguousarray(xTp),
            "xTl_in": np.ascontiguousarray(_bf(xl.T)),
            "x_in": np.ascontiguousarray(
                xl.reshape(NB, 128, D).transpose(1, 0, 2)),
            "c0_in": np.ascontiguousarray(
                c0l.reshape(NB, 128, D).transpose(1, 0, 2)),
            "waug_in": waug, "wr_in": wr,
            "ident_in": ident,
            "scal_in": scal,
            "idx_in": np.ascontiguousarray(
                np.tile(meta["idx_wrapped"][p], (8, 1))),
            "cnt_in": cnt,
            "mt8_in": np.ascontiguousarray(mt8),
            "m8_in": np.ascontiguousarray(m8h),
        })

    trace = bool(int(os.environ.get("GAT_TRACE", "0")))
    res = run_bass_kernel_spmd(nc, in_maps, core_ids=list(range(N_CORES)),
                               trace=trace,
                               trace_cores=[0] if trace else None,
                               stitch_traces=False)
    _last_results = res
    out = np.concatenate([res.results[p]["x_out"] for p in range(N_CORES)],
                         axis=0)
    return out.astype(np.float32)


# revision 38
# speedup vs baseline: 1.2897x; 1.2035x over previous
"""GAT (graph attention) message-passing kernel for Trainium2, 8 NeuronCores.

Strategy (graph/data parallel, dst-sharded):
  - Nodes are partitioned across 8 cores by destination id (12500 each).
  - Edges are sharded by dst partition, sorted by (dst-block, src-subtable),
    and padded so every core runs an identical (SPMD) program.
  - Per step, every core projects ALL nodes (h = x @ [W | W@attn_l]) into an
    fp8 row table in its HBM ([h(256B) | el(4B) | pad] @ 512B stride).  The
    table rows use a permuted layout (node l -> row (l%128)*196 + l//128 per
    25088-row subtable) so an 8-block projection batch stores 8 consecutive
    512B rows per partition with ONE contiguous descriptor per partition;
    lhs loads cover 1024 contiguous xT columns.  This cuts the sync-engine
    (DMA descriptor-gen) time ~8x vs per-block DMAs.
  - Per edge chunk (128 edges), h[src] rows are indirect-gathered
    (gpsimd dma_gather, 4 SWDGE queues round robin).  The per-call idx count
    is a per-core RUNTIME register (value_load from a counts table), so each
    core only transfers its true edges; SPMD padding slots are trimmed.
  - Attention scores: er via host-precomputed transposed one-hot masks
    (fp8, streamed from HBM per superblock) matmul'd with er_sb on TensorE,
    el added on DVE; [softmax-denominator | weighted message sum] accumulate
    into per-dst-block PSUM with mask matmuls.
  - Block epilogue: normalize by the segment sum, head-mean, residual update.
  - Between the 2 conv steps, the updated x (transposed, bf16) is AllGathered
    across the 8 cores in 8-block (4-superblock) chunks.
"""

import os
import math
import numpy as np
import ml_dtypes

import concourse.bass as bass
import concourse.tile as tile
import concourse.mybir as mybir
from concourse import library_config
from concourse.library_overlay import lower_extended_insts
from concourse.bass_utils import run_bass_kernel_spmd

BF16 = mybir.dt.bfloat16
F32 = mybir.dt.float32
F8 = mybir.dt.float8e4
I16 = mybir.dt.int16
I32 = mybir.dt.int32
AF = mybir.ActivationFunctionType
ALU = mybir.AluOpType

NEG_SLOPE = 0.2
STEP = int(os.environ.get("GAT_STEPS", "2"))
SKIP_COLL = bool(int(os.environ.get("GAT_SKIP_COLL", "0")))
SKIP_GATHER = bool(int(os.environ.get("GAT_SKIP_GATHER", "0")))
N_QUEUES = int(os.environ.get("GAT_QUEUES", "4"))
SINGLE_PACKET = bool(int(os.environ.get("GAT_SINGLE_PACKET", "1")))
DYN_CNT = bool(int(os.environ.get("GAT_DYN_CNT", "1")))
N_CORES = 8
SB = 2            # blocks per superblock (PSUM accumulators alive at once)
OCT = 8           # blocks per projection/AllGather batch
MAX_CALL = int(os.environ.get("GAT_MAX_CALL", "8"))  # chunks per dma_gather call
GS = 8            # chunks per elementwise batch group
# With per-core runtime idx counts the padding must be NEGATIVE: the Q7
# ucode trims trailing negative idxs and the decode-side ring reservation
# uses the num_idxs register — both sides then agree on the descriptor
# count.  (Negative pads with a full static register, or zero pads with a
# trimmed register, desync the ring and hang the DMA engines.)
PAD_IDX = -1 if DYN_CNT else 0

_last_results = None  # BassKernelResults stash for test harness


def _bf(x):
    return np.asarray(x, np.float32).astype(ml_dtypes.bfloat16)


def _f8(x):
    return np.asarray(x, np.float32).astype(ml_dtypes.float8_e4m3fn)


# ----------------------------------------------------------------------------
# host-side preprocessing
# ----------------------------------------------------------------------------

def _plan_and_arrays(src, dst, N):
    """Shard/sort/pad edges; build the shared chunk plan and per-core arrays."""
    Nl = N // N_CORES                 # 12500
    NB = (Nl + 127) // 128            # 98
    NBP = NB * 128                    # 12544 padded per-core region
    NSB = (NB + SB - 1) // SB         # 49
    CPST = 2                          # core regions per subtable
    ST_ROWS = CPST * NBP              # 25088 = 196*128
    SRD = ST_ROWS // 128              # 196
    NST = N_CORES // CPST             # 4

    # permuted padded gather-row id for each global src node
    def rowof(s):
        npad = NBP * (s // Nl) + (s % Nl)
        l = npad % ST_ROWS
        return (l % 128) * SRD + l // 128, npad // ST_ROWS

    core = dst // Nl
    percore = []
    for p in range(N_CORES):
        sel = np.nonzero(core == p)[0]
        s = src[sel].astype(np.int64)
        d = (dst[sel] - p * Nl).astype(np.int64)
        blk = d >> 7
        row, st = rowof(s)
        order = np.lexsort((row, st, blk))
        percore.append((row[order], d[order], blk[order], st[order]))

    counts = np.zeros((N_CORES, NB, NST), np.int64)
    for p in range(N_CORES):
        _, _, blk, st = percore[p]
        np.add.at(counts, (p, blk, st), 1)
    nchunks = (counts.max(axis=0) + 127) // 128          # [NB, NST]

    # canonical chunk emission order.  One call per (b, st) run; per-core
    # TRUE idx counts ride in a counts table read into the gather's
    # num_idxs register at runtime: padding below the count is idx 0
    # (transferred, masked out), trailing padding is negative (trimmed).
    chunk_meta = []   # (isb, st, b) per chunk
    calls = []        # (st, chunk_lo, n_chunks, [(b, run_lo, nch_b), ...])
    for isb in range(NSB):
        blocks = list(range(isb * SB, min((isb + 1) * SB, NB)))
        for st in range(NST):
            for b in blocks:
                run_lo = len(chunk_meta)
                for _ in range(int(nchunks[b, st])):
                    chunk_meta.append((isb, st, b))
                n = len(chunk_meta) - run_lo
                o = run_lo
                while n > 0:
                    take = min(n, MAX_CALL)
                    calls.append((st, o, take, [(b, run_lo, take)]))
                    o += take
                    n -= take
    NCH = len(chunk_meta)

    # first/last chunk index per (isb, b) for PSUM start/stop flags
    first = {}
    last = {}
    for ci, (isb, st, b) in enumerate(chunk_meta):
        key = (isb, b)
        if key not in first:
            first[key] = ci
        last[key] = ci

    # per-core edge arrays in padded chunk order + per-call true counts
    idx_all = np.full((N_CORES, NCH * 128), PAD_IDX, np.int16)
    doff_all = np.full((N_CORES, NCH * 128), 255.0, np.float32)
    cnt_all = np.zeros((N_CORES, len(calls)), np.int32)
    for p in range(N_CORES):
        s, d, blk, st = percore[p]
        runs = {}
        i = 0
        M = len(s)
        while i < M:
            k = (blk[i], st[i])
            j = i
            while j < M and blk[j] == k[0] and st[j] == k[1]:
                j += 1
            runs[k] = (i, j)
            i = j
        cursor = {k: v[0] for k, v in runs.items()}
        for ci, (isb, t, b) in enumerate(chunk_meta):
            base = ci * 128
            k = (b, t)
            if k in runs:
                lo = cursor[k]
                hi = min(lo + 128, runs[k][1])
                n = hi - lo
                cursor[k] = hi
                if n > 0:
                    idx_all[p, base:base + n] = s[lo:hi].astype(np.int16)
                    doff_all[p, base:base + n] = (d[lo:hi] - b * 128).astype(np.float32)
        for k, (lo, hi) in runs.items():
            assert cursor[k] == hi, "edge run not fully consumed"
        for ci_call, (t, lo, nch, runs_b) in enumerate(calls):
            cnt = 0
            for (b, run_lo, nch_b) in runs_b:
                c = int(counts[p, b, t])
                if c > 0:
                    cnt = max(cnt, int(np.clip(
                        (run_lo - lo) * 128 + c, 0, nch * 128)))
            cnt_all[p, ci_call] = cnt
            # padding below the runtime count must be >= 0 (transferred,
            # masked); only trailing padding may be negative (trimmed)
            seg = idx_all[p, lo * 128: lo * 128 + cnt]
            seg[seg < 0] = 0

    # gather-call wrapped idx layout: per call [16, n/16], concat on free axis
    idxw_cols = NCH * 8
    idx_wrapped = np.zeros((N_CORES, 16, idxw_cols), np.int16)
    col = 0
    call_cols = []
    for (t, lo, nch, _) in calls:
        n = nch * 128
        for p in range(N_CORES):
            seg = idx_all[p, lo * 128: lo * 128 + n]
            idx_wrapped[p, :, col:col + n // 16] = seg.reshape(-1, 16).T
        call_cols.append(col)
        col += n // 16
    assert col == idxw_cols

    groups = []
    for (t, lo, nch, _) in calls:
        g = lo
        while g < lo + nch:
            take = min(GS, lo + nch - g)
            groups.append((t, lo, g, take))  # (st, call_lo, group_lo, size)
            g += take

    # chunks per superblock (for per-sb mask loads)
    sb_c0 = [None] * NSB
    sb_nch = [0] * NSB
    for ci, (isb, st, b) in enumerate(chunk_meta):
        if sb_c0[isb] is None:
            sb_c0[isb] = ci
        sb_nch[isb] += 1
    max_chsb = max(sb_nch)

    return dict(Nl=Nl, NB=NB, NBP=NBP, NSB=NSB, NST=NST, ST_ROWS=ST_ROWS,
                SRD=SRD, NCH=NCH,
                chunk_meta=chunk_meta, calls=calls, call_cols=call_cols,
                groups=groups, first=first, last=last,
                idx_wrapped=idx_wrapped, doff_raw=doff_all, cnt_all=cnt_all,
                idxw_cols=idxw_cols, sb_c0=sb_c0, sb_nch=sb_nch,
                max_chsb=max_chsb)


# ----------------------------------------------------------------------------
# device program
# ----------------------------------------------------------------------------

def _split_multi_waits(nc):
    """walrus codegen only accepts one sync-wait per instruction; hoist any
    extra waits onto same-engine NOPs inserted right before the instruction."""
    n_id = 0
    for f in nc.m.functions:
        for blk in f.blocks:
            out = []
            for ins in blk.instructions:
                si = ins.sync_info
                if si is not None and len(si.on_wait) > 1 \
                        and ins.engine is not None:
                    waits = list(si.on_wait)
                    for w in waits[:-1]:
                        nop = mybir.InstNoOp(name=f"I-wsplit-{n_id}", ins=[],
                                             outs=[])
                        n_id += 1
                        nop.engine = ins.engine
                        nop.sync_info = mybir.SyncInfo(on_wait=[w],
                                                       on_update=[])
                        nc.inst_map[nop.name] = nop
                        out.append(nop)
                    ins.sync_info = mybir.SyncInfo(on_wait=[waits[-1]],
                                                   on_update=list(si.on_update))
                out.append(ins)
            blk.instructions = out

def _ap(base, *dims):
    """Rebuild AP with the same tensor/offset/partition dim, custom free dims."""
    return bass.AP(base.tensor, base.offset,
                   [list(base.ap[0])] + [list(d) for d in dims])


def _dram_ap(t, offset, pdim, *dims):
    """DRAM AP with custom partition dim and free dims (offset in elems)."""
    base = t.ap()
    return bass.AP(base.tensor, offset,
                   [list(pdim)] + [list(d) for d in dims])


def _build(meta, N, D, H):
    Nl, NB, NBP, NSB, NST = (meta["Nl"], meta["NB"], meta["NBP"], meta["NSB"],
                             meta["NST"])
    ST_ROWS, SRD = meta["ST_ROWS"], meta["SRD"]
    MAXCHSB = meta["max_chsb"]
    NCALLS = len(meta["calls"])
    HD = H * D            # 256
    RW = HD + H           # 260 elems (h | el), fp8 -> 260B used
    TW = 512              # fp8 table row stride: 512B (gather elem size)
    NOCT = (NB + OCT - 1) // OCT      # 13 projection/AG batches
    octs = [(j, min(OCT, NB - OCT * j)) for j in range(NOCT)]

    nc = bass.Bass("TRN2", target_bir_lowering=False, debug=False,
                   enable_asserts=False, num_devices=N_CORES,
                   num_swdge_queues=N_QUEUES,
                   dynamic_dma_scratch_size=32768)

    # ---- DRAM tensors
    xT_in = nc.dram_tensor("xT_in", [D, NBP * N_CORES], BF16,
                           kind="ExternalInput")
    xTl_in = nc.dram_tensor("xTl_in", [D, NBP], BF16, kind="ExternalInput")
    x_in = nc.dram_tensor("x_in", [128, NB, D], F32, kind="ExternalInput")
    c0_in = nc.dram_tensor("c0_in", [128, NB, D], F32, kind="ExternalInput")
    waug_in = nc.dram_tensor("waug_in", [D, RW], BF16, kind="ExternalInput")
    wr_in = nc.dram_tensor("wr_in", [D, H], BF16, kind="ExternalInput")
    ident_in = nc.dram_tensor("ident_in", [128, 128], BF16, kind="ExternalInput")
    scal_in = nc.dram_tensor("scal_in", [128, 4], F32, kind="ExternalInput")
    idx_in = nc.dram_tensor("idx_in", [128, meta["idxw_cols"]], I16,
                            kind="ExternalInput")
    cnt_in = nc.dram_tensor("cnt_in", [128, NCALLS], I32,
                            kind="ExternalInput")
    mt8_in = nc.dram_tensor("mt8_in", [128, meta["NCH"] * 128], F8,
                            kind="ExternalInput")
    m8_in = nc.dram_tensor("m8_in", [128, meta["NCH"] * 128], F8,
                           kind="ExternalInput")

    # double-buffered row table (step-1 projection writes overlap step-0
    # gather reads), split per subtable so gathers start as soon as their
    # subtable's projection slice has landed
    tables = [[nc.dram_tensor("table%d_%d" % (s, t), [ST_ROWS, TW], F8,
                              kind="Internal") for t in range(NST)]
              for s in range(STEP)]
    x_mid = nc.dram_tensor("x_mid", [128, NB, D], F32, kind="Internal")
    # per-octblock xT shards + AllGather outputs (chunked collective so
    # step-1 projection can start as soon as each octblock's AG lands)
    oct_cols = [128 * nblk for (_, nblk) in octs]
    xT_sh = [nc.dram_tensor("xT_sh%d" % k, [D, oct_cols[k]], BF16,
                            kind="Internal") for k in range(NOCT)]
    xT_ag = [nc.dram_tensor("xT_ag%d" % k, [D * N_CORES, oct_cols[k]], BF16,
                            kind="Internal", addr_space="Shared")
             for k in range(NOCT)]
    x_out = nc.dram_tensor("x_out", [Nl, D], F32, kind="ExternalOutput")

    from contextlib import ExitStack
    with tile.TileContext(nc) as tc, ExitStack() as es_:
        nc.gpsimd.load_library(library_config.mlp)
        # per-call runtime gather idx counts cycle through a few dedicated
        # Pool registers (allocated before tile pools exhaust the pool)
        cnt_regs = [nc.gpsimd.alloc_register("gidx%d" % i) for i in range(4)]
        cp = es_.enter_context(tc.tile_pool(name="consts", bufs=1))
        pools = {}
        for nm, bufs in [("xt", 6), ("rows", 8), ("mask", 2), ("m8p", 2),
                         ("rhs", 6), ("sm", 8), ("tbl", 4), ("blk", 6),
                         ("big", 4)]:
            pools[nm] = es_.enter_context(tc.tile_pool(name=nm, bufs=bufs))
        pA = es_.enter_context(tc.tile_pool(name="pacc", bufs=2, space="PSUM"))
        pB = es_.enter_context(tc.tile_pool(name="per8", bufs=1, space="PSUM"))
        pC = es_.enter_context(tc.tile_pool(name="ppj", bufs=3, space="PSUM"))

        # ---- load constants
        ident_t = cp.tile([128, 128], BF16, tag="ident")
        waug_t = cp.tile([D, RW], BF16, tag="waug")
        wr_t = cp.tile([D, H], BF16, tag="wr")
        scal_t = cp.tile([128, 4], F32, tag="scal")
        idx_t = cp.tile([128, meta["idxw_cols"]], I16, tag="idx")
        cnt_t = cp.tile([128, NCALLS], I32, tag="cnt")
        for t, s in [(ident_t, ident_in), (waug_t, waug_in), (wr_t, wr_in),
                     (scal_t, scal_in), (idx_t, idx_in), (cnt_t, cnt_in)]:
            nc.sync.dma_start(t[:], s.ap()[:])

        # zero-init rotating buffers whose stale contents are DMA'd or fed
        # to matmuls before every lane is overwritten (per-core gather trim
        # leaves pad slots stale; tbl junk columns are stored to DRAM)
        for _ in range(8):
            rz = pools["rows"].tile([128, MAX_CALL, TW], F8, tag="rows")
            nc.vector.memset(rz[:], 0)
        for _ in range(4):
            tz = pools["tbl"].tile([128, OCT, TW], F8, tag="tbl")
            nc.vector.memset(tz[:, :, RW:TW], 0)

        tails = {NB - 1: Nl - 128 * (NB - 1)}
        nidx_regs = {}

        def nidx_reg(n):
            if n not in nidx_regs:
                nidx_regs[n] = nc.gpsimd.to_reg(n)
            return nidx_regs[n]

        def proj_oct(step, r, j):
            """Project blocks j*8..j*8+nblk of core region r into the fp8
            row table (batched: 1 lhs load, nblk matmuls, 1 store)."""
            nblk = octs[j][1]
            w = 128 * nblk
            xt = pools["xt"].tile([D, 128 * OCT], BF16, tag="projlhs")
            if step == 0:
                g0 = NBP * r + 128 * OCT * j
                nc.sync.dma_start(xt[:, :w], xT_in.ap()[:, g0:g0 + w])
            else:
                nc.sync.dma_start(xt[:, :w],
                                  xT_ag[j].ap()[D * r:D * (r + 1), :])
            tb = pools["tbl"].tile([128, OCT, TW], F8, tag="tbl")
            for k in range(nblk):
                pp = pC.tile([128, RW], F32, tag="pj")
                nc.tensor.matmul(pp[:], xt[:, 128 * k:128 * (k + 1)],
                                 waug_t[:], start=True, stop=True)
                # in the step-0 prologue the DVE is idle; split the PSUM
                # eviction across scalar+vector so it isn't scalar-bound
                if step == 0 and k % 2 == 1:
                    nc.vector.tensor_copy(tb[:, k, 0:RW], pp[:])
                else:
                    nc.scalar.activation(tb[:, k, 0:RW], pp[:], AF.Copy)
            st_i = r // 2
            rb = NB * (r % 2) + OCT * j
            nc.sync.dma_start(
                _dram_ap(tables[step][st_i], rb * TW, [SRD * TW, 128],
                         [1, nblk * TW]),
                _ap(tb[:], [1, nblk * TW]))

        for step in range(STEP):
            # ---------------------------------------------- step-0 projection
            # (step-1 projection is emitted interleaved into step 0's
            # superblock loop, gated on the per-octblock AllGathers)
            if step == 0:
                for r in range(N_CORES):
                    for j in range(NOCT):
                        proj_oct(0, r, j)

            # ------------------------------------------------ gather + attn
            x_src = x_in if step == 0 else x_mid
            table = tables[step]
            call_i = 0
            group_i = 0
            for isb in range(NSB):
                blocks = list(range(isb * SB, min((isb + 1) * SB, NB)))
                nb = len(blocks)
                b0 = blocks[0]
                oc = isb // 4          # owning octblock (4 sbs per oct)
                oco = 256 * (isb % 4)  # column offset within octblock
                acc = pA.tile([128, SB, 512], F32, tag="acc")
                er8w = pB.tile([128, 512], F32, tag="er8w")
                x4 = pools["blk"].tile([128, SB, D], F32, tag="x4")
                c04 = pools["blk"].tile([128, SB, D], F32, tag="c04")
                nc.sync.dma_start(x4[:, :nb, :], x_src.ap()[:, b0:b0 + nb, :])
                nc.sync.dma_start(c04[:, :nb, :], c0_in.ap()[:, b0:b0 + nb, :])
                # x4p = (1-alpha) * x4 + c0
                x4p = pools["blk"].tile([128, SB, D], F32, tag="x4p")
                nc.vector.scalar_tensor_tensor(
                    x4p[:, :nb, :], x4[:, :nb, :], scal_t[:, 0:1],
                    c04[:, :nb, :], op0=ALU.mult, op1=ALU.add)
                # er_sb: batched lhs load for both blocks of the superblock
                xtb = pools["xt"].tile([D, SB * 128], BF16, tag="erlhs")
                if step == 0:
                    nc.sync.dma_start(
                        xtb[:, :nb * 128],
                        xTl_in.ap()[:, 128 * b0:128 * (b0 + nb)])
                else:
                    nc.sync.dma_start(
                        xtb[:, :nb * 128],
                        xT_sh[oc].ap()[:, oco:oco + nb * 128])
                er_sb = {}
                for j, b in enumerate(blocks):
                    nc.tensor.matmul(acc[:, j, 264:264 + H],
                                     xtb[:, 128 * j:128 * (j + 1)], wr_t[:],
                                     start=True, stop=True)
                    es = pools["sm"].tile([128, H], F8, tag="erblk%d" % j)
                    nc.scalar.activation(es[:], acc[:, j, 264:264 + H], AF.Copy)
                    er_sb[b] = es

                # per-superblock mask streams (one DMA each)
                sb_c0 = meta["sb_c0"][isb]
                chsb = meta["sb_nch"][isb]
                mt = pools["mask"].tile([128, MAXCHSB * 128], F8, tag="mt")
                nc.sync.dma_start(
                    mt[:, :chsb * 128],
                    mt8_in.ap()[:, sb_c0 * 128:(sb_c0 + chsb) * 128])
                m8 = pools["m8p"].tile([128, MAXCHSB, 128], F8, tag="m8")
                nc.sync.dma_start(
                    _ap(m8[:], [128, chsb], [1, 128]),
                    m8_in.ap()[:, sb_c0 * 128:(sb_c0 + chsb) * 128])

                # walk this superblock's calls/groups/chunks
                while call_i < len(meta["calls"]):
                    st, lo, nch, _ = meta["calls"][call_i]
                    if lo >= len(meta["chunk_meta"]) or \
                       meta["chunk_meta"][lo][0] != isb:
                        break
                    n = nch * 128
                    rows = pools["rows"].tile([128, MAX_CALL, TW], F8,
                                              tag="rows")
                    icol = meta["call_cols"][call_i]
                    rows_ap = _ap(rows[:], [TW, nch], [1, TW])
                    tbl_ap = table[st].ap()[:]
                    if not SKIP_GATHER:
                        if DYN_CNT:
                            cv = cnt_regs[call_i % len(cnt_regs)]
                            nc.gpsimd.reg_load(
                                cv, cnt_t[0:1, call_i:call_i + 1])
                        else:
                            cv = nidx_reg(n)
                        nc.gpsimd.dma_gather(
                            rows_ap, tbl_ap, idx_t[:, icol:icol + n // 16],
                            num_idxs=n, num_idxs_reg=cv, elem_size=TW,
                            single_packet=SINGLE_PACKET,
                            queue_num=call_i % N_QUEUES)
                    call_i += 1

                    while group_i < len(meta["groups"]):
                        gst, glo_call, g, gs = meta["groups"][group_i]
                        if glo_call != lo:
                            break
                        group_i += 1
                        cc0 = g - lo   # chunk offset within call
                        kk = g - sb_c0  # chunk offset within superblock
                        # er per edge via fp8 mask matmul, into a
                        # rotating window of the dedicated er8 PSUM bank
                        ew = 32 * (group_i % 16)
                        er8 = er8w[:, ew:ew + GS * H]
                        for k in range(gs):
                            ci = g + k
                            _, _, b = meta["chunk_meta"][ci]
                            nc.tensor.matmul(er8[:, H * k:H * (k + 1)],
                                             mt[:, 128 * (kk + k):
                                                128 * (kk + k + 1)],
                                             er_sb[b], start=True, stop=True,
                                             skip_group_check=True)
                        # t8 = er8 + el (gathered, fp8)
                        t8 = pools["sm"].tile([128, GS * H], BF16, tag="t8")
                        nc.vector.tensor_tensor(
                            t8[:, :gs * H], er8[:, :gs * H],
                            _ap(rows[:, cc0:cc0 + gs, HD:HD + H],
                                [TW, gs], [1, H]),
                            op=ALU.add)
                        lr8 = pools["sm"].tile([128, GS * H], BF16, tag="lr8")
                        nc.vector.scalar_tensor_tensor(
                            lr8[:, :gs * H], t8[:, :gs * H], NEG_SLOPE,
                            t8[:, :gs * H], op0=ALU.mult, op1=ALU.max)
                        rhs8 = pools["rhs"].tile([128, GS, RW], BF16, tag="rhs8")
                        nc.scalar.activation(
                            _ap(rhs8[:], [RW, gs], [1, H]),
                            _ap(lr8[:], [H, gs], [1, H]), AF.Exp)
                        # h is d-major (host permutes waug) so the weight
                        # broadcast is innermost-contiguous over heads
                        nc.vector.tensor_tensor(
                            _ap(rhs8[:, :, H:RW], [RW, gs], [H, D], [1, H]),
                            _ap(rows[:, cc0:cc0 + gs, 0:HD],
                                [TW, gs], [H, D], [1, H]),
                            _ap(rhs8[:], [RW, gs], [0, D], [1, H]),
                            op=ALU.mult)
                        for k in range(gs):
                            ci = g + k
                            _, _, b = meta["chunk_meta"][ci]
                            j = b - b0
                            nc.tensor.matmul(
                                acc[:, j, 0:RW], m8[:, kk + k, :],
                                rhs8[:, k, :],
                                start=(meta["first"][(isb, b)] == ci),
                                stop=(meta["last"][(isb, b)] == ci),
                                skip_group_check=True)

                # ---- superblock epilogue (batched over blocks)
                # smax = max(denom, eps) * H/alpha, so its reciprocal is the
                # final (alpha/H)/denom normalizer in one fewer DVE op
                smax = pools["sm"].tile([128, SB * H], F32, tag="smax")
                nc.vector.tensor_scalar(
                    _ap(smax[:], [H, nb], [1, H]),
                    _ap(acc[:], [512, nb], [1, H]),
                    1e-30, scal_t[:, 2:3], op0=ALU.max, op1=ALU.mult)
                srec2 = pools["sm"].tile([128, SB * H], BF16, tag="srec2")
                with nc.allow_low_precision(reason="bf16 softmax normalizer"):
                    nc.vector.reciprocal(srec2[:, :nb * H],
                                         smax[:, :nb * H])
                onb = pools["big"].tile([128, SB, D, H], BF16, tag="onb")
                nc.scalar.activation(
                    _ap(onb[:], [H * D, nb], [1, H * D]),
                    _ap(acc[:, :, H:RW], [512, nb], [1, H * D]), AF.Copy)
                # d-major layout: head index is innermost-contiguous, so the
                # normalize multiply and head reduce run in DVE 2x mode
                onorm = pools["big"].tile([128, SB, D, H], BF16, tag="onorm")
                nc.vector.tensor_tensor(
                    _ap(onorm[:], [H * D, nb], [H, D], [1, H]),
                    _ap(onb[:], [H * D, nb], [H, D], [1, H]),
                    _ap(srec2[:], [H, nb], [0, D], [1, H]),
                    op=ALU.mult)
                red = pools["blk"].tile([128, SB, D], BF16, tag="red")
                with nc.allow_low_precision(reason="4-way head mean in bf16"):
                    nc.vector.tensor_reduce(
                        _ap(red[:], [D, nb], [1, D]),
                        _ap(onorm[:], [H * D, nb], [H, D], [1, H]),
                        axis=mybir.AxisListType.X, op=ALU.add)
                xn = pools["blk"].tile([128, SB, D], F32, tag="xn")
                nc.vector.tensor_add(xn[:, :nb, :], x4p[:, :nb, :],
                                     red[:, :nb, :])
                if step < STEP - 1:
                    xnb = pools["blk"].tile([128, SB, D], BF16, tag="xnb")
                    nc.scalar.activation(xnb[:, :nb, :], xn[:, :nb, :],
                                         AF.Copy)
                    nc.sync.dma_start(x_mid.ap()[:, b0:b0 + nb, :],
                                      xn[:, :nb, :])
                    xts = pools["sm"].tile([D, SB, 128], BF16, tag="xts")
                    for j, b in enumerate(blocks):
                        tp = pC.tile([D, 128], BF16, tag="pj")
                        nc.tensor.transpose(tp[:], xnb[:, j, :], ident_t[:])
                        nc.scalar.activation(xts[:, j, :], tp[:], AF.Copy)
                    nc.sync.dma_start(
                        xT_sh[oc].ap()[:, oco:oco + nb * 128],
                        _ap(xts[:], [1, nb * 128]))
                    # octblock AllGather of the updated xT once its 4 sbs are
                    # done; then emit step-1 projection for a 2-octblock-
                    # earlier AG so engine FIFO heads never stall on an
                    # in-flight collective
                    if isb % 4 == 3 or isb == NSB - 1:
                        if not SKIP_COLL:
                            nc.gpsimd.collective_compute(
                                "AllGather", ALU.bypass,
                                replica_groups=[list(range(N_CORES))],
                                ins=[xT_sh[oc].ap()[:]],
                                outs=[xT_ag[oc].ap()[:]])
                        if oc - 2 >= 0:
                            for r in range(N_CORES):
                                proj_oct(step + 1, r, oc - 2)
                else:
                    for j, b in enumerate(blocks):
                        w = tails.get(b, 128)
                        nc.sync.dma_start(x_out.ap()[128 * b:128 * b + w, :],
                                          xn[:w, j, :])
            assert call_i == len(meta["calls"]) and \
                group_i == len(meta["groups"])

            if step < STEP - 1:
                for k in range(max(0, NOCT - 2), NOCT):
                    for r in range(N_CORES):
                        proj_oct(step + 1, r, k)

    _split_multi_waits(nc)
    lower_extended_insts(nc)
    return nc


# ----------------------------------------------------------------------------
# entry point
# ----------------------------------------------------------------------------

def kernel(x, x0, src, dst, W, attn_l, attn_r, alpha, lamda, **kw):
    global _last_results
    x = np.asarray(x, np.float32)
    x0 = np.asarray(x0, np.float32)
    src = np.asarray(src)
    dst = np.asarray(dst)
    W = np.asarray(W, np.float32)
    attn_l = np.asarray(attn_l, np.float32)
    attn_r = np.asarray(attn_r, np.float32)
    alpha_f = float(np.asarray(alpha))
    lamda_f = float(np.asarray(lamda))

    N, D = x.shape
    H = attn_l.shape[0]
    assert N % N_CORES == 0
    meta = _plan_and_arrays(src, dst, N)
    Nl, NB, NBP = meta["Nl"], meta["NB"], meta["NBP"]

    nc = _build(meta, N, D, H)

    # host-side weight prep.  The projection's h columns are permuted
    # d-major (h index innermost) so on-device head broadcasts/reductions
    # are innermost-contiguous (DVE 2x mode).
    W3 = W.reshape(D, H, D)
    WL = np.einsum("khd,hd->kh", W3, attn_l)
    WR = np.einsum("khd,hd->kh", W3, attn_r)
    W_dm = np.ascontiguousarray(W3.transpose(0, 2, 1)).reshape(D, H * D)
    waug = _bf(np.concatenate([W_dm, WL], axis=1))
    wr = _bf(WR)
    ident = _bf(np.eye(128, dtype=np.float32))
    scal = np.zeros((128, 4), np.float32)
    scal[:, 0] = 1.0 - alpha_f
    scal[:, 1] = alpha_f / H
    scal[:, 2] = H / alpha_f
    c0 = (alpha_f * lamda_f) * x0

    d_idx = np.arange(128, dtype=np.float32)
    # zero-padded per-core-region transposed x: [D, NBP*8]
    xTp = np.zeros((D, NBP * N_CORES), np.float32)
    for r in range(N_CORES):
        xTp[:, NBP * r:NBP * r + Nl] = x[Nl * r:Nl * (r + 1)].T
    xTp = _bf(xTp)
    in_maps = []
    for p in range(N_CORES):
        lo = p * Nl
        xl = np.zeros((NBP, D), np.float32)
        xl[:Nl] = x[lo:lo + Nl]
        c0l = np.zeros((NBP, D), np.float32)
        c0l[:Nl] = c0[lo:lo + Nl]
        # transposed multi-chunk one-hot mask: mt8[d, ci*128+e] =
        # (dst_off(ci, e) == d), fp8 {0,1}
        mt8 = _f8(meta["doff_raw"][p][None, :] == d_idx[:, None])
        # untransposed: m8[e, ci*128+d] = (dst_off(ci, e) == d)
        dd = meta["doff_raw"][p].reshape(-1, 128)
        m8h = _f8((dd[:, :, None] == d_idx[None, None, :])
                  .transpose(1, 0, 2).reshape(128, -1))
        cnt = np.zeros((128, len(meta["calls"])), np.int32)
        cnt[0] = meta["cnt_all"][p]
        in_maps.append({
            "xT_in": np.ascontiguousarray(xTp),
            "xTl_in": np.ascontiguousarray(_bf(xl.T)),
            "x_in": np.ascontiguousarray(
                xl.reshape(NB, 128, D).transpose(1, 0, 2)),
            "c0_in": np.ascontiguousarray(
                c0l.reshape(NB, 128, D).transpose(1, 0, 2)),
            "waug_in": waug, "wr_in": wr,
            "ident_in": ident,
            "scal_in": scal,
            "idx_in": np.ascontiguousarray(
                np.tile(meta["idx_wrapped"][p], (8, 1))),
            "cnt_in": cnt,
            "mt8_in": np.ascontiguousarray(mt8),
            "m8_in": np.ascontiguousarray(m8h),
        })

    trace = bool(int(os.environ.get("GAT_TRACE", "0")))
    res = run_bass_kernel_spmd(nc, in_maps, core_ids=list(range(N_CORES)),
                               trace=trace,
                               trace_cores=[0] if trace else None,
                               stitch_traces=False)
    _last_results = res
    out = np.concatenate([res.results[p]["x_out"] for p in range(N_CORES)],
                         axis=0)
    return out.astype(np.float32)


# revision 39
# speedup vs baseline: 1.2950x; 1.0042x over previous
"""GAT (graph attention) message-passing kernel for Trainium2, 8 NeuronCores.

Strategy (graph/data parallel, dst-sharded):
  - Nodes are partitioned across 8 cores by destination id (12500 each).
  - Edges are sharded by dst partition, sorted by (dst-block, src-subtable),
    and padded so every core runs an identical (SPMD) program.
  - Per step, every core projects ALL nodes (h = x @ [W | W@attn_l]) into an
    fp8 row table in its HBM ([h(256B) | el(4B) | pad] @ 512B stride).  The
    table rows use a permuted layout (node l -> row (l%128)*196 + l//128 per
    25088-row subtable) so an 8-block projection batch stores 8 consecutive
    512B rows per partition with ONE contiguous descriptor per partition;
    lhs loads cover 1024 contiguous xT columns.  This cuts the sync-engine
    (DMA descriptor-gen) time ~8x vs per-block DMAs.
  - Per edge chunk (128 edges), h[src] rows are indirect-gathered
    (gpsimd dma_gather, 4 SWDGE queues round robin).  The per-call idx count
    is a per-core RUNTIME register (value_load from a counts table), so each
    core only transfers its true edges; SPMD padding slots are trimmed.
  - Attention scores: er via host-precomputed transposed one-hot masks
    (fp8, streamed from HBM per superblock) matmul'd with er_sb on TensorE,
    el added on DVE; [softmax-denominator | weighted message sum] accumulate
    into per-dst-block PSUM with mask matmuls.
  - Block epilogue: normalize by the segment sum, head-mean, residual update.
  - Between the 2 conv steps, the updated x (transposed, bf16) is AllGathered
    across the 8 cores in 8-block (4-superblock) chunks.
"""

import os
import math
import numpy as np
import ml_dtypes

import concourse.bass as bass
import concourse.tile as tile
import concourse.mybir as mybir
from concourse import library_config
from concourse.library_overlay import lower_extended_insts
from concourse.bass_utils import run_bass_kernel_spmd

BF16 = mybir.dt.bfloat16
F32 = mybir.dt.float32
F8 = mybir.dt.float8e4
I16 = mybir.dt.int16
I32 = mybir.dt.int32
AF = mybir.ActivationFunctionType
ALU = mybir.AluOpType

NEG_SLOPE = 0.2
STEP = int(os.environ.get("GAT_STEPS", "2"))
SKIP_COLL = bool(int(os.environ.get("GAT_SKIP_COLL", "0")))
SKIP_GATHER = bool(int(os.environ.get("GAT_SKIP_GATHER", "0")))
N_QUEUES = int(os.environ.get("GAT_QUEUES", "4"))
SINGLE_PACKET = bool(int(os.environ.get("GAT_SINGLE_PACKET", "1")))
DYN_CNT = bool(int(os.environ.get("GAT_DYN_CNT", "1")))
N_CORES = 8
SB = 2            # blocks per superblock (PSUM accumulators alive at once)
OCT = 8           # blocks per projection/AllGather batch
MAX_CALL = int(os.environ.get("GAT_MAX_CALL", "8"))  # chunks per dma_gather call
GS = 8            # chunks per elementwise batch group
# With per-core runtime idx counts the padding must be NEGATIVE: the Q7
# ucode trims trailing negative idxs and the decode-side ring reservation
# uses the num_idxs register — both sides then agree on the descriptor
# count.  (Negative pads with a full static register, or zero pads with a
# trimmed register, desync the ring and hang the DMA engines.)
PAD_IDX = -1 if DYN_CNT else 0

_last_results = None  # BassKernelResults stash for test harness


def _bf(x):
    return np.asarray(x, np.float32).astype(ml_dtypes.bfloat16)


def _f8(x):
    return np.asarray(x, np.float32).astype(ml_dtypes.float8_e4m3fn)


# ----------------------------------------------------------------------------
# host-side preprocessing
# ----------------------------------------------------------------------------

def _plan_and_arrays(src, dst, N):
    """Shard/sort/pad edges; build the shared chunk plan and per-core arrays."""
    Nl = N // N_CORES                 # 12500
    NB = (Nl + 127) // 128            # 98
    NBP = NB * 128                    # 12544 padded per-core region
    NSB = (NB + SB - 1) // SB         # 49
    CPST = 2                          # core regions per subtable
    ST_ROWS = CPST * NBP              # 25088 = 196*128
    SRD = ST_ROWS // 128              # 196
    NST = N_CORES // CPST             # 4

    # permuted padded gather-row id for each global src node
    def rowof(s):
        npad = NBP * (s // Nl) + (s % Nl)
        l = npad % ST_ROWS
        return (l % 128) * SRD + l // 128, npad // ST_ROWS

    core = dst // Nl
    percore = []
    for p in range(N_CORES):
        sel = np.nonzero(core == p)[0]
        s = src[sel].astype(np.int64)
        d = (dst[sel] - p * Nl).astype(np.int64)
        blk = d >> 7
        row, st = rowof(s)
        order = np.lexsort((row, st, blk))
        percore.append((row[order], d[order], blk[order], st[order]))

    counts = np.zeros((N_CORES, NB, NST), np.int64)
    for p in range(N_CORES):
        _, _, blk, st = percore[p]
        np.add.at(counts, (p, blk, st), 1)
    nchunks = (counts.max(axis=0) + 127) // 128          # [NB, NST]

    # canonical chunk emission order.  One call per (b, st) run; per-core
    # TRUE idx counts ride in a counts table read into the gather's
    # num_idxs register at runtime: padding below the count is idx 0
    # (transferred, masked out), trailing padding is negative (trimmed).
    chunk_meta = []   # (isb, st, b) per chunk
    calls = []        # (st, chunk_lo, n_chunks, [(b, run_lo, nch_b), ...])
    for isb in range(NSB):
        blocks = list(range(isb * SB, min((isb + 1) * SB, NB)))
        for st in range(NST):
            for b in blocks:
                run_lo = len(chunk_meta)
                for _ in range(int(nchunks[b, st])):
                    chunk_meta.append((isb, st, b))
                n = len(chunk_meta) - run_lo
                o = run_lo
                while n > 0:
                    take = min(n, MAX_CALL)
                    calls.append((st, o, take, [(b, run_lo, take)]))
                    o += take
                    n -= take
    NCH = len(chunk_meta)

    # first/last chunk index per (isb, b) for PSUM start/stop flags
    first = {}
    last = {}
    for ci, (isb, st, b) in enumerate(chunk_meta):
        key = (isb, b)
        if key not in first:
            first[key] = ci
        last[key] = ci

    # per-core edge arrays in padded chunk order + per-call true counts
    idx_all = np.full((N_CORES, NCH * 128), PAD_IDX, np.int16)
    doff_all = np.full((N_CORES, NCH * 128), 255.0, np.float32)
    cnt_all = np.zeros((N_CORES, len(calls)), np.int32)
    for p in range(N_CORES):
        s, d, blk, st = percore[p]
        runs = {}
        i = 0
        M = len(s)
        while i < M:
            k = (blk[i], st[i])
            j = i
            while j < M and blk[j] == k[0] and st[j] == k[1]:
                j += 1
            runs[k] = (i, j)
            i = j
        cursor = {k: v[0] for k, v in runs.items()}
        for ci, (isb, t, b) in enumerate(chunk_meta):
            base = ci * 128
            k = (b, t)
            if k in runs:
                lo = cursor[k]
                hi = min(lo + 128, runs[k][1])
                n = hi - lo
                cursor[k] = hi
                if n > 0:
                    idx_all[p, base:base + n] = s[lo:hi].astype(np.int16)
                    doff_all[p, base:base + n] = (d[lo:hi] - b * 128).astype(np.float32)
        for k, (lo, hi) in runs.items():
            assert cursor[k] == hi, "edge run not fully consumed"
        for ci_call, (t, lo, nch, runs_b) in enumerate(calls):
            cnt = 0
            for (b, run_lo, nch_b) in runs_b:
                c = int(counts[p, b, t])
                if c > 0:
                    cnt = max(cnt, int(np.clip(
                        (run_lo - lo) * 128 + c, 0, nch * 128)))
            cnt_all[p, ci_call] = cnt
            # padding below the runtime count must be >= 0 (transferred,
            # masked); only trailing padding may be negative (trimmed)
            seg = idx_all[p, lo * 128: lo * 128 + cnt]
            seg[seg < 0] = 0

    # gather-call wrapped idx layout: per call [16, n/16], concat on free axis
    idxw_cols = NCH * 8
    idx_wrapped = np.zeros((N_CORES, 16, idxw_cols), np.int16)
    col = 0
    call_cols = []
    for (t, lo, nch, _) in calls:
        n = nch * 128
        for p in range(N_CORES):
            seg = idx_all[p, lo * 128: lo * 128 + n]
            idx_wrapped[p, :, col:col + n // 16] = seg.reshape(-1, 16).T
        call_cols.append(col)
        col += n // 16
    assert col == idxw_cols

    groups = []
    for (t, lo, nch, _) in calls:
        g = lo
        while g < lo + nch:
            take = min(GS, lo + nch - g)
            groups.append((t, lo, g, take))  # (st, call_lo, group_lo, size)
            g += take

    # chunks per superblock (for per-sb mask loads)
    sb_c0 = [None] * NSB
    sb_nch = [0] * NSB
    for ci, (isb, st, b) in enumerate(chunk_meta):
        if sb_c0[isb] is None:
            sb_c0[isb] = ci
        sb_nch[isb] += 1
    max_chsb = max(sb_nch)

    return dict(Nl=Nl, NB=NB, NBP=NBP, NSB=NSB, NST=NST, ST_ROWS=ST_ROWS,
                SRD=SRD, NCH=NCH,
                chunk_meta=chunk_meta, calls=calls, call_cols=call_cols,
                groups=groups, first=first, last=last,
                idx_wrapped=idx_wrapped, doff_raw=doff_all, cnt_all=cnt_all,
                idxw_cols=idxw_cols, sb_c0=sb_c0, sb_nch=sb_nch,
                max_chsb=max_chsb)


# ----------------------------------------------------------------------------
# device program
# ----------------------------------------------------------------------------

def _split_multi_waits(nc):
    """walrus codegen only accepts one sync-wait per instruction; hoist any
    extra waits onto same-engine NOPs inserted right before the instruction."""
    n_id = 0
    for f in nc.m.functions:
        for blk in f.blocks:
            out = []
            for ins in blk.instructions:
                si = ins.sync_info
                if si is not None and len(si.on_wait) > 1 \
                        and ins.engine is not None:
                    waits = list(si.on_wait)
                    for w in waits[:-1]:
                        nop = mybir.InstNoOp(name=f"I-wsplit-{n_id}", ins=[],
                                             outs=[])
                        n_id += 1
                        nop.engine = ins.engine
                        nop.sync_info = mybir.SyncInfo(on_wait=[w],
                                                       on_update=[])
                        nc.inst_map[nop.name] = nop
                        out.append(nop)
                    ins.sync_info = mybir.SyncInfo(on_wait=[waits[-1]],
                                                   on_update=list(si.on_update))
                out.append(ins)
            blk.instructions = out

def _ap(base, *dims):
    """Rebuild AP with the same tensor/offset/partition dim, custom free dims."""
    return bass.AP(base.tensor, base.offset,
                   [list(base.ap[0])] + [list(d) for d in dims])


def _dram_ap(t, offset, pdim, *dims):
    """DRAM AP with custom partition dim and free dims (offset in elems)."""
    base = t.ap()
    return bass.AP(base.tensor, offset,
                   [list(pdim)] + [list(d) for d in dims])


def _build(meta, N, D, H):
    Nl, NB, NBP, NSB, NST = (meta["Nl"], meta["NB"], meta["NBP"], meta["NSB"],
                             meta["NST"])
    ST_ROWS, SRD = meta["ST_ROWS"], meta["SRD"]
    MAXCHSB = meta["max_chsb"]
    NCALLS = len(meta["calls"])
    HD = H * D            # 256
    RW = HD + H           # 260 elems (h | el), fp8 -> 260B used
    TW = 512              # fp8 table row stride: 512B (gather elem size)
    NOCT = (NB + OCT - 1) // OCT      # 13 projection/AG batches
    octs = [(j, min(OCT, NB - OCT * j)) for j in range(NOCT)]

    nc = bass.Bass("TRN2", target_bir_lowering=False, debug=False,
                   enable_asserts=False, num_devices=N_CORES,
                   num_swdge_queues=N_QUEUES,
                   dynamic_dma_scratch_size=32768)

    # ---- DRAM tensors
    xT_in = nc.dram_tensor("xT_in", [D, NBP * N_CORES], BF16,
                           kind="ExternalInput")
    xTl_in = nc.dram_tensor("xTl_in", [D, NBP], BF16, kind="ExternalInput")
    x_in = nc.dram_tensor("x_in", [128, NB, D], F32, kind="ExternalInput")
    c0_in = nc.dram_tensor("c0_in", [128, NB, D], F32, kind="ExternalInput")
    waug_in = nc.dram_tensor("waug_in", [D, RW], BF16, kind="ExternalInput")
    wr_in = nc.dram_tensor("wr_in", [D, H], BF16, kind="ExternalInput")
    ident_in = nc.dram_tensor("ident_in", [128, 128], BF16, kind="ExternalInput")
    scal_in = nc.dram_tensor("scal_in", [128, 4], F32, kind="ExternalInput")
    idx_in = nc.dram_tensor("idx_in", [128, meta["idxw_cols"]], I16,
                            kind="ExternalInput")
    cnt_in = nc.dram_tensor("cnt_in", [128, NCALLS], I32,
                            kind="ExternalInput")
    mt8_in = nc.dram_tensor("mt8_in", [128, meta["NCH"] * 128], F8,
                            kind="ExternalInput")
    m8_in = nc.dram_tensor("m8_in", [128, meta["NCH"] * 128], F8,
                           kind="ExternalInput")

    # double-buffered row table (step-1 projection writes overlap step-0
    # gather reads), split per subtable so gathers start as soon as their
    # subtable's projection slice has landed
    tables = [[nc.dram_tensor("table%d_%d" % (s, t), [ST_ROWS, TW], F8,
                              kind="Internal") for t in range(NST)]
              for s in range(STEP)]
    x_mid = nc.dram_tensor("x_mid", [128, NB, D], F32, kind="Internal")
    # per-octblock xT shards + AllGather outputs (chunked collective so
    # step-1 projection can start as soon as each octblock's AG lands)
    oct_cols = [128 * nblk for (_, nblk) in octs]
    xT_sh = [nc.dram_tensor("xT_sh%d" % k, [D, oct_cols[k]], BF16,
                            kind="Internal") for k in range(NOCT)]
    xT_ag = [nc.dram_tensor("xT_ag%d" % k, [D * N_CORES, oct_cols[k]], BF16,
                            kind="Internal", addr_space="Shared")
             for k in range(NOCT)]
    x_out = nc.dram_tensor("x_out", [Nl, D], F32, kind="ExternalOutput")

    from contextlib import ExitStack
    with tile.TileContext(nc) as tc, ExitStack() as es_:
        nc.gpsimd.load_library(library_config.mlp)
        # per-call runtime gather idx counts cycle through a few dedicated
        # Pool registers (allocated before tile pools exhaust the pool)
        cnt_regs = [nc.gpsimd.alloc_register("gidx%d" % i) for i in range(4)]
        cp = es_.enter_context(tc.tile_pool(name="consts", bufs=1))
        pools = {}
        for nm, bufs in [("xt", 6), ("rows", 8), ("mask", 2), ("m8p", 2),
                         ("rhs", 6), ("sm", 8), ("tbl", 4), ("blk", 6),
                         ("big", 4)]:
            pools[nm] = es_.enter_context(tc.tile_pool(name=nm, bufs=bufs))
        pA = es_.enter_context(tc.tile_pool(name="pacc", bufs=2, space="PSUM"))
        pB = es_.enter_context(tc.tile_pool(name="per8", bufs=1, space="PSUM"))
        pC = es_.enter_context(tc.tile_pool(name="ppj", bufs=3, space="PSUM"))

        # ---- load constants
        ident_t = cp.tile([128, 128], BF16, tag="ident")
        waug_t = cp.tile([D, RW], BF16, tag="waug")
        wr_t = cp.tile([D, H], BF16, tag="wr")
        scal_t = cp.tile([128, 4], F32, tag="scal")
        idx_t = cp.tile([128, meta["idxw_cols"]], I16, tag="idx")
        cnt_t = cp.tile([128, NCALLS], I32, tag="cnt")
        for t, s in [(ident_t, ident_in), (waug_t, waug_in), (wr_t, wr_in),
                     (scal_t, scal_in), (idx_t, idx_in), (cnt_t, cnt_in)]:
            nc.sync.dma_start(t[:], s.ap()[:])

        # zero-init rotating buffers whose stale contents are DMA'd or fed
        # to matmuls before every lane is overwritten (per-core gather trim
        # leaves pad slots stale; tbl junk columns are stored to DRAM)
        for _ in range(8):
            rz = pools["rows"].tile([128, MAX_CALL, TW], F8, tag="rows")
            nc.vector.memset(rz[:], 0)
        for _ in range(4):
            tz = pools["tbl"].tile([128, OCT, TW], F8, tag="tbl")
            nc.vector.memset(tz[:, :, RW:TW], 0)

        tails = {NB - 1: Nl - 128 * (NB - 1)}
        nidx_regs = {}

        def nidx_reg(n):
            if n not in nidx_regs:
                nidx_regs[n] = nc.gpsimd.to_reg(n)
            return nidx_regs[n]

        def proj_oct(step, r, j):
            """Project blocks j*8..j*8+nblk of core region r into the fp8
            row table (batched: 1 lhs load, nblk matmuls, 1 store)."""
            nblk = octs[j][1]
            w = 128 * nblk
            xt = pools["xt"].tile([D, 128 * OCT], BF16, tag="projlhs")
            if step == 0:
                g0 = NBP * r + 128 * OCT * j
                nc.sync.dma_start(xt[:, :w], xT_in.ap()[:, g0:g0 + w])
            else:
                nc.sync.dma_start(xt[:, :w],
                                  xT_ag[j].ap()[D * r:D * (r + 1), :])
            tb = pools["tbl"].tile([128, OCT, TW], F8, tag="tbl")
            for k in range(nblk):
                pp = pC.tile([128, RW], F32, tag="pj")
                nc.tensor.matmul(pp[:], xt[:, 128 * k:128 * (k + 1)],
                                 waug_t[:], start=True, stop=True)
                # in the step-0 prologue the DVE is idle; split the PSUM
                # eviction across scalar+vector so it isn't scalar-bound
                if step == 0 and k % 2 == 1:
                    nc.vector.tensor_copy(tb[:, k, 0:RW], pp[:])
                else:
                    nc.scalar.activation(tb[:, k, 0:RW], pp[:], AF.Copy)
            st_i = r // 2
            rb = NB * (r % 2) + OCT * j
            nc.sync.dma_start(
                _dram_ap(tables[step][st_i], rb * TW, [SRD * TW, 128],
                         [1, nblk * TW]),
                _ap(tb[:], [1, nblk * TW]))

        for step in range(STEP):
            # ---------------------------------------------- step-0 projection
            # (step-1 projection is emitted interleaved into step 0's
            # superblock loop, gated on the per-octblock AllGathers)
            if step == 0:
                for r in range(N_CORES):
                    for j in range(NOCT):
                        proj_oct(0, r, j)

            # ------------------------------------------------ gather + attn
            x_src = x_in if step == 0 else x_mid
            table = tables[step]
            call_i = 0
            group_i = 0
            for isb in range(NSB):
                blocks = list(range(isb * SB, min((isb + 1) * SB, NB)))
                nb = len(blocks)
                b0 = blocks[0]
                oc = isb // 4          # owning octblock (4 sbs per oct)
                oco = 256 * (isb % 4)  # column offset within octblock
                acc = pA.tile([128, SB, 512], F32, tag="acc")
                er8w = pB.tile([128, 512], F32, tag="er8w")
                x4 = pools["blk"].tile([128, SB, D], F32, tag="x4")
                c04 = pools["blk"].tile([128, SB, D], F32, tag="c04")
                nc.sync.dma_start(x4[:, :nb, :], x_src.ap()[:, b0:b0 + nb, :])
                nc.sync.dma_start(c04[:, :nb, :], c0_in.ap()[:, b0:b0 + nb, :])
                # x4p = (1-alpha) * x4 + c0
                x4p = pools["blk"].tile([128, SB, D], F32, tag="x4p")
                nc.vector.scalar_tensor_tensor(
                    x4p[:, :nb, :], x4[:, :nb, :], scal_t[:, 0:1],
                    c04[:, :nb, :], op0=ALU.mult, op1=ALU.add)
                # er_sb: batched lhs load for both blocks of the superblock
                xtb = pools["xt"].tile([D, SB * 128], BF16, tag="erlhs")
                if step == 0:
                    nc.sync.dma_start(
                        xtb[:, :nb * 128],
                        xTl_in.ap()[:, 128 * b0:128 * (b0 + nb)])
                else:
                    nc.sync.dma_start(
                        xtb[:, :nb * 128],
                        xT_sh[oc].ap()[:, oco:oco + nb * 128])
                er_sb = {}
                for j, b in enumerate(blocks):
                    nc.tensor.matmul(acc[:, j, 264:264 + H],
                                     xtb[:, 128 * j:128 * (j + 1)], wr_t[:],
                                     start=True, stop=True)
                    es = pools["sm"].tile([128, H], F8, tag="erblk%d" % j)
                    nc.scalar.activation(es[:], acc[:, j, 264:264 + H], AF.Copy)
                    er_sb[b] = es

                # per-superblock mask streams (one DMA each)
                sb_c0 = meta["sb_c0"][isb]
                chsb = meta["sb_nch"][isb]
                mt = pools["mask"].tile([128, MAXCHSB * 128], F8, tag="mt")
                nc.sync.dma_start(
                    mt[:, :chsb * 128],
                    mt8_in.ap()[:, sb_c0 * 128:(sb_c0 + chsb) * 128])
                m8 = pools["m8p"].tile([128, MAXCHSB, 128], F8, tag="m8")
                nc.sync.dma_start(
                    _ap(m8[:], [128, chsb], [1, 128]),
                    m8_in.ap()[:, sb_c0 * 128:(sb_c0 + chsb) * 128])

                # walk this superblock's calls/groups/chunks
                while call_i < len(meta["calls"]):
                    st, lo, nch, _ = meta["calls"][call_i]
                    if lo >= len(meta["chunk_meta"]) or \
                       meta["chunk_meta"][lo][0] != isb:
                        break
                    n = nch * 128
                    rows = pools["rows"].tile([128, MAX_CALL, TW], F8,
                                              tag="rows")
                    icol = meta["call_cols"][call_i]
                    rows_ap = _ap(rows[:], [TW, nch], [1, TW])
                    tbl_ap = table[st].ap()[:]
                    if not SKIP_GATHER:
                        if DYN_CNT:
                            # one 4-register load covers 4 calls' counts,
                            # keeping the gather-critical gpsimd queue short
                            if call_i % 4 == 0:
                                hi = min(call_i + 4, len(meta["calls"]))
                                nc.gpsimd.reg_load(
                                    cnt_regs[:hi - call_i],
                                    cnt_t[0:1, call_i:hi])
                            cv = cnt_regs[call_i % 4]
                        else:
                            cv = nidx_reg(n)
                        nc.gpsimd.dma_gather(
                            rows_ap, tbl_ap, idx_t[:, icol:icol + n // 16],
                            num_idxs=n, num_idxs_reg=cv, elem_size=TW,
                            single_packet=SINGLE_PACKET,
                            queue_num=call_i % N_QUEUES)
                    call_i += 1

                    while group_i < len(meta["groups"]):
                        gst, glo_call, g, gs = meta["groups"][group_i]
                        if glo_call != lo:
                            break
                        group_i += 1
                        cc0 = g - lo   # chunk offset within call
                        kk = g - sb_c0  # chunk offset within superblock
                        # er per edge via fp8 mask matmul, into a
                        # rotating window of the dedicated er8 PSUM bank
                        ew = 32 * (group_i % 16)
                        er8 = er8w[:, ew:ew + GS * H]
                        for k in range(gs):
                            ci = g + k
                            _, _, b = meta["chunk_meta"][ci]
                            nc.tensor.matmul(er8[:, H * k:H * (k + 1)],
                                             mt[:, 128 * (kk + k):
                                                128 * (kk + k + 1)],
                                             er_sb[b], start=True, stop=True,
                                             skip_group_check=True)
                        # t8 = er8 + el (gathered, fp8)
                        t8 = pools["sm"].tile([128, GS * H], BF16, tag="t8")
                        nc.vector.tensor_tensor(
                            t8[:, :gs * H], er8[:, :gs * H],
                            _ap(rows[:, cc0:cc0 + gs, HD:HD + H],
                                [TW, gs], [1, H]),
                            op=ALU.add)
                        lr8 = pools["sm"].tile([128, GS * H], BF16, tag="lr8")
                        nc.vector.scalar_tensor_tensor(
                            lr8[:, :gs * H], t8[:, :gs * H], NEG_SLOPE,
                            t8[:, :gs * H], op0=ALU.mult, op1=ALU.max)
                        rhs8 = pools["rhs"].tile([128, GS, RW], BF16, tag="rhs8")
                        nc.scalar.activation(
                            _ap(rhs8[:], [RW, gs], [1, H]),
                            _ap(lr8[:], [H, gs], [1, H]), AF.Exp)
                        # h is d-major (host permutes waug) so the weight
                        # broadcast is innermost-contiguous over heads
                        nc.vector.tensor_tensor(
                            _ap(rhs8[:, :, H:RW], [RW, gs], [H, D], [1, H]),
                            _ap(rows[:, cc0:cc0 + gs, 0:HD],
                                [TW, gs], [H, D], [1, H]),
                            _ap(rhs8[:], [RW, gs], [0, D], [1, H]),
                            op=ALU.mult)
                        for k in range(gs):
                            ci = g + k
                            _, _, b = meta["chunk_meta"][ci]
                            j = b - b0
                            nc.tensor.matmul(
                                acc[:, j, 0:RW], m8[:, kk + k, :],
                                rhs8[:, k, :],
                                start=(meta["first"][(isb, b)] == ci),
                                stop=(meta["last"][(isb, b)] == ci),
                                skip_group_check=True)

                # ---- superblock epilogue (batched over blocks)
                # smax = max(denom, eps) * H/alpha, so its reciprocal is the
                # final (alpha/H)/denom normalizer in one fewer DVE op
                smax = pools["sm"].tile([128, SB * H], F32, tag="smax")
                nc.vector.tensor_scalar(
                    _ap(smax[:], [H, nb], [1, H]),
                    _ap(acc[:], [512, nb], [1, H]),
                    1e-30, scal_t[:, 2:3], op0=ALU.max, op1=ALU.mult)
                srec2 = pools["sm"].tile([128, SB * H], BF16, tag="srec2")
                with nc.allow_low_precision(reason="bf16 softmax normalizer"):
                    nc.vector.reciprocal(srec2[:, :nb * H],
                                         smax[:, :nb * H])
                onb = pools["big"].tile([128, SB, D, H], BF16, tag="onb")
                nc.scalar.activation(
                    _ap(onb[:], [H * D, nb], [1, H * D]),
                    _ap(acc[:, :, H:RW], [512, nb], [1, H * D]), AF.Copy)
                # d-major layout: head index is innermost-contiguous, so the
                # normalize multiply and head reduce run in DVE 2x mode
                onorm = pools["big"].tile([128, SB, D, H], BF16, tag="onorm")
                nc.vector.tensor_tensor(
                    _ap(onorm[:], [H * D, nb], [H, D], [1, H]),
                    _ap(onb[:], [H * D, nb], [H, D], [1, H]),
                    _ap(srec2[:], [H, nb], [0, D], [1, H]),
                    op=ALU.mult)
                red = pools["blk"].tile([128, SB, D], BF16, tag="red")
                with nc.allow_low_precision(reason="4-way head mean in bf16"):
                    nc.vector.tensor_reduce(
                        _ap(red[:], [D, nb], [1, D]),
                        _ap(onorm[:], [H * D, nb], [H, D], [1, H]),
                        axis=mybir.AxisListType.X, op=ALU.add)
                xn = pools["blk"].tile([128, SB, D], F32, tag="xn")
                nc.vector.tensor_add(xn[:, :nb, :], x4p[:, :nb, :],
                                     red[:, :nb, :])
                if step < STEP - 1:
                    xnb = pools["blk"].tile([128, SB, D], BF16, tag="xnb")
                    nc.scalar.activation(xnb[:, :nb, :], xn[:, :nb, :],
                                         AF.Copy)
                    nc.sync.dma_start(x_mid.ap()[:, b0:b0 + nb, :],
                                      xn[:, :nb, :])
                    xts = pools["sm"].tile([D, SB, 128], BF16, tag="xts")
                    for j, b in enumerate(blocks):
                        tp = pC.tile([D, 128], BF16, tag="pj")
                        nc.tensor.transpose(tp[:], xnb[:, j, :], ident_t[:])
                        nc.scalar.activation(xts[:, j, :], tp[:], AF.Copy)
                    nc.sync.dma_start(
                        xT_sh[oc].ap()[:, oco:oco + nb * 128],
                        _ap(xts[:], [1, nb * 128]))
                    # octblock AllGather of the updated xT once its 4 sbs are
                    # done; then emit step-1 projection for a 2-octblock-
                    # earlier AG so engine FIFO heads never stall on an
                    # in-flight collective
                    if isb % 4 == 3 or isb == NSB - 1:
                        if not SKIP_COLL:
                            nc.gpsimd.collective_compute(
                                "AllGather", ALU.bypass,
                                replica_groups=[list(range(N_CORES))],
                                ins=[xT_sh[oc].ap()[:]],
                                outs=[xT_ag[oc].ap()[:]])
                        if oc - 2 >= 0:
                            for r in range(N_CORES):
                                proj_oct(step + 1, r, oc - 2)
                else:
                    for j, b in enumerate(blocks):
                        w = tails.get(b, 128)
                        nc.sync.dma_start(x_out.ap()[128 * b:128 * b + w, :],
                                          xn[:w, j, :])
            assert call_i == len(meta["calls"]) and \
                group_i == len(meta["groups"])

            if step < STEP - 1:
                for k in range(max(0, NOCT - 2), NOCT):
                    for r in range(N_CORES):
                        proj_oct(step + 1, r, k)

    _split_multi_waits(nc)
    lower_extended_insts(nc)
    return nc


# ----------------------------------------------------------------------------
# entry point
# ----------------------------------------------------------------------------

def kernel(x, x0, src, dst, W, attn_l, attn_r, alpha, lamda, **kw):
    global _last_results
    x = np.asarray(x, np.float32)
    x0 = np.asarray(x0, np.float32)
    src = np.asarray(src)
    dst = np.asarray(dst)
    W = np.asarray(W, np.float32)
    attn_l = np.asarray(attn_l, np.float32)
    attn_r = np.asarray(attn_r, np.float32)
    alpha_f = float(np.asarray(alpha))
    lamda_f = float(np.asarray(lamda))

    N, D = x.shape
    H = attn_l.shape[0]
    assert N % N_CORES == 0
    meta = _plan_and_arrays(src, dst, N)
    Nl, NB, NBP = meta["Nl"], meta["NB"], meta["NBP"]

    nc = _build(meta, N, D, H)

    # host-side weight prep.  The projection's h columns are permuted
    # d-major (h index innermost) so on-device head broadcasts/reductions
    # are innermost-contiguous (DVE 2x mode).
    W3 = W.reshape(D, H, D)
    WL = np.einsum("khd,hd->kh", W3, attn_l)
    WR = np.einsum("khd,hd->kh", W3, attn_r)
    W_dm = np.ascontiguousarray(W3.transpose(0, 2, 1)).reshape(D, H * D)
    waug = _bf(np.concatenate([W_dm, WL], axis=1))
    wr = _bf(WR)
    ident = _bf(np.eye(128, dtype=np.float32))
    scal = np.zeros((128, 4), np.float32)
    scal[:, 0] = 1.0 - alpha_f
    scal[:, 1] = alpha_f / H
    scal[:, 2] = H / alpha_f
    c0 = (alpha_f * lamda_f) * x0

    d_idx = np.arange(128, dtype=np.float32)
    # zero-padded per-core-region transposed x: [D, NBP*8]
    xTp = np.zeros((D, NBP * N_CORES), np.float32)
    for r in range(N_CORES):
        xTp[:, NBP * r:NBP * r + Nl] = x[Nl * r:Nl * (r + 1)].T
    xTp = _bf(xTp)
    in_maps = []
    for p in range(N_CORES):
        lo = p * Nl
        xl = np.zeros((NBP, D), np.float32)
        xl[:Nl] = x[lo:lo + Nl]
        c0l = np.zeros((NBP, D), np.float32)
        c0l[:Nl] = c0[lo:lo + Nl]
        # transposed multi-chunk one-hot mask: mt8[d, ci*128+e] =
        # (dst_off(ci, e) == d), fp8 {0,1}
        mt8 = _f8(meta["doff_raw"][p][None, :] == d_idx[:, None])
        # untransposed: m8[e, ci*128+d] = (dst_off(ci, e) == d)
        dd = meta["doff_raw"][p].reshape(-1, 128)
        m8h = _f8((dd[:, :, None] == d_idx[None, None, :])
                  .transpose(1, 0, 2).reshape(128, -1))
        cnt = np.zeros((128, len(meta["calls"])), np.int32)
        cnt[0] = meta["cnt_all"][p]
        in_maps.append({
            "xT_in": np.ascontiguousarray(xTp),
            "xTl_in": np.ascontiguousarray(_bf(xl.T)),
            "x_in": np.ascontiguousarray(
                xl.reshape(NB, 128, D).transpose(1, 0, 2)),
            "c0_in": np.ascontiguousarray(
                c0l.reshape(NB, 128, D).transpose(1, 0, 2)),
            "waug_in": waug, "wr_in": wr,
            "ident_in": ident,
            "scal_in": scal,
            "idx_in": np.ascontiguousarray(
                np.tile(meta["idx_wrapped"][p], (8, 1))),
            "cnt_in": cnt,
            "mt8_in": np.ascontiguousarray(mt8),
            "m8_in": np.ascontiguousarray(m8h),
        })

    trace = bool(int(os.environ.get("GAT_TRACE", "0")))
    res = run_bass_kernel_spmd(nc, in_maps, core_ids=list(range(N_CORES)),
                               trace=trace,
                               trace_cores=[0] if trace else None,
                               stitch_traces=False)
    _last_results = res
    out = np.concatenate([res.results[p]["x_out"] for p in range(N_CORES)],
                         axis=0)
    return out.astype(np.float32)


# revision 42
# speedup vs baseline: 1.2998x; 1.0037x over previous
"""GAT (graph attention) message-passing kernel for Trainium2, 8 NeuronCores.

Strategy (graph/data parallel, dst-sharded):
  - Nodes are partitioned across 8 cores by destination id (12500 each).
  - Edges are sharded by dst partition, sorted by (dst-block, src-subtable),
    and padded so every core runs an identical (SPMD) program.
  - Per step, every core projects ALL nodes (h = x @ [W | W@attn_l]) into an
    fp8 row table in its HBM ([h(256B) | el(4B) | pad] @ 512B stride).  The
    table rows use a permuted layout (node l -> row (l%128)*196 + l//128 per
    25088-row subtable) so an 8-block projection batch stores 8 consecutive
    512B rows per partition with ONE contiguous descriptor per partition;
    lhs loads cover 1024 contiguous xT columns.  This cuts the sync-engine
    (DMA descriptor-gen) time ~8x vs per-block DMAs.
  - Per edge chunk (128 edges), h[src] rows are indirect-gathered
    (gpsimd dma_gather, 4 SWDGE queues round robin).  The per-call idx count
    is a per-core RUNTIME register (value_load from a counts table), so each
    core only transfers its true edges; SPMD padding slots are trimmed.
  - Attention scores: er via host-precomputed transposed one-hot masks
    (fp8, streamed from HBM per superblock) matmul'd with er_sb on TensorE,
    el added on DVE; [softmax-denominator | weighted message sum] accumulate
    into per-dst-block PSUM with mask matmuls.
  - Block epilogue: normalize by the segment sum, head-mean, residual update.
  - Between the 2 conv steps, the updated x (transposed, bf16) is AllGathered
    across the 8 cores in 8-block (4-superblock) chunks.
"""

import os
import math
import numpy as np
import ml_dtypes

import concourse.bass as bass
import concourse.tile as tile
import concourse.mybir as mybir
from concourse import library_config
from concourse.library_overlay import lower_extended_insts
from concourse.bass_utils import run_bass_kernel_spmd

BF16 = mybir.dt.bfloat16
F32 = mybir.dt.float32
F8 = mybir.dt.float8e4
I16 = mybir.dt.int16
I32 = mybir.dt.int32
AF = mybir.ActivationFunctionType
ALU = mybir.AluOpType

NEG_SLOPE = 0.2
STEP = int(os.environ.get("GAT_STEPS", "2"))
SKIP_COLL = bool(int(os.environ.get("GAT_SKIP_COLL", "0")))
SKIP_GATHER = bool(int(os.environ.get("GAT_SKIP_GATHER", "0")))
N_QUEUES = int(os.environ.get("GAT_QUEUES", "4"))
SINGLE_PACKET = bool(int(os.environ.get("GAT_SINGLE_PACKET", "1")))
DYN_CNT = bool(int(os.environ.get("GAT_DYN_CNT", "1")))
N_CORES = 8
SB = 2            # blocks per superblock (PSUM accumulators alive at once)
OCT = 8           # blocks per projection/AllGather batch
MAX_CALL = int(os.environ.get("GAT_MAX_CALL", "8"))  # chunks per dma_gather call
GS = 8            # chunks per elementwise batch group
# With per-core runtime idx counts the padding must be NEGATIVE: the Q7
# ucode trims trailing negative idxs and the decode-side ring reservation
# uses the num_idxs register — both sides then agree on the descriptor
# count.  (Negative pads with a full static register, or zero pads with a
# trimmed register, desync the ring and hang the DMA engines.)
PAD_IDX = -1 if DYN_CNT else 0

_last_results = None  # BassKernelResults stash for test harness


def _bf(x):
    return np.asarray(x, np.float32).astype(ml_dtypes.bfloat16)


def _f8(x):
    return np.asarray(x, np.float32).astype(ml_dtypes.float8_e4m3fn)


# ----------------------------------------------------------------------------
# host-side preprocessing
# ----------------------------------------------------------------------------

def _plan_and_arrays(src, dst, N):
    """Shard/sort/pad edges; build the shared chunk plan and per-core arrays."""
    Nl = N // N_CORES                 # 12500
    NB = (Nl + 127) // 128            # 98
    NBP = NB * 128                    # 12544 padded per-core region
    NSB = (NB + SB - 1) // SB         # 49
    CPST = 2                          # core regions per subtable
    ST_ROWS = CPST * NBP              # 25088 = 196*128
    SRD = ST_ROWS // 128              # 196
    NST = N_CORES // CPST             # 4

    # permuted padded gather-row id for each global src node
    def rowof(s):
        npad = NBP * (s // Nl) + (s % Nl)
        l = npad % ST_ROWS
        return (l % 128) * SRD + l // 128, npad // ST_ROWS

    core = dst // Nl
    percore = []
    for p in range(N_CORES):
        sel = np.nonzero(core == p)[0]
        s = src[sel].astype(np.int64)
        d = (dst[sel] - p * Nl).astype(np.int64)
        blk = d >> 7
        row, st = rowof(s)
        order = np.lexsort((row, st, blk))
        percore.append((row[order], d[order], blk[order], st[order]))

    counts = np.zeros((N_CORES, NB, NST), np.int64)
    for p in range(N_CORES):
        _, _, blk, st = percore[p]
        np.add.at(counts, (p, blk, st), 1)
    nchunks = (counts.max(axis=0) + 127) // 128          # [NB, NST]

    # canonical chunk emission order.  One call per (b, st) run; per-core
    # TRUE idx counts ride in a counts table read into the gather's
    # num_idxs register at runtime: padding below the count is idx 0
    # (transferred, masked out), trailing padding is negative (trimmed).
    chunk_meta = []   # (isb, st, b) per chunk
    calls = []        # (st, chunk_lo, n_chunks, [(b, run_lo, nch_b), ...])
    for isb in range(NSB):
        blocks = list(range(isb * SB, min((isb + 1) * SB, NB)))
        for st in range(NST):
            for b in blocks:
                run_lo = len(chunk_meta)
                for _ in range(int(nchunks[b, st])):
                    chunk_meta.append((isb, st, b))
                n = len(chunk_meta) - run_lo
                o = run_lo
                while n > 0:
                    take = min(n, MAX_CALL)
                    calls.append((st, o, take, [(b, run_lo, take)]))
                    o += take
                    n -= take
    NCH = len(chunk_meta)

    # first/last chunk index per (isb, b) for PSUM start/stop flags
    first = {}
    last = {}
    for ci, (isb, st, b) in enumerate(chunk_meta):
        key = (isb, b)
        if key not in first:
            first[key] = ci
        last[key] = ci

    # per-core edge arrays in padded chunk order + per-call true counts
    idx_all = np.full((N_CORES, NCH * 128), PAD_IDX, np.int16)
    doff_all = np.full((N_CORES, NCH * 128), 255.0, np.float32)
    cnt_all = np.zeros((N_CORES, len(calls)), np.int32)
    for p in range(N_CORES):
        s, d, blk, st = percore[p]
        runs = {}
        i = 0
        M = len(s)
        while i < M:
            k = (blk[i], st[i])
            j = i
            while j < M and blk[j] == k[0] and st[j] == k[1]:
                j += 1
            runs[k] = (i, j)
            i = j
        cursor = {k: v[0] for k, v in runs.items()}
        for ci, (isb, t, b) in enumerate(chunk_meta):
            base = ci * 128
            k = (b, t)
            if k in runs:
                lo = cursor[k]
                hi = min(lo + 128, runs[k][1])
                n = hi - lo
                cursor[k] = hi
                if n > 0:
                    idx_all[p, base:base + n] = s[lo:hi].astype(np.int16)
                    doff_all[p, base:base + n] = (d[lo:hi] - b * 128).astype(np.float32)
        for k, (lo, hi) in runs.items():
            assert cursor[k] == hi, "edge run not fully consumed"
        for ci_call, (t, lo, nch, runs_b) in enumerate(calls):
            cnt = 0
            for (b, run_lo, nch_b) in runs_b:
                c = int(counts[p, b, t])
                if c > 0:
                    cnt = max(cnt, int(np.clip(
                        (run_lo - lo) * 128 + c, 0, nch * 128)))
            cnt_all[p, ci_call] = cnt
            # padding below the runtime count must be >= 0 (transferred,
            # masked); only trailing padding may be negative (trimmed)
            seg = idx_all[p, lo * 128: lo * 128 + cnt]
            seg[seg < 0] = 0

    # gather-call wrapped idx layout: per call [16, n/16], concat on free axis
    idxw_cols = NCH * 8
    idx_wrapped = np.zeros((N_CORES, 16, idxw_cols), np.int16)
    col = 0
    call_cols = []
    for (t, lo, nch, _) in calls:
        n = nch * 128
        for p in range(N_CORES):
            seg = idx_all[p, lo * 128: lo * 128 + n]
            idx_wrapped[p, :, col:col + n // 16] = seg.reshape(-1, 16).T
        call_cols.append(col)
        col += n // 16
    assert col == idxw_cols

    groups = []
    for (t, lo, nch, _) in calls:
        g = lo
        while g < lo + nch:
            take = min(GS, lo + nch - g)
            groups.append((t, lo, g, take))  # (st, call_lo, group_lo, size)
            g += take

    # chunks per superblock (for per-sb mask loads)
    sb_c0 = [None] * NSB
    sb_nch = [0] * NSB
    for ci, (isb, st, b) in enumerate(chunk_meta):
        if sb_c0[isb] is None:
            sb_c0[isb] = ci
        sb_nch[isb] += 1
    max_chsb = max(sb_nch)

    return dict(Nl=Nl, NB=NB, NBP=NBP, NSB=NSB, NST=NST, ST_ROWS=ST_ROWS,
                SRD=SRD, NCH=NCH,
                chunk_meta=chunk_meta, calls=calls, call_cols=call_cols,
                groups=groups, first=first, last=last,
                idx_wrapped=idx_wrapped, doff_raw=doff_all, cnt_all=cnt_all,
                idxw_cols=idxw_cols, sb_c0=sb_c0, sb_nch=sb_nch,
                max_chsb=max_chsb)


# ----------------------------------------------------------------------------
# device program
# ----------------------------------------------------------------------------

def _split_multi_waits(nc):
    """walrus codegen only accepts one sync-wait per instruction; hoist any
    extra waits onto same-engine NOPs inserted right before the instruction."""
    n_id = 0
    for f in nc.m.functions:
        for blk in f.blocks:
            out = []
            for ins in blk.instructions:
                si = ins.sync_info
                if si is not None and len(si.on_wait) > 1 \
                        and ins.engine is not None:
                    waits = list(si.on_wait)
                    for w in waits[:-1]:
                        nop = mybir.InstNoOp(name=f"I-wsplit-{n_id}", ins=[],
                                             outs=[])
                        n_id += 1
                        nop.engine = ins.engine
                        nop.sync_info = mybir.SyncInfo(on_wait=[w],
                                                       on_update=[])
                        nc.inst_map[nop.name] = nop
                        out.append(nop)
                    ins.sync_info = mybir.SyncInfo(on_wait=[waits[-1]],
                                                   on_update=list(si.on_update))
                out.append(ins)
            blk.instructions = out

def _ap(base, *dims):
    """Rebuild AP with the same tensor/offset/partition dim, custom free dims."""
    return bass.AP(base.tensor, base.offset,
                   [list(base.ap[0])] + [list(d) for d in dims])


def _dram_ap(t, offset, pdim, *dims):
    """DRAM AP with custom partition dim and free dims (offset in elems)."""
    base = t.ap()
    return bass.AP(base.tensor, offset,
                   [list(pdim)] + [list(d) for d in dims])


def _build(meta, N, D, H):
    Nl, NB, NBP, NSB, NST = (meta["Nl"], meta["NB"], meta["NBP"], meta["NSB"],
                             meta["NST"])
    ST_ROWS, SRD = meta["ST_ROWS"], meta["SRD"]
    MAXCHSB = meta["max_chsb"]
    NCALLS = len(meta["calls"])
    HD = H * D            # 256
    RW = HD + H           # 260 elems (h | el), fp8 -> 260B used
    TW = 512              # fp8 table row stride: 512B (gather elem size)
    NOCT = (NB + OCT - 1) // OCT      # 13 projection/AG batches
    octs = [(j, min(OCT, NB - OCT * j)) for j in range(NOCT)]

    nc = bass.Bass("TRN2", target_bir_lowering=False, debug=False,
                   enable_asserts=False, num_devices=N_CORES,
                   num_swdge_queues=N_QUEUES,
                   dynamic_dma_scratch_size=32768)

    # ---- DRAM tensors
    xT_in = nc.dram_tensor("xT_in", [D, NBP * N_CORES], BF16,
                           kind="ExternalInput")
    xTl_in = nc.dram_tensor("xTl_in", [D, NBP], BF16, kind="ExternalInput")
    x_in = nc.dram_tensor("x_in", [128, NB, D], F32, kind="ExternalInput")
    c0_in = nc.dram_tensor("c0_in", [128, NB, D], F32, kind="ExternalInput")
    waug_in = nc.dram_tensor("waug_in", [D, RW], BF16, kind="ExternalInput")
    wr_in = nc.dram_tensor("wr_in", [D, H], BF16, kind="ExternalInput")
    ident_in = nc.dram_tensor("ident_in", [128, 128], BF16, kind="ExternalInput")
    scal_in = nc.dram_tensor("scal_in", [128, 4], F32, kind="ExternalInput")
    idx_in = nc.dram_tensor("idx_in", [128, meta["idxw_cols"]], I16,
                            kind="ExternalInput")
    cnt_in = nc.dram_tensor("cnt_in", [128, NCALLS], I32,
                            kind="ExternalInput")
    mt8_in = nc.dram_tensor("mt8_in", [128, meta["NCH"] * 128], F8,
                            kind="ExternalInput")
    m8_in = nc.dram_tensor("m8_in", [128, meta["NCH"] * 128], F8,
                           kind="ExternalInput")

    # double-buffered row table (step-1 projection writes overlap step-0
    # gather reads), split per subtable so gathers start as soon as their
    # subtable's projection slice has landed
    tables = [[nc.dram_tensor("table%d_%d" % (s, t), [ST_ROWS, TW], F8,
                              kind="Internal") for t in range(NST)]
              for s in range(STEP)]
    x_mid = nc.dram_tensor("x_mid", [128, NB, D], F32, kind="Internal")
    # per-octblock xT shards + AllGather outputs (chunked collective so
    # step-1 projection can start as soon as each octblock's AG lands)
    oct_cols = [128 * nblk for (_, nblk) in octs]
    xT_sh = [nc.dram_tensor("xT_sh%d" % k, [D, oct_cols[k]], BF16,
                            kind="Internal") for k in range(NOCT)]
    xT_ag = [nc.dram_tensor("xT_ag%d" % k, [D * N_CORES, oct_cols[k]], BF16,
                            kind="Internal", addr_space="Shared")
             for k in range(NOCT)]
    x_out = nc.dram_tensor("x_out", [Nl, D], F32, kind="ExternalOutput")

    from contextlib import ExitStack
    with tile.TileContext(nc) as tc, ExitStack() as es_:
        nc.gpsimd.load_library(library_config.mlp)
        # per-call runtime gather idx counts cycle through a few dedicated
        # Pool registers (allocated before tile pools exhaust the pool)
        cnt_regs = [nc.gpsimd.alloc_register("gidx%d" % i) for i in range(4)]
        cp = es_.enter_context(tc.tile_pool(name="consts", bufs=1))
        pools = {}
        for nm, bufs in [("xt", 6), ("rows", 10), ("mask", 3), ("m8p", 3),
                         ("rhs", 6), ("sm", 8), ("tbl", 4), ("blk", 6),
                         ("big", 4)]:
            pools[nm] = es_.enter_context(tc.tile_pool(name=nm, bufs=bufs))
        pA = es_.enter_context(tc.tile_pool(name="pacc", bufs=2, space="PSUM"))
        pB = es_.enter_context(tc.tile_pool(name="per8", bufs=1, space="PSUM"))
        pC = es_.enter_context(tc.tile_pool(name="ppj", bufs=3, space="PSUM"))

        # ---- load constants
        ident_t = cp.tile([128, 128], BF16, tag="ident")
        waug_t = cp.tile([D, RW], BF16, tag="waug")
        wr_t = cp.tile([D, H], BF16, tag="wr")
        scal_t = cp.tile([128, 4], F32, tag="scal")
        idx_t = cp.tile([128, meta["idxw_cols"]], I16, tag="idx")
        cnt_t = cp.tile([128, NCALLS], I32, tag="cnt")
        for t, s in [(ident_t, ident_in), (waug_t, waug_in), (wr_t, wr_in),
                     (scal_t, scal_in), (idx_t, idx_in), (cnt_t, cnt_in)]:
            nc.sync.dma_start(t[:], s.ap()[:])

        # zero-init rotating buffers whose stale contents are DMA'd or fed
        # to matmuls before every lane is overwritten (per-core gather trim
        # leaves pad slots stale; tbl junk columns are stored to DRAM)
        for _ in range(10):
            rz = pools["rows"].tile([128, MAX_CALL, TW], F8, tag="rows")
            nc.vector.memset(rz[:], 0)
        for _ in range(4):
            tz = pools["tbl"].tile([128, OCT, TW], F8, tag="tbl")
            nc.vector.memset(tz[:, :, RW:TW], 0)

        tails = {NB - 1: Nl - 128 * (NB - 1)}
        nidx_regs = {}

        def nidx_reg(n):
            if n not in nidx_regs:
                nidx_regs[n] = nc.gpsimd.to_reg(n)
            return nidx_regs[n]

        def proj_oct(step, r, j):
            """Project blocks j*8..j*8+nblk of core region r into the fp8
            row table (batched: 1 lhs load, nblk matmuls, 1 store)."""
            nblk = octs[j][1]
            w = 128 * nblk
            xt = pools["xt"].tile([D, 128 * OCT], BF16, tag="projlhs")
            if step == 0:
                g0 = NBP * r + 128 * OCT * j
                nc.sync.dma_start(xt[:, :w], xT_in.ap()[:, g0:g0 + w])
            else:
                nc.sync.dma_start(xt[:, :w],
                                  xT_ag[j].ap()[D * r:D * (r + 1), :])
            tb = pools["tbl"].tile([128, OCT, TW], F8, tag="tbl")
            for k in range(nblk):
                pp = pC.tile([128, RW], F32, tag="pj")
                nc.tensor.matmul(pp[:], xt[:, 128 * k:128 * (k + 1)],
                                 waug_t[:], start=True, stop=True)
                # in the step-0 prologue the DVE is idle; split the PSUM
                # eviction across scalar+vector so it isn't scalar-bound
                if step == 0 and k % 2 == 1:
                    nc.vector.tensor_copy(tb[:, k, 0:RW], pp[:])
                else:
                    nc.scalar.activation(tb[:, k, 0:RW], pp[:], AF.Copy)
            st_i = r // 2
            rb = NB * (r % 2) + OCT * j
            nc.sync.dma_start(
                _dram_ap(tables[step][st_i], rb * TW, [SRD * TW, 128],
                         [1, nblk * TW]),
                _ap(tb[:], [1, nblk * TW]))

        for step in range(STEP):
            # ---------------------------------------------- step-0 projection
            # (step-1 projection is emitted interleaved into step 0's
            # superblock loop, gated on the per-octblock AllGathers)
            if step == 0:
                for r in range(N_CORES):
                    for j in range(NOCT):
                        proj_oct(0, r, j)

            # ------------------------------------------------ gather + attn
            x_src = x_in if step == 0 else x_mid
            table = tables[step]
            call_i = 0
            group_i = 0
            for isb in range(NSB):
                blocks = list(range(isb * SB, min((isb + 1) * SB, NB)))
                nb = len(blocks)
                b0 = blocks[0]
                oc = isb // 4          # owning octblock (4 sbs per oct)
                oco = 256 * (isb % 4)  # column offset within octblock
                acc = pA.tile([128, SB, 512], F32, tag="acc")
                er8w = pB.tile([128, 512], F32, tag="er8w")
                x4 = pools["blk"].tile([128, SB, D], F32, tag="x4")
                c04 = pools["blk"].tile([128, SB, D], F32, tag="c04")
                nc.sync.dma_start(x4[:, :nb, :], x_src.ap()[:, b0:b0 + nb, :])
                nc.sync.dma_start(c04[:, :nb, :], c0_in.ap()[:, b0:b0 + nb, :])
                # x4p = (1-alpha) * x4 + c0
                x4p = pools["blk"].tile([128, SB, D], F32, tag="x4p")
                nc.vector.scalar_tensor_tensor(
                    x4p[:, :nb, :], x4[:, :nb, :], scal_t[:, 0:1],
                    c04[:, :nb, :], op0=ALU.mult, op1=ALU.add)
                # er_sb: batched lhs load for both blocks of the superblock
                xtb = pools["xt"].tile([D, SB * 128], BF16, tag="erlhs")
                if step == 0:
                    nc.sync.dma_start(
                        xtb[:, :nb * 128],
                        xTl_in.ap()[:, 128 * b0:128 * (b0 + nb)])
                else:
                    nc.sync.dma_start(
                        xtb[:, :nb * 128],
                        xT_sh[oc].ap()[:, oco:oco + nb * 128])
                er_sb = {}
                for j, b in enumerate(blocks):
                    nc.tensor.matmul(acc[:, j, 264:264 + H],
                                     xtb[:, 128 * j:128 * (j + 1)], wr_t[:],
                                     start=True, stop=True)
                    es = pools["sm"].tile([128, H], F8, tag="erblk%d" % j)
                    nc.scalar.activation(es[:], acc[:, j, 264:264 + H], AF.Copy)
                    er_sb[b] = es

                # per-superblock mask streams (one DMA each)
                sb_c0 = meta["sb_c0"][isb]
                chsb = meta["sb_nch"][isb]
                mt = pools["mask"].tile([128, MAXCHSB * 128], F8, tag="mt")
                nc.sync.dma_start(
                    mt[:, :chsb * 128],
                    mt8_in.ap()[:, sb_c0 * 128:(sb_c0 + chsb) * 128])
                m8 = pools["m8p"].tile([128, MAXCHSB, 128], F8, tag="m8")
                nc.sync.dma_start(
                    _ap(m8[:], [128, chsb], [1, 128]),
                    m8_in.ap()[:, sb_c0 * 128:(sb_c0 + chsb) * 128])

                # walk this superblock's calls/groups/chunks
                while call_i < len(meta["calls"]):
                    st, lo, nch, _ = meta["calls"][call_i]
                    if lo >= len(meta["chunk_meta"]) or \
                       meta["chunk_meta"][lo][0] != isb:
                        break
                    n = nch * 128
                    rows = pools["rows"].tile([128, MAX_CALL, TW], F8,
                                              tag="rows")
                    icol = meta["call_cols"][call_i]
                    rows_ap = _ap(rows[:], [TW, nch], [1, TW])
                    tbl_ap = table[st].ap()[:]
                    if not SKIP_GATHER:
                        if DYN_CNT:
                            # one 4-register load covers 4 calls' counts,
                            # keeping the gather-critical gpsimd queue short
                            if call_i % 4 == 0:
                                hi = min(call_i + 4, len(meta["calls"]))
                                nc.gpsimd.reg_load(
                                    cnt_regs[:hi - call_i],
                                    cnt_t[0:1, call_i:hi])
                            cv = cnt_regs[call_i % 4]
                        else:
                            cv = nidx_reg(n)
                        nc.gpsimd.dma_gather(
                            rows_ap, tbl_ap, idx_t[:, icol:icol + n // 16],
                            num_idxs=n, num_idxs_reg=cv, elem_size=TW,
                            single_packet=SINGLE_PACKET,
                            queue_num=call_i % N_QUEUES)
                    call_i += 1

                    while group_i < len(meta["groups"]):
                        gst, glo_call, g, gs = meta["groups"][group_i]
                        if glo_call != lo:
                            break
                        group_i += 1
                        cc0 = g - lo   # chunk offset within call
                        kk = g - sb_c0  # chunk offset within superblock
                        # er per edge via fp8 mask matmul, into a
                        # rotating window of the dedicated er8 PSUM bank
                        ew = 32 * (group_i % 16)
                        er8 = er8w[:, ew:ew + GS * H]
                        for k in range(gs):
                            ci = g + k
                            _, _, b = meta["chunk_meta"][ci]
                            nc.tensor.matmul(er8[:, H * k:H * (k + 1)],
                                             mt[:, 128 * (kk + k):
                                                128 * (kk + k + 1)],
                                             er_sb[b], start=True, stop=True,
                                             skip_group_check=True)
                        # t8 = er8 + el (gathered, fp8)
                        t8 = pools["sm"].tile([128, GS * H], BF16, tag="t8")
                        nc.vector.tensor_tensor(
                            t8[:, :gs * H], er8[:, :gs * H],
                            _ap(rows[:, cc0:cc0 + gs, HD:HD + H],
                                [TW, gs], [1, H]),
                            op=ALU.add)
                        lr8 = pools["sm"].tile([128, GS * H], BF16, tag="lr8")
                        nc.vector.scalar_tensor_tensor(
                            lr8[:, :gs * H], t8[:, :gs * H], NEG_SLOPE,
                            t8[:, :gs * H], op0=ALU.mult, op1=ALU.max)
                        rhs8 = pools["rhs"].tile([128, GS, RW], BF16, tag="rhs8")
                        nc.scalar.activation(
                            _ap(rhs8[:], [RW, gs], [1, H]),
                            _ap(lr8[:], [H, gs], [1, H]), AF.Exp)
                        # h is d-major (host permutes waug) so the weight
                        # broadcast is innermost-contiguous over heads
                        nc.vector.tensor_tensor(
                            _ap(rhs8[:, :, H:RW], [RW, gs], [H, D], [1, H]),
                            _ap(rows[:, cc0:cc0 + gs, 0:HD],
                                [TW, gs], [H, D], [1, H]),
                            _ap(rhs8[:], [RW, gs], [0, D], [1, H]),
                            op=ALU.mult)
                        for k in range(gs):
                            ci = g + k
                            _, _, b = meta["chunk_meta"][ci]
                            j = b - b0
                            nc.tensor.matmul(
                                acc[:, j, 0:RW], m8[:, kk + k, :],
                                rhs8[:, k, :],
                                start=(meta["first"][(isb, b)] == ci),
                                stop=(meta["last"][(isb, b)] == ci),
                                skip_group_check=True)

                # ---- superblock epilogue (batched over blocks)
                # smax = max(denom, eps) * H/alpha, so its reciprocal is the
                # final (alpha/H)/denom normalizer in one fewer DVE op
                smax = pools["sm"].tile([128, SB * H], F32, tag="smax")
                nc.vector.tensor_scalar(
                    _ap(smax[:], [H, nb], [1, H]),
                    _ap(acc[:], [512, nb], [1, H]),
                    1e-30, scal_t[:, 2:3], op0=ALU.max, op1=ALU.mult)
                srec2 = pools["sm"].tile([128, SB * H], BF16, tag="srec2")
                with nc.allow_low_precision(reason="bf16 softmax normalizer"):
                    nc.vector.reciprocal(srec2[:, :nb * H],
                                         smax[:, :nb * H])
                onb = pools["big"].tile([128, SB, D, H], BF16, tag="onb")
                nc.scalar.activation(
                    _ap(onb[:], [H * D, nb], [1, H * D]),
                    _ap(acc[:, :, H:RW], [512, nb], [1, H * D]), AF.Copy)
                # d-major layout: head index is innermost-contiguous, so the
                # normalize multiply and head reduce run in DVE 2x mode
                onorm = pools["big"].tile([128, SB, D, H], BF16, tag="onorm")
                nc.vector.tensor_tensor(
                    _ap(onorm[:], [H * D, nb], [H, D], [1, H]),
                    _ap(onb[:], [H * D, nb], [H, D], [1, H]),
                    _ap(srec2[:], [H, nb], [0, D], [1, H]),
                    op=ALU.mult)
                red = pools["blk"].tile([128, SB, D], BF16, tag="red")
                with nc.allow_low_precision(reason="4-way head mean in bf16"):
                    nc.vector.tensor_reduce(
                        _ap(red[:], [D, nb], [1, D]),
                        _ap(onorm[:], [H * D, nb], [H, D], [1, H]),
                        axis=mybir.AxisListType.X, op=ALU.add)
                xn = pools["blk"].tile([128, SB, D], F32, tag="xn")
                nc.vector.tensor_add(xn[:, :nb, :], x4p[:, :nb, :],
                                     red[:, :nb, :])
                if step < STEP - 1:
                    xnb = pools["blk"].tile([128, SB, D], BF16, tag="xnb")
                    nc.scalar.activation(xnb[:, :nb, :], xn[:, :nb, :],
                                         AF.Copy)
                    nc.sync.dma_start(x_mid.ap()[:, b0:b0 + nb, :],
                                      xn[:, :nb, :])
                    xts = pools["sm"].tile([D, SB, 128], BF16, tag="xts")
                    for j, b in enumerate(blocks):
                        tp = pC.tile([D, 128], BF16, tag="pj")
                        nc.tensor.transpose(tp[:], xnb[:, j, :], ident_t[:])
                        nc.scalar.activation(xts[:, j, :], tp[:], AF.Copy)
                    nc.sync.dma_start(
                        xT_sh[oc].ap()[:, oco:oco + nb * 128],
                        _ap(xts[:], [1, nb * 128]))
                    # octblock AllGather of the updated xT once its 4 sbs are
                    # done; then emit step-1 projection for a 2-octblock-
                    # earlier AG so engine FIFO heads never stall on an
                    # in-flight collective
                    if isb % 4 == 3 or isb == NSB - 1:
                        if not SKIP_COLL:
                            nc.gpsimd.collective_compute(
                                "AllGather", ALU.bypass,
                                replica_groups=[list(range(N_CORES))],
                                ins=[xT_sh[oc].ap()[:]],
                                outs=[xT_ag[oc].ap()[:]])
                        if oc - 2 >= 0:
                            for r in range(N_CORES):
                                proj_oct(step + 1, r, oc - 2)
                else:
                    for j, b in enumerate(blocks):
                        w = tails.get(b, 128)
                        nc.sync.dma_start(x_out.ap()[128 * b:128 * b + w, :],
                                          xn[:w, j, :])
            assert call_i == len(meta["calls"]) and \
                group_i == len(meta["groups"])

            if step < STEP - 1:
                for k in range(max(0, NOCT - 2), NOCT):
                    for r in range(N_CORES):
                        proj_oct(step + 1, r, k)

    _split_multi_waits(nc)
    lower_extended_insts(nc)
    return nc


# ----------------------------------------------------------------------------
# entry point
# ----------------------------------------------------------------------------

def kernel(x, x0, src, dst, W, attn_l, attn_r, alpha, lamda, **kw):
    global _last_results
    x = np.asarray(x, np.float32)
    x0 = np.asarray(x0, np.float32)
    src = np.asarray(src)
    dst = np.asarray(dst)
    W = np.asarray(W, np.float32)
    attn_l = np.asarray(attn_l, np.float32)
    attn_r = np.asarray(attn_r, np.float32)
    alpha_f = float(np.asarray(alpha))
    lamda_f = float(np.asarray(lamda))

    N, D = x.shape
    H = attn_l.shape[0]
    assert N % N_CORES == 0
    meta = _plan_and_arrays(src, dst, N)
    Nl, NB, NBP = meta["Nl"], meta["NB"], meta["NBP"]

    nc = _build(meta, N, D, H)

    # host-side weight prep.  The projection's h columns are permuted
    # d-major (h index innermost) so on-device head broadcasts/reductions
    # are innermost-contiguous (DVE 2x mode).
    W3 = W.reshape(D, H, D)
    WL = np.einsum("khd,hd->kh", W3, attn_l)
    WR = np.einsum("khd,hd->kh", W3, attn_r)
    W_dm = np.ascontiguousarray(W3.transpose(0, 2, 1)).reshape(D, H * D)
    waug = _bf(np.concatenate([W_dm, WL], axis=1))
    wr = _bf(WR)
    ident = _bf(np.eye(128, dtype=np.float32))
    scal = np.zeros((128, 4), np.float32)
    scal[:, 0] = 1.0 - alpha_f
    scal[:, 1] = alpha_f / H
    scal[:, 2] = H / alpha_f
    c0 = (alpha_f * lamda_f) * x0

    d_idx = np.arange(128, dtype=np.float32)
    # zero-padded per-core-region transposed x: [D, NBP*8]
    xTp = np.zeros((D, NBP * N_CORES), np.float32)
    for r in range(N_CORES):
        xTp[:, NBP * r:NBP * r + Nl] = x[Nl * r:Nl * (r + 1)].T
    xTp = _bf(xTp)
    in_maps = []
    for p in range(N_CORES):
        lo = p * Nl
        xl = np.zeros((NBP, D), np.float32)
        xl[:Nl] = x[lo:lo + Nl]
        c0l = np.zeros((NBP, D), np.float32)
        c0l[:Nl] = c0[lo:lo + Nl]
        # transposed multi-chunk one-hot mask: mt8[d, ci*128+e] =
        # (dst_off(ci, e) == d), fp8 {0,1}
        mt8 = _f8(meta["doff_raw"][p][None, :] == d_idx[:, None])
        # untransposed: m8[e, ci*128+d] = (dst_off(ci, e) == d)
        dd = meta["doff_raw"][p].reshape(-1, 128)
        m8h = _f8((dd[:, :, None] == d_idx[None, None, :])
                  .transpose(1, 0, 2).reshape(128, -1))
        cnt = np.zeros((128, len(meta["calls"])), np.int32)
        cnt[0] = meta["cnt_all"][p]
        in_maps.append({
            "xT_in": np.ascontiguousarray(xTp),
            "xTl_in": np.ascontiguousarray(_bf(xl.T)),
            "x_in": np.ascontiguousarray(
                xl.reshape(NB, 128, D).transpose(1, 0, 2)),
            "c0_in": np.ascontiguousarray(
                c0l.reshape(NB, 128, D).transpose(1, 0, 2)),
            "waug_in": waug, "wr_in": wr,
            "ident_in": ident,
            "scal_in": scal,
            "idx_in": np.ascontiguousarray(
                np.tile(meta["idx_wrapped"][p], (8, 1))),
            "cnt_in": cnt,
            "mt8_in": np.ascontiguousarray(mt8),
            "m8_in": np.ascontiguousarray(m8h),
        })

    trace = bool(int(os.environ.get("GAT_TRACE", "0")))
    res = run_bass_kernel_spmd(nc, in_maps, core_ids=list(range(N_CORES)),
                               trace=trace,
                               trace_cores=[0] if trace else None,
                               stitch_traces=False)
    _last_results = res
    out = np.concatenate([res.results[p]["x_out"] for p in range(N_CORES)],
                         axis=0)
    return out.astype(np.float32)


# revision 45
# speedup vs baseline: 1.3082x; 1.0065x over previous
"""GAT (graph attention) message-passing kernel for Trainium2, 8 NeuronCores.

Strategy (graph/data parallel, dst-sharded):
  - Nodes are partitioned across 8 cores by destination id (12500 each).
  - Edges are sharded by dst partition, sorted by (dst-block, src-subtable),
    and padded so every core runs an identical (SPMD) program.
  - Per step, every core projects ALL nodes (h = x @ [W | W@attn_l]) into an
    fp8 row table in its HBM ([h(256B) | el(4B) | pad] @ 512B stride).  The
    table rows use a permuted layout (node l -> row (l%128)*196 + l//128 per
    25088-row subtable) so an 8-block projection batch stores 8 consecutive
    512B rows per partition with ONE contiguous descriptor per partition;
    lhs loads cover 1024 contiguous xT columns.  This cuts the sync-engine
    (DMA descriptor-gen) time ~8x vs per-block DMAs.
  - Per edge chunk (128 edges), h[src] rows are indirect-gathered
    (gpsimd dma_gather, 4 SWDGE queues round robin).  The per-call idx count
    is a per-core RUNTIME register (value_load from a counts table), so each
    core only transfers its true edges; SPMD padding slots are trimmed.
  - Attention scores: er via host-precomputed transposed one-hot masks
    (fp8, streamed from HBM per superblock) matmul'd with er_sb on TensorE,
    el added on DVE; [softmax-denominator | weighted message sum] accumulate
    into per-dst-block PSUM with mask matmuls.
  - Block epilogue: normalize by the segment sum, head-mean, residual update.
  - Between the 2 conv steps, the updated x (transposed, bf16) is AllGathered
    across the 8 cores in 8-block (4-superblock) chunks.
"""

import os
import math
import numpy as np
import ml_dtypes

import concourse.bass as bass
import concourse.tile as tile
import concourse.mybir as mybir
from concourse import library_config
from concourse.library_overlay import lower_extended_insts
from concourse.bass_utils import run_bass_kernel_spmd

BF16 = mybir.dt.bfloat16
F32 = mybir.dt.float32
F8 = mybir.dt.float8e4
I16 = mybir.dt.int16
I32 = mybir.dt.int32
AF = mybir.ActivationFunctionType
ALU = mybir.AluOpType

NEG_SLOPE = 0.2
STEP = int(os.environ.get("GAT_STEPS", "2"))
SKIP_COLL = bool(int(os.environ.get("GAT_SKIP_COLL", "0")))
SKIP_GATHER = bool(int(os.environ.get("GAT_SKIP_GATHER", "0")))
N_QUEUES = int(os.environ.get("GAT_QUEUES", "4"))
SINGLE_PACKET = bool(int(os.environ.get("GAT_SINGLE_PACKET", "1")))
DYN_CNT = bool(int(os.environ.get("GAT_DYN_CNT", "1")))
N_CORES = 8
SB = 2            # blocks per superblock (PSUM accumulators alive at once)
OCT = 8           # blocks per projection/AllGather batch
MAX_CALL = int(os.environ.get("GAT_MAX_CALL", "8"))  # chunks per dma_gather call
GS = 8            # chunks per elementwise batch group
# With per-core runtime idx counts the padding must be NEGATIVE: the Q7
# ucode trims trailing negative idxs and the decode-side ring reservation
# uses the num_idxs register — both sides then agree on the descriptor
# count.  (Negative pads with a full static register, or zero pads with a
# trimmed register, desync the ring and hang the DMA engines.)
PAD_IDX = -1 if DYN_CNT else 0

_last_results = None  # BassKernelResults stash for test harness


def _bf(x):
    return np.asarray(x, np.float32).astype(ml_dtypes.bfloat16)


def _f8(x):
    return np.asarray(x, np.float32).astype(ml_dtypes.float8_e4m3fn)


# ----------------------------------------------------------------------------
# host-side preprocessing
# ----------------------------------------------------------------------------

def _plan_and_arrays(src, dst, N):
    """Shard/sort/pad edges; build the shared chunk plan and per-core arrays."""
    Nl = N // N_CORES                 # 12500
    NB = (Nl + 127) // 128            # 98
    NBP = NB * 128                    # 12544 padded per-core region
    NSB = (NB + SB - 1) // SB         # 49
    CPST = 2                          # core regions per subtable
    ST_ROWS = CPST * NBP              # 25088 = 196*128
    SRD = ST_ROWS // 128              # 196
    NST = N_CORES // CPST             # 4

    # permuted padded gather-row id for each global src node
    def rowof(s):
        npad = NBP * (s // Nl) + (s % Nl)
        l = npad % ST_ROWS
        return (l % 128) * SRD + l // 128, npad // ST_ROWS

    core = dst // Nl
    percore = []
    for p in range(N_CORES):
        sel = np.nonzero(core == p)[0]
        s = src[sel].astype(np.int64)
        d = (dst[sel] - p * Nl).astype(np.int64)
        blk = d >> 7
        row, st = rowof(s)
        order = np.lexsort((row, st, blk))
        percore.append((row[order], d[order], blk[order], st[order]))

    counts = np.zeros((N_CORES, NB, NST), np.int64)
    for p in range(N_CORES):
        _, _, blk, st = percore[p]
        np.add.at(counts, (p, blk, st), 1)
    nchunks = (counts.max(axis=0) + 127) // 128          # [NB, NST]

    # canonical chunk emission order.  One call per (b, st) run; per-core
    # TRUE idx counts ride in a counts table read into the gather's
    # num_idxs register at runtime: padding below the count is idx 0
    # (transferred, masked out), trailing padding is negative (trimmed).
    chunk_meta = []   # (isb, st, b) per chunk
    calls = []        # (st, chunk_lo, n_chunks, [(b, run_lo, nch_b), ...])
    for isb in range(NSB):
        blocks = list(range(isb * SB, min((isb + 1) * SB, NB)))
        for st in range(NST):
            for b in blocks:
                run_lo = len(chunk_meta)
                for _ in range(int(nchunks[b, st])):
                    chunk_meta.append((isb, st, b))
                n = len(chunk_meta) - run_lo
                o = run_lo
                while n > 0:
                    take = min(n, MAX_CALL)
                    calls.append((st, o, take, [(b, run_lo, take)]))
                    o += take
                    n -= take
    NCH = len(chunk_meta)

    # first/last chunk index per (isb, b) for PSUM start/stop flags
    first = {}
    last = {}
    for ci, (isb, st, b) in enumerate(chunk_meta):
        key = (isb, b)
        if key not in first:
            first[key] = ci
        last[key] = ci

    # per-core edge arrays in padded chunk order + per-call true counts
    idx_all = np.full((N_CORES, NCH * 128), PAD_IDX, np.int16)
    doff_all = np.full((N_CORES, NCH * 128), 255.0, np.float32)
    cnt_all = np.zeros((N_CORES, len(calls)), np.int32)
    for p in range(N_CORES):
        s, d, blk, st = percore[p]
        runs = {}
        i = 0
        M = len(s)
        while i < M:
            k = (blk[i], st[i])
            j = i
            while j < M and blk[j] == k[0] and st[j] == k[1]:
                j += 1
            runs[k] = (i, j)
            i = j
        cursor = {k: v[0] for k, v in runs.items()}
        for ci, (isb, t, b) in enumerate(chunk_meta):
            base = ci * 128
            k = (b, t)
            if k in runs:
                lo = cursor[k]
                hi = min(lo + 128, runs[k][1])
                n = hi - lo
                cursor[k] = hi
                if n > 0:
                    idx_all[p, base:base + n] = s[lo:hi].astype(np.int16)
                    doff_all[p, base:base + n] = (d[lo:hi] - b * 128).astype(np.float32)
        for k, (lo, hi) in runs.items():
            assert cursor[k] == hi, "edge run not fully consumed"
        for ci_call, (t, lo, nch, runs_b) in enumerate(calls):
            cnt = 0
            for (b, run_lo, nch_b) in runs_b:
                c = int(counts[p, b, t])
                if c > 0:
                    cnt = max(cnt, int(np.clip(
                        (run_lo - lo) * 128 + c, 0, nch * 128)))
            cnt_all[p, ci_call] = cnt
            # padding below the runtime count must be >= 0 (transferred,
            # masked); only trailing padding may be negative (trimmed)
            seg = idx_all[p, lo * 128: lo * 128 + cnt]
            seg[seg < 0] = 0

    # gather-call wrapped idx layout: per call [16, n/16], concat on free axis
    idxw_cols = NCH * 8
    idx_wrapped = np.zeros((N_CORES, 16, idxw_cols), np.int16)
    col = 0
    call_cols = []
    for (t, lo, nch, _) in calls:
        n = nch * 128
        for p in range(N_CORES):
            seg = idx_all[p, lo * 128: lo * 128 + n]
            idx_wrapped[p, :, col:col + n // 16] = seg.reshape(-1, 16).T
        call_cols.append(col)
        col += n // 16
    assert col == idxw_cols

    groups = []
    for (t, lo, nch, _) in calls:
        g = lo
        while g < lo + nch:
            take = min(GS, lo + nch - g)
            groups.append((t, lo, g, take))  # (st, call_lo, group_lo, size)
            g += take

    # chunks per superblock (for per-sb mask loads)
    sb_c0 = [None] * NSB
    sb_nch = [0] * NSB
    for ci, (isb, st, b) in enumerate(chunk_meta):
        if sb_c0[isb] is None:
            sb_c0[isb] = ci
        sb_nch[isb] += 1
    max_chsb = max(sb_nch)

    return dict(Nl=Nl, NB=NB, NBP=NBP, NSB=NSB, NST=NST, ST_ROWS=ST_ROWS,
                SRD=SRD, NCH=NCH,
                chunk_meta=chunk_meta, calls=calls, call_cols=call_cols,
                groups=groups, first=first, last=last,
                idx_wrapped=idx_wrapped, doff_raw=doff_all, cnt_all=cnt_all,
                idxw_cols=idxw_cols, sb_c0=sb_c0, sb_nch=sb_nch,
                max_chsb=max_chsb)


# ----------------------------------------------------------------------------
# device program
# ----------------------------------------------------------------------------

def _split_multi_waits(nc):
    """walrus codegen only accepts one sync-wait per instruction; hoist any
    extra waits onto same-engine NOPs inserted right before the instruction."""
    n_id = 0
    for f in nc.m.functions:
        for blk in f.blocks:
            out = []
            for ins in blk.instructions:
                si = ins.sync_info
                if si is not None and len(si.on_wait) > 1 \
                        and ins.engine is not None:
                    waits = list(si.on_wait)
                    for w in waits[:-1]:
                        nop = mybir.InstNoOp(name=f"I-wsplit-{n_id}", ins=[],
                                             outs=[])
                        n_id += 1
                        nop.engine = ins.engine
                        nop.sync_info = mybir.SyncInfo(on_wait=[w],
                                                       on_update=[])
                        nc.inst_map[nop.name] = nop
                        out.append(nop)
                    ins.sync_info = mybir.SyncInfo(on_wait=[waits[-1]],
                                                   on_update=list(si.on_update))
                out.append(ins)
            blk.instructions = out

def _ap(base, *dims):
    """Rebuild AP with the same tensor/offset/partition dim, custom free dims."""
    return bass.AP(base.tensor, base.offset,
                   [list(base.ap[0])] + [list(d) for d in dims])


def _dram_ap(t, offset, pdim, *dims):
    """DRAM AP with custom partition dim and free dims (offset in elems)."""
    base = t.ap()
    return bass.AP(base.tensor, offset,
                   [list(pdim)] + [list(d) for d in dims])


def _build(meta, N, D, H):
    Nl, NB, NBP, NSB, NST = (meta["Nl"], meta["NB"], meta["NBP"], meta["NSB"],
                             meta["NST"])
    ST_ROWS, SRD = meta["ST_ROWS"], meta["SRD"]
    MAXCHSB = meta["max_chsb"]
    NCALLS = len(meta["calls"])
    HD = H * D            # 256
    RW = HD + H           # 260 elems (h | el), fp8 -> 260B used
    TW = 512              # fp8 table row stride: 512B (gather elem size)
    NOCT = (NB + OCT - 1) // OCT      # 13 projection/AG batches
    octs = [(j, min(OCT, NB - OCT * j)) for j in range(NOCT)]

    nc = bass.Bass("TRN2", target_bir_lowering=False, debug=False,
                   enable_asserts=False, num_devices=N_CORES,
                   num_swdge_queues=N_QUEUES,
                   dynamic_dma_scratch_size=32768)

    # ---- DRAM tensors
    xT_in = nc.dram_tensor("xT_in", [D, NBP * N_CORES], BF16,
                           kind="ExternalInput")
    xTl_in = nc.dram_tensor("xTl_in", [D, NBP], BF16, kind="ExternalInput")
    x_in = nc.dram_tensor("x_in", [128, NB, D], F32, kind="ExternalInput")
    c0_in = nc.dram_tensor("c0_in", [128, NB, D], F32, kind="ExternalInput")
    waug_in = nc.dram_tensor("waug_in", [D, RW], BF16, kind="ExternalInput")
    wr_in = nc.dram_tensor("wr_in", [D, H], BF16, kind="ExternalInput")
    ident_in = nc.dram_tensor("ident_in", [128, 128], BF16, kind="ExternalInput")
    scal_in = nc.dram_tensor("scal_in", [128, 4], F32, kind="ExternalInput")
    idx_in = nc.dram_tensor("idx_in", [128, meta["idxw_cols"]], I16,
                            kind="ExternalInput")
    cnt_in = nc.dram_tensor("cnt_in", [128, NCALLS], I32,
                            kind="ExternalInput")
    mt8_in = nc.dram_tensor("mt8_in", [128, meta["NCH"] * 128], F8,
                            kind="ExternalInput")
    m8_in = nc.dram_tensor("m8_in", [128, meta["NCH"] * 128], F8,
                           kind="ExternalInput")

    # double-buffered row table (step-1 projection writes overlap step-0
    # gather reads), split per subtable so gathers start as soon as their
    # subtable's projection slice has landed
    tables = [[nc.dram_tensor("table%d_%d" % (s, t), [ST_ROWS, TW], F8,
                              kind="Internal") for t in range(NST)]
              for s in range(STEP)]
    x_mid = nc.dram_tensor("x_mid", [128, NB, D], F32, kind="Internal")
    # per-octblock xT shards + AllGather outputs (chunked collective so
    # step-1 projection can start as soon as each octblock's AG lands)
    oct_cols = [128 * nblk for (_, nblk) in octs]
    xT_sh = [nc.dram_tensor("xT_sh%d" % k, [D, oct_cols[k]], BF16,
                            kind="Internal") for k in range(NOCT)]
    xT_ag = [nc.dram_tensor("xT_ag%d" % k, [D * N_CORES, oct_cols[k]], BF16,
                            kind="Internal", addr_space="Shared")
             for k in range(NOCT)]
    x_out = nc.dram_tensor("x_out", [Nl, D], F32, kind="ExternalOutput")

    from contextlib import ExitStack
    with tile.TileContext(nc) as tc, ExitStack() as es_:
        nc.gpsimd.load_library(library_config.mlp)
        # per-call runtime gather idx counts cycle through a few dedicated
        # Pool registers (allocated before tile pools exhaust the pool)
        cnt_regs = [nc.gpsimd.alloc_register("gidx%d" % i) for i in range(4)]
        cp = es_.enter_context(tc.tile_pool(name="consts", bufs=1))
        pools = {}
        for nm, bufs in [("xt", 6), ("rows", 10), ("mask", 3), ("m8p", 3),
                         ("rhs", 6), ("sm", 8), ("tbl", 4), ("blk", 6),
                         ("big", 4)]:
            pools[nm] = es_.enter_context(tc.tile_pool(name=nm, bufs=bufs))
        pA = es_.enter_context(tc.tile_pool(name="pacc", bufs=2, space="PSUM"))
        pB = es_.enter_context(tc.tile_pool(name="per8", bufs=1, space="PSUM"))
        pC = es_.enter_context(tc.tile_pool(name="ppj", bufs=3, space="PSUM"))

        # ---- load constants
        ident_t = cp.tile([128, 128], BF16, tag="ident")
        waug_t = cp.tile([D, RW], BF16, tag="waug")
        wr_t = cp.tile([D, H], BF16, tag="wr")
        scal_t = cp.tile([128, 4], F32, tag="scal")
        idx_t = cp.tile([128, meta["idxw_cols"]], I16, tag="idx")
        cnt_t = cp.tile([128, NCALLS], I32, tag="cnt")
        for t, s in [(ident_t, ident_in), (waug_t, waug_in), (wr_t, wr_in),
                     (scal_t, scal_in), (idx_t, idx_in), (cnt_t, cnt_in)]:
            nc.sync.dma_start(t[:], s.ap()[:])

        # zero-init rotating buffers whose stale contents are DMA'd or fed
        # to matmuls before every lane is overwritten (per-core gather trim
        # leaves pad slots stale; tbl junk columns are stored to DRAM)
        for _ in range(10):
            rz = pools["rows"].tile([128, MAX_CALL, TW], F8, tag="rows")
            nc.vector.memset(rz[:], 0)
        for _ in range(4):
            tz = pools["tbl"].tile([128, OCT, TW], F8, tag="tbl")
            nc.vector.memset(tz[:, :, RW:TW], 0)

        tails = {NB - 1: Nl - 128 * (NB - 1)}
        nidx_regs = {}

        def nidx_reg(n):
            if n not in nidx_regs:
                nidx_regs[n] = nc.gpsimd.to_reg(n)
            return nidx_regs[n]

        def proj_oct(step, r, j):
            """Project blocks j*8..j*8+nblk of core region r into the fp8
            row table (batched: 1 lhs load, nblk matmuls, 1 store)."""
            nblk = octs[j][1]
            w = 128 * nblk
            xt = pools["xt"].tile([D, 128 * OCT], BF16, tag="projlhs")
            if step == 0:
                g0 = NBP * r + 128 * OCT * j
                nc.sync.dma_start(xt[:, :w], xT_in.ap()[:, g0:g0 + w])
            else:
                nc.sync.dma_start(xt[:, :w],
                                  xT_ag[j].ap()[D * r:D * (r + 1), :])
            tb = pools["tbl"].tile([128, OCT, TW], F8, tag="tbl")
            for k in range(nblk):
                pp = pC.tile([128, RW], F32, tag="pj")
                nc.tensor.matmul(pp[:], xt[:, 128 * k:128 * (k + 1)],
                                 waug_t[:], start=True, stop=True)
                # in the step-0 prologue the DVE is idle; split the PSUM
                # eviction across scalar+vector so it isn't scalar-bound
                if step == 0 and k % 2 == 1:
                    nc.vector.tensor_copy(tb[:, k, 0:RW], pp[:])
                else:
                    nc.scalar.activation(tb[:, k, 0:RW], pp[:], AF.Copy)
            st_i = r // 2
            rb = NB * (r % 2) + OCT * j
            nc.sync.dma_start(
                _dram_ap(tables[step][st_i], rb * TW, [SRD * TW, 128],
                         [1, nblk * TW]),
                _ap(tb[:], [1, nblk * TW]))

        for step in range(STEP):
            # ---------------------------------------------- step-0 projection
            # (step-1 projection is emitted interleaved into step 0's
            # superblock loop, gated on the per-octblock AllGathers)
            if step == 0:
                for r in range(N_CORES):
                    for j in range(NOCT):
                        proj_oct(0, r, j)

            # ------------------------------------------------ gather + attn
            x_src = x_in if step == 0 else x_mid
            table = tables[step]
            call_i = 0
            group_i = 0
            proj_pending = []   # interleaved next-step projection work
            for isb in range(NSB):
                blocks = list(range(isb * SB, min((isb + 1) * SB, NB)))
                nb = len(blocks)
                b0 = blocks[0]
                oc = isb // 4          # owning octblock (4 sbs per oct)
                oco = 256 * (isb % 4)  # column offset within octblock
                acc = pA.tile([128, SB, 512], F32, tag="acc")
                er8w = pB.tile([128, 512], F32, tag="er8w")
                x4 = pools["blk"].tile([128, SB, D], F32, tag="x4")
                c04 = pools["blk"].tile([128, SB, D], F32, tag="c04")
                nc.sync.dma_start(x4[:, :nb, :], x_src.ap()[:, b0:b0 + nb, :])
                nc.sync.dma_start(c04[:, :nb, :], c0_in.ap()[:, b0:b0 + nb, :])
                # x4p = (1-alpha) * x4 + c0
                x4p = pools["blk"].tile([128, SB, D], F32, tag="x4p")
                nc.vector.scalar_tensor_tensor(
                    x4p[:, :nb, :], x4[:, :nb, :], scal_t[:, 0:1],
                    c04[:, :nb, :], op0=ALU.mult, op1=ALU.add)
                # er_sb: batched lhs load for both blocks of the superblock
                xtb = pools["xt"].tile([D, SB * 128], BF16, tag="erlhs")
                if step == 0:
                    nc.sync.dma_start(
                        xtb[:, :nb * 128],
                        xTl_in.ap()[:, 128 * b0:128 * (b0 + nb)])
                else:
                    nc.sync.dma_start(
                        xtb[:, :nb * 128],
                        xT_sh[oc].ap()[:, oco:oco + nb * 128])
                er_sb = {}
                for j, b in enumerate(blocks):
                    nc.tensor.matmul(acc[:, j, 264:264 + H],
                                     xtb[:, 128 * j:128 * (j + 1)], wr_t[:],
                                     start=True, stop=True)
                    es = pools["sm"].tile([128, H], F8, tag="erblk%d" % j)
                    nc.scalar.activation(es[:], acc[:, j, 264:264 + H], AF.Copy)
                    er_sb[b] = es

                # per-superblock mask streams (one DMA each)
                sb_c0 = meta["sb_c0"][isb]
                chsb = meta["sb_nch"][isb]
                mt = pools["mask"].tile([128, MAXCHSB * 128], F8, tag="mt")
                nc.sync.dma_start(
                    mt[:, :chsb * 128],
                    mt8_in.ap()[:, sb_c0 * 128:(sb_c0 + chsb) * 128])
                m8 = pools["m8p"].tile([128, MAXCHSB, 128], F8, tag="m8")
                nc.sync.dma_start(
                    _ap(m8[:], [128, chsb], [1, 128]),
                    m8_in.ap()[:, sb_c0 * 128:(sb_c0 + chsb) * 128])

                # walk this superblock's calls/groups/chunks
                while call_i < len(meta["calls"]):
                    st, lo, nch, _ = meta["calls"][call_i]
                    if lo >= len(meta["chunk_meta"]) or \
                       meta["chunk_meta"][lo][0] != isb:
                        break
                    n = nch * 128
                    rows = pools["rows"].tile([128, MAX_CALL, TW], F8,
                                              tag="rows")
                    icol = meta["call_cols"][call_i]
                    rows_ap = _ap(rows[:], [TW, nch], [1, TW])
                    tbl_ap = table[st].ap()[:]
                    if not SKIP_GATHER:
                        if DYN_CNT:
                            # one 4-register load covers 4 calls' counts,
                            # keeping the gather-critical gpsimd queue short
                            if call_i % 4 == 0:
                                hi = min(call_i + 4, len(meta["calls"]))
                                nc.gpsimd.reg_load(
                                    cnt_regs[:hi - call_i],
                                    cnt_t[0:1, call_i:hi])
                            cv = cnt_regs[call_i % 4]
                        else:
                            cv = nidx_reg(n)
                        nc.gpsimd.dma_gather(
                            rows_ap, tbl_ap, idx_t[:, icol:icol + n // 16],
                            num_idxs=n, num_idxs_reg=cv, elem_size=TW,
                            single_packet=SINGLE_PACKET,
                            queue_num=call_i % N_QUEUES)
                    call_i += 1

                    while group_i < len(meta["groups"]):
                        gst, glo_call, g, gs = meta["groups"][group_i]
                        if glo_call != lo:
                            break
                        group_i += 1
                        cc0 = g - lo   # chunk offset within call
                        kk = g - sb_c0  # chunk offset within superblock
                        # er per edge via fp8 mask matmul, into a
                        # rotating window of the dedicated er8 PSUM bank
                        ew = 32 * (group_i % 16)
                        er8 = er8w[:, ew:ew + GS * H]
                        for k in range(gs):
                            ci = g + k
                            _, _, b = meta["chunk_meta"][ci]
                            nc.tensor.matmul(er8[:, H * k:H * (k + 1)],
                                             mt[:, 128 * (kk + k):
                                                128 * (kk + k + 1)],
                                             er_sb[b], start=True, stop=True,
                                             skip_group_check=True)
                        # t8 = er8 + el (gathered, fp8)
                        t8 = pools["sm"].tile([128, GS * H], BF16, tag="t8")
                        nc.vector.tensor_tensor(
                            t8[:, :gs * H], er8[:, :gs * H],
                            _ap(rows[:, cc0:cc0 + gs, HD:HD + H],
                                [TW, gs], [1, H]),
                            op=ALU.add)
                        lr8 = pools["sm"].tile([128, GS * H], BF16, tag="lr8")
                        nc.vector.scalar_tensor_tensor(
                            lr8[:, :gs * H], t8[:, :gs * H], NEG_SLOPE,
                            t8[:, :gs * H], op0=ALU.mult, op1=ALU.max)
                        rhs8 = pools["rhs"].tile([128, GS, RW], BF16, tag="rhs8")
                        nc.scalar.activation(
                            _ap(rhs8[:], [RW, gs], [1, H]),
                            _ap(lr8[:], [H, gs], [1, H]), AF.Exp)
                        # h is d-major (host permutes waug) so the weight
                        # broadcast is innermost-contiguous over heads
                        nc.vector.tensor_tensor(
                            _ap(rhs8[:, :, H:RW], [RW, gs], [H, D], [1, H]),
                            _ap(rows[:, cc0:cc0 + gs, 0:HD],
                                [TW, gs], [H, D], [1, H]),
                            _ap(rhs8[:], [RW, gs], [0, D], [1, H]),
                            op=ALU.mult)
                        for k in range(gs):
                            ci = g + k
                            _, _, b = meta["chunk_meta"][ci]
                            j = b - b0
                            nc.tensor.matmul(
                                acc[:, j, 0:RW], m8[:, kk + k, :],
                                rhs8[:, k, :],
                                start=(meta["first"][(isb, b)] == ci),
                                stop=(meta["last"][(isb, b)] == ci),
                                skip_group_check=True)

                # ---- superblock epilogue (batched over blocks)
                # smax = max(denom, eps) * H/alpha, so its reciprocal is the
                # final (alpha/H)/denom normalizer in one fewer DVE op
                smax = pools["sm"].tile([128, SB * H], F32, tag="smax")
                nc.vector.tensor_scalar(
                    _ap(smax[:], [H, nb], [1, H]),
                    _ap(acc[:], [512, nb], [1, H]),
                    1e-30, scal_t[:, 2:3], op0=ALU.max, op1=ALU.mult)
                srec2 = pools["sm"].tile([128, SB * H], BF16, tag="srec2")
                with nc.allow_low_precision(reason="bf16 softmax normalizer"):
                    nc.vector.reciprocal(srec2[:, :nb * H],
                                         smax[:, :nb * H])
                onb = pools["big"].tile([128, SB, D, H], BF16, tag="onb")
                nc.scalar.activation(
                    _ap(onb[:], [H * D, nb], [1, H * D]),
                    _ap(acc[:, :, H:RW], [512, nb], [1, H * D]), AF.Copy)
                # d-major layout: head index is innermost-contiguous, so the
                # normalize multiply and head reduce run in DVE 2x mode
                onorm = pools["big"].tile([128, SB, D, H], BF16, tag="onorm")
                nc.vector.tensor_tensor(
                    _ap(onorm[:], [H * D, nb], [H, D], [1, H]),
                    _ap(onb[:], [H * D, nb], [H, D], [1, H]),
                    _ap(srec2[:], [H, nb], [0, D], [1, H]),
                    op=ALU.mult)
                red = pools["blk"].tile([128, SB, D], BF16, tag="red")
                with nc.allow_low_precision(reason="4-way head mean in bf16"):
                    nc.vector.tensor_reduce(
                        _ap(red[:], [D, nb], [1, D]),
                        _ap(onorm[:], [H * D, nb], [H, D], [1, H]),
                        axis=mybir.AxisListType.X, op=ALU.add)
                xn = pools["blk"].tile([128, SB, D], F32, tag="xn")
                nc.vector.tensor_add(xn[:, :nb, :], x4p[:, :nb, :],
                                     red[:, :nb, :])
                if step < STEP - 1:
                    xnb = pools["blk"].tile([128, SB, D], BF16, tag="xnb")
                    nc.scalar.activation(xnb[:, :nb, :], xn[:, :nb, :],
                                         AF.Copy)
                    nc.sync.dma_start(x_mid.ap()[:, b0:b0 + nb, :],
                                      xn[:, :nb, :])
                    xts = pools["sm"].tile([D, SB, 128], BF16, tag="xts")
                    for j, b in enumerate(blocks):
                        tp = pC.tile([D, 128], BF16, tag="pj")
                        nc.tensor.transpose(tp[:], xnb[:, j, :], ident_t[:])
                        nc.scalar.activation(xts[:, j, :], tp[:], AF.Copy)
                    nc.sync.dma_start(
                        xT_sh[oc].ap()[:, oco:oco + nb * 128],
                        _ap(xts[:], [1, nb * 128]))
                    # octblock AllGather of the updated xT once its 4 sbs are
                    # done; queue step-1 projection for a 2-octblock-earlier
                    # AG so engine FIFO heads never stall on an in-flight
                    # collective.  The queued work is drained 2 regions per
                    # superblock — emitting all 8 at once bursts ~64 matmuls
                    # that delay the acc MMs and backpressure the gather.
                    if isb % 4 == 3 or isb == NSB - 1:
                        if not SKIP_COLL:
                            nc.gpsimd.collective_compute(
                                "AllGather", ALU.bypass,
                                replica_groups=[list(range(N_CORES))],
                                ins=[xT_sh[oc].ap()[:]],
                                outs=[xT_ag[oc].ap()[:]])
                        if oc - 2 >= 0:
                            proj_pending.extend(
                                (step + 1, r, oc - 2)
                                for r in range(N_CORES))
                    for _ in range(2):
                        if proj_pending:
                            proj_oct(*proj_pending.pop(0))
                else:
                    for j, b in enumerate(blocks):
                        w = tails.get(b, 128)
                        nc.sync.dma_start(x_out.ap()[128 * b:128 * b + w, :],
                                          xn[:w, j, :])
            assert call_i == len(meta["calls"]) and \
                group_i == len(meta["groups"])

            if step < STEP - 1:
                for args in proj_pending:
                    proj_oct(*args)
                for k in range(max(0, NOCT - 2), NOCT):
                    for r in range(N_CORES):
                        proj_oct(step + 1, r, k)

    _split_multi_waits(nc)
    lower_extended_insts(nc)
    return nc


# ----------------------------------------------------------------------------
# entry point
# ----------------------------------------------------------------------------

def kernel(x, x0, src, dst, W, attn_l, attn_r, alpha, lamda, **kw):
    global _last_results
    x = np.asarray(x, np.float32)
    x0 = np.asarray(x0, np.float32)
    src = np.asarray(src)
    dst = np.asarray(dst)
    W = np.asarray(W, np.float32)
    attn_l = np.asarray(attn_l, np.float32)
    attn_r = np.asarray(attn_r, np.float32)
    alpha_f = float(np.asarray(alpha))
    lamda_f = float(np.asarray(lamda))

    N, D = x.shape
    H = attn_l.shape[0]
    assert N % N_CORES == 0
    meta = _plan_and_arrays(src, dst, N)
    Nl, NB, NBP = meta["Nl"], meta["NB"], meta["NBP"]

    nc = _build(meta, N, D, H)

    # host-side weight prep.  The projection's h columns are permuted
    # d-major (h index innermost) so on-device head broadcasts/reductions
    # are innermost-contiguous (DVE 2x mode).
    W3 = W.reshape(D, H, D)
    WL = np.einsum("khd,hd->kh", W3, attn_l)
    WR = np.einsum("khd,hd->kh", W3, attn_r)
    W_dm = np.ascontiguousarray(W3.transpose(0, 2, 1)).reshape(D, H * D)
    waug = _bf(np.concatenate([W_dm, WL], axis=1))
    wr = _bf(WR)
    ident = _bf(np.eye(128, dtype=np.float32))
    scal = np.zeros((128, 4), np.float32)
    scal[:, 0] = 1.0 - alpha_f
    scal[:, 1] = alpha_f / H
    scal[:, 2] = H / alpha_f
    c0 = (alpha_f * lamda_f) * x0

    d_idx = np.arange(128, dtype=np.float32)
    # zero-padded per-core-region transposed x: [D, NBP*8]
    xTp = np.zeros((D, NBP * N_CORES), np.float32)
    for r in range(N_CORES):
        xTp[:, NBP * r:NBP * r + Nl] = x[Nl * r:Nl * (r + 1)].T
    xTp = _bf(xTp)
    in_maps = []
    for p in range(N_CORES):
        lo = p * Nl
        xl = np.zeros((NBP, D), np.float32)
        xl[:Nl] = x[lo:lo + Nl]
        c0l = np.zeros((NBP, D), np.float32)
        c0l[:Nl] = c0[lo:lo + Nl]
        # transposed multi-chunk one-hot mask: mt8[d, ci*128+e] =
        # (dst_off(ci, e) == d), fp8 {0,1}
        mt8 = _f8(meta["doff_raw"][p][None, :] == d_idx[:, None])
        # untransposed: m8[e, ci*128+d] = (dst_off(ci, e) == d)
        dd = meta["doff_raw"][p].reshape(-1, 128)
        m8h = _f8((dd[:, :, None] == d_idx[None, None, :])
                  .transpose(1, 0, 2).reshape(128, -1))
        cnt = np.zeros((128, len(meta["calls"])), np.int32)
        cnt[0] = meta["cnt_all"][p]
        in_maps.append({
            "xT_in": np.ascontiguousarray(xTp),
            "xTl_in": np.ascontiguousarray(_bf(xl.T)),
            "x_in": np.ascontiguousarray(
                xl.reshape(NB, 128, D).transpose(1, 0, 2)),
            "c0_in": np.ascontiguousarray(
                c0l.reshape(NB, 128, D).transpose(1, 0, 2)),
            "waug_in": waug, "wr_in": wr,
            "ident_in": ident,
            "scal_in": scal,
            "idx_in": np.ascontiguousarray(
                np.tile(meta["idx_wrapped"][p], (8, 1))),
            "cnt_in": cnt,
            "mt8_in": np.ascontiguousarray(mt8),
            "m8_in": np.ascontiguousarray(m8h),
        })

    trace = bool(int(os.environ.get("GAT_TRACE", "0")))
    res = run_bass_kernel_spmd(nc, in_maps, core_ids=list(range(N_CORES)),
                               trace=trace,
                               trace_cores=[0] if trace else None,
                               stitch_traces=False)
    _last_results = res
    out = np.concatenate([res.results[p]["x_out"] for p in range(N_CORES)],
                         axis=0)
    return out.astype(np.float32)
